# revision 1
# baseline (speedup 1.0000x reference)
"""Causal self-attention with RoPE on 8 trn2 NeuronCores.

Problem: B=2, T=2048, D=1024, H=16 heads, head_dim=64, fp32.
Sharding: core = b*4 + g  (data parallel over batch, tensor parallel over
head groups of 4). Each core computes its 4 heads' attention plus the
row-slice of the output projection; the host sums the 4 partial Y^T per
batch and transposes back.

Per-core dataflow (everything transposed so matmuls contract on partitions):
  xT (1024, 2048)  =  x[b].T                     [ExternalInput, f32r]
  QT/KT packs [128, 2048] (2 heads of 64 rows)   = Wq/Wk-slices^T @ xT
  RoPE: QT' = QT*cos + (R2 @ QT)*sin   (R2 = block-diag rotate-half matrix)
  V_aug [128, 16, 260]: V natural layout per key block, 4 heads x (64 dims
      + ones column) -> fused softmax denominator.
  S^T tile [keys 128, q 512] = KT'_h-slice^T @ QT'_h  (PE, K=64)
  P^T = exp(S^T * 0.125) (ACT), diag blocks masked by DVE mul.
  Oacc [65, 512] += V_aug_h-slice^T @ P^T  (row 64 = denominator)
  O^T = Oacc[0:64] * bcast(1/denom)   (PE ones-matmul broadcast + DVE mul)
  Y^T partial [1024, 2048] = Wp-slice^T @ O^T packs  -> DRAM out.

The three phases are emitted fused per t-chunk i: A(i) computes q/k/v for
chunk i, B(i) runs attention for query chunk i (needs only chunks <= i by
causality), C(i) projects B(i)'s output. This maximizes cross-engine
overlap (A is PE-bound, B is ACT/PE-bound, C is DVE/DMA-bound).
"""

import sys
import numpy as np

sys.path.insert(0, "/opt/trn_rl_repo")

B, T, D, H = 2, 2048, 1024, 16
HD = 64          # head dim
HPC = 4          # heads per core
NCORES = 8
ROPE_BASE = 10000.0

USE_F32R = True  # f32r matmuls: 4x PE throughput, ~1e-4 rel err

_PROGRAM = None  # cached compiled program


def _rope_tables_np():
    inv_freq = 1.0 / (ROPE_BASE ** (np.arange(0, HD, 2, dtype=np.float32) / np.float32(HD)))
    pos = np.arange(T, dtype=np.float32)
    freqs = np.outer(pos, inv_freq).astype(np.float32)          # (T, 32)
    emb = np.concatenate([freqs, freqs], axis=-1)               # (T, 64)
    cosT = np.cos(emb).T.astype(np.float32)                     # (64, T)
    sinT = np.sin(emb).T.astype(np.float32)
    cos2 = np.vstack([cosT, cosT]).copy()                       # (128, T) two heads
    sin2 = np.vstack([sinT, sinT]).copy()
    return cos2, sin2


def _r2_np():
    # qrot[d] = -q[d+32] (d<32) ; q[d-32] (d>=32), per 64-row block.
    # matmul computes out[d, t] = sum_k r2[k, d] q[k, t]
    r2 = np.zeros((128, 128), dtype=np.float32)
    for base in (0, 64):
        for d in range(32):
            r2[base + d + 32, base + d] = -1.0
            r2[base + d, base + d + 32] = 1.0
    return r2


def _masks_np():
    # tri[j, ql] = 1 if key j may attend query ql within a diagonal block
    j = np.arange(128)[:, None]
    ql = np.arange(128)[None, :]
    return (j <= ql).astype(np.float32)                         # [128, 128]


def build_program():
    import concourse.bass as bass
    import concourse.tile as tile
    from concourse import bacc, mybir
    from contextlib import ExitStack

    f32 = mybir.dt.float32
    fmm = mybir.dt.float32r if USE_F32R else mybir.dt.float32

    nc = bacc.Bacc(None, target_bir_lowering=False, debug=False)

    # xT pre-tiled on host: xTr[kc, tch, p, t] = x[b].T[kc*128+p, tch*512+t]
    # so each [128, 512] working tile is a contiguous 256 KB DMA.
    xT = nc.declare_dram_parameter("xT", [D // 128, T // 512, 128, 512], fmm, isOutput=False)
    wq = nc.declare_dram_parameter("wq", [D, 256], fmm, isOutput=False)
    wk = nc.declare_dram_parameter("wk", [D, 256], fmm, isOutput=False)
    wv = nc.declare_dram_parameter("wv", [D, 256], fmm, isOutput=False)
    wp = nc.declare_dram_parameter("wp", [256, D], fmm, isOutput=False)
    # yT tiled the same way: yTr[ech, tch, p, t] = yT_partial[ech*128+p, tch*512+t]
    yT = nc.declare_dram_parameter("yT", [8, T // 512, 128, 512], f32, isOutput=True)

    cos2_np, sin2_np = _rope_tables_np()
    cos_d = nc.inline_tensor(cos2_np, name="cos2")
    sin_d = nc.inline_tensor(sin2_np, name="sin2")
    r2_d = nc.inline_tensor(_r2_np(), name="r2")
    masks_d = nc.inline_tensor(_masks_np(), name="masks")

    NT = T // 512            # 4 t-chunks
    NJ = T // 128            # 16 key blocks
    KC = D // 128            # 8 contraction chunks

    with tile.TileContext(nc) as tc, ExitStack() as ctx:
        # --- persistent SBUF ---
        wts = ctx.enter_context(tc.tile_pool(name="wts", bufs=1))
        packs = ctx.enter_context(tc.tile_pool(name="packs", bufs=1))
        consts = ctx.enter_context(tc.tile_pool(name="consts", bufs=1))

        # --- working pools (xts first: its loads gate the first matmuls) ---
        xts = ctx.enter_context(tc.tile_pool(name="xts", bufs=2))

        wq_sb = wts.tile([128, KC, 256], fmm, tag="wq")
        wk_sb = wts.tile([128, KC, 256], fmm, tag="wk")
        wv_sb = wts.tile([128, KC, 256], fmm, tag="wv")
        wp_sb = wts.tile([128, 2, 1024], fmm, tag="wp")
        # DMA issue order = priority order: the first proj matmul needs wq+xt0,
        # so those go first on the (serial) DMA resource.
        xt_tiles = {}

        def prefetch_xt(tch):
            if tch >= NT or tch in xt_tiles:
                return
            xt = xts.tile([128, KC, 512], fmm, tag="xt", name=f"xt{tch}")
            # split so the first accumulation chunks start sooner
            nsplit = 4 if tch == 0 else 2
            step = KC // nsplit
            for s in range(nsplit):
                nc.sync.dma_start(
                    out=xt[:, s * step:(s + 1) * step, :],
                    in_=xT[s * step:(s + 1) * step, tch].rearrange("k p t -> p k t"))
            xt_tiles[tch] = xt

        cos_sb = consts.tile([128, T], f32, tag="cos")
        sin_sb = consts.tile([128, T], f32, tag="sin")
        r2f_sb = consts.tile([128, 128], f32, tag="r2f")
        masks_sb = consts.tile([128, 128], f32, tag="masks")
        # serial-DMA issue order == need order: wq+xt0 gate the first matmuls,
        # then wk (K proj), cos/sin/r2 (rope), wv (V proj), masks, wp (C).
        wq_r = wq.rearrange("(k p) d -> p k d", p=128)
        nc.sync.dma_start(out=wq_sb[:, :, 0:128], in_=wq_r[:, :, 0:128])
        prefetch_xt(0)
        nc.sync.dma_start(out=wq_sb[:, :, 128:256], in_=wq_r[:, :, 128:256])
        nc.sync.dma_start(out=wk_sb[:], in_=wk.rearrange("(k p) d -> p k d", p=128))
        nc.sync.dma_start(out=r2f_sb[:], in_=r2_d[:, :])
        nc.sync.dma_start(out=cos_sb[:], in_=cos_d[:, :])
        nc.sync.dma_start(out=sin_sb[:], in_=sin_d[:, :])
        nc.sync.dma_start(out=wv_sb[:], in_=wv.rearrange("(k p) d -> p k d", p=128))
        nc.sync.dma_start(out=masks_sb[:], in_=masks_d[:, :])
        nc.sync.dma_start(out=wp_sb[:], in_=wp.rearrange("(k p) d -> p k d", p=128))

        # rounded copies of constants that feed matmuls
        r2_sb = consts.tile([128, 128], fmm, tag="r2")
        nc.vector.tensor_copy(r2_sb[:], r2f_sb[:])
        ones64f = consts.tile([1, 64], f32, tag="ones64f")
        nc.vector.memset(ones64f[:], 1.0)
        ones64 = consts.tile([1, 64], fmm, tag="ones64")
        nc.vector.tensor_copy(ones64[:], ones64f[:])
        onescol = consts.tile([128, NJ, HPC], f32, tag="onescol")
        nc.vector.memset(onescol[:], 1.0)

        qt_sb = [packs.tile([128, T], fmm, tag=f"qt{p}", name=f"qt{p}") for p in range(2)]
        kt_sb = [packs.tile([128, T], fmm, tag=f"kt{p}", name=f"kt{p}") for p in range(2)]
        ot_sb = [packs.tile([128, T], fmm, tag=f"ot{p}", name=f"ot{p}") for p in range(2)]
        v_sb = packs.tile([128, NJ, HPC * (HD + 1)], fmm, tag="vaug")

        # ones columns of v_aug (fused softmax denominator)
        nc.vector.tensor_copy(
            v_sb[:].rearrange("p j (h c) -> p j h c", h=HPC)[:, :, :, HD:HD + 1],
            onescol[:],
        )

        # --- working pools ---
        tmps = ctx.enter_context(tc.tile_pool(name="tmps", bufs=3))
        pts = ctx.enter_context(tc.tile_pool(name="pts", bufs=14))
        outs = ctx.enter_context(tc.tile_pool(name="outs", bufs=3))
        smalls = ctx.enter_context(tc.tile_pool(name="smalls", bufs=2))

        # 8 PSUM banks: big(4) shared by A's qk accum and B's st tiles,
        # aux(2) for rot/v/bcast, o(2) for oacc pairs and C's proj psum.
        psBig = ctx.enter_context(tc.tile_pool(name="psBig", bufs=4, space="PSUM"))
        psAUX = ctx.enter_context(tc.tile_pool(name="psAUX", bufs=2, space="PSUM"))
        psO = ctx.enter_context(tc.tile_pool(name="psO", bufs=2, space="PSUM"))

        def emit_rope(tch, acc, raw, dst, p):
            ts = slice(tch * 512, (tch + 1) * 512)
            rot = psAUX.tile([128, 512], f32, tag="aux", name="rot")
            nc.tensor.matmul(rot[:], r2_sb[:], raw[:], start=True, stop=True)
            tc_t = tmps.tile([128, 512], f32, tag="tc", name="tc_t")
            nc.vector.tensor_mul(tc_t[:], acc[:], cos_sb[:, ts])
            ts_t = tmps.tile([128, 512], f32, tag="ts", name="ts_t")
            nc.vector.tensor_mul(ts_t[:], rot[:], sin_sb[:, ts])
            nc.vector.tensor_add(dst[p][:, ts], tc_t[:], ts_t[:])

        def attn_stream(qi, p, oaccs, pending, kj_lo, kj_hi, kj_max):
            """st -> exp -> (mask) -> O for kj in [kj_lo, kj_hi), software-
            pipelined by one step so the PE never queues behind its own exp."""
            def emit_o(kj, hh, pt, lo):
                hl = 2 * p + hh
                nc.tensor.matmul(
                    oaccs[hh][0:65, lo:512],
                    v_sb[:, kj, 65 * hl:65 * hl + 65],
                    pt[:, lo:512],
                    start=(kj == 0), stop=(kj == kj_max - 1),
                )

            for kj in range(kj_lo, kj_hi):
                r = kj - 4 * qi          # >= 0 on the causal diagonal
                lo = max(r, 0) * 128     # first valid column
                for hh in range(2):
                    off = 64 * hh
                    st = psBig.tile([128, 512], f32, tag="big", name="st")
                    nc.tensor.matmul(
                        st[:, lo:512],
                        kt_sb[p][off:off + 64, kj * 128:(kj + 1) * 128],
                        qt_sb[p][off:off + 64, qi * 512 + lo:(qi + 1) * 512],
                        start=True, stop=True,
                    )
                    pt = pts.tile([128, 512], fmm, tag="pt", name="pt")
                    nc.scalar.activation(
                        pt[:, lo:512], st[:, lo:512],
                        mybir.ActivationFunctionType.Exp, scale=0.125,
                    )
                    if r >= 0:
                        nc.vector.tensor_mul(
                            pt[:, lo:lo + 128], pt[:, lo:lo + 128], masks_sb[:],
                        )
                    pending.append((kj, hh, pt, lo))
                while len(pending) > 8:
                    emit_o(*pending.pop(0))
            if kj_hi == kj_max:
                while pending:
                    emit_o(*pending.pop(0))

        def attn_norm(qi, p, oaccs):
            qs = slice(qi * 512, (qi + 1) * 512)
            for hh in range(2):
                off = 64 * hh
                recip = smalls.tile([1, 512], fmm, tag="recip", name="recip")
                with nc.allow_low_precision(reason="f32r rounding of softmax recip"):
                    nc.vector.reciprocal(recip[:], oaccs[hh][64:65, :])
                bc_ps = psAUX.tile([64, 512], f32, tag="aux", name="bc_ps")
                nc.tensor.matmul(bc_ps[:], ones64[:], recip[:], start=True, stop=True)
                bc = smalls.tile([64, 512], f32, tag="bc", name="bc")
                nc.vector.tensor_copy(bc[:], bc_ps[:])
                nc.vector.tensor_mul(ot_sb[p][off:off + 64, qs], oaccs[hh][0:64, :], bc[:])

        def a_unit_list(tch):
            """A(tch) as a list of emission closures (proj groups, V blocks).
            The rope skew chains across units via `state`."""
            if tch >= NT:
                return []
            state = {"pend": None}

            def start():
                prefetch_xt(tch)
                prefetch_xt(tch + 1)

            def qk_group(w_sb, dst, p):
                def emit():
                    xt = xt_tiles[tch]
                    acc = psBig.tile([128, 512], f32, tag="big", name=f"acc{tch}_{p}")
                    for kc in range(KC):
                        nc.tensor.matmul(
                            acc[:],
                            w_sb[:, kc, 128 * p:128 * (p + 1)],
                            xt[:, kc, :],
                            start=(kc == 0), stop=(kc == KC - 1),
                        )
                    raw = tmps.tile([128, 512], fmm, tag="raw", name="raw")
                    nc.vector.tensor_copy(raw[:], acc[:])
                    if state["pend"] is not None:
                        emit_rope(*state["pend"])
                    state["pend"] = (tch, acc, raw, dst, p)
                return emit

            def v_block(jb):
                def emit():
                    xt = xt_tiles[tch]
                    if state["pend"] is not None:
                        emit_rope(*state["pend"])
                        state["pend"] = None
                    jbg = tch * 4 + jb
                    vacc = psAUX.tile([128, 256], f32, tag="aux", name=f"vacc{jbg}")
                    for kc in range(KC):
                        nc.tensor.matmul(
                            vacc[:],
                            xt[:, kc, 128 * jb:128 * (jb + 1)],
                            wv_sb[:, kc, :],
                            start=(kc == 0), stop=(kc == KC - 1),
                        )
                    nc.vector.tensor_copy(
                        v_sb[:].rearrange("p j (h c) -> p j h c", h=HPC)[:, jbg, :, 0:HD],
                        vacc[:].rearrange("p (h c) -> p h c", h=HPC),
                    )
                    if jb == 3:
                        xt_tiles.pop(tch)
                return emit

            units = [start]
            units.append(qk_group(wq_sb, qt_sb, 0))
            units.append(qk_group(wq_sb, qt_sb, 1))
            units.append(qk_group(wk_sb, kt_sb, 0))
            units.append(qk_group(wk_sb, kt_sb, 1))
            for jb in range(4):
                units.append(v_block(jb))
            return units

        def c_unit_list(tch):
            ts = slice(tch * 512, (tch + 1) * 512)

            def c_block(ech):
                def emit():
                    pc = psO.tile([128, 512], f32, tag="oacc", name=f"pc{tch}_{ech}")
                    for kd in range(2):
                        nc.tensor.matmul(
                            pc[:],
                            wp_sb[:, kd, ech * 128:(ech + 1) * 128],
                            ot_sb[kd][:, ts],
                            start=(kd == 0), stop=(kd == 1),
                        )
                    oc = outs.tile([128, 512], f32, tag="oc", name="oc")
                    nc.vector.tensor_copy(oc[:], pc[:])
                    nc.gpsimd.dma_start(out=yT[ech, tch], in_=oc[:])
                return emit
            return [c_block(e) for e in range(8)]

        def b_emit(qi, fill_units):
            """Emit B(qi)'s two attention streams, sprinkling `fill_units`
            (A(qi+1) / C(qi-1) closures) between kj iterations so the PE
            always has independent matmul work queued behind exp waits."""
            kj_max = 4 * (qi + 1)
            n_slots = 2 * kj_max + 2
            fill = list(fill_units)
            n_fill = len(fill)
            slot = [0]

            def maybe_fill():
                # spread the n_fill units evenly across the n_slots slots
                k = (n_fill * (slot[0] + 1)) // n_slots - (n_fill * slot[0]) // n_slots
                for _ in range(k):
                    if fill:
                        fill.pop(0)()
                slot[0] += 1

            for p in range(2):
                oaccs = [psO.tile([128, 512], f32, tag="oacc", name=f"oacc{qi}_{p}_{j}")
                         for j in range(2)]
                pending = []
                for kj in range(kj_max):
                    attn_stream(qi, p, oaccs, pending, kj, kj + 1, kj_max)
                    maybe_fill()
                attn_norm(qi, p, oaccs)
                maybe_fill()
            while fill:
                fill.pop(0)()

        for i in range(NT):
            for u in a_unit_list(i):
                u()
            b_emit(i, [])
            for u in c_unit_list(i):
                u()

    nc.compile()
    return nc


def get_program():
    global _PROGRAM
    if _PROGRAM is None:
        _PROGRAM = build_program()
    return _PROGRAM


def make_in_maps(x, W_qkv, W_proj):
    x = np.asarray(x, dtype=np.float32)
    W_qkv = np.asarray(W_qkv, dtype=np.float32)
    W_proj = np.asarray(W_proj, dtype=np.float32)
    in_maps = []
    xtr = {}
    for b in range(B):
        xt = x[b].T.reshape(D // 128, 128, T // 512, 512)
        xtr[b] = np.ascontiguousarray(xt.transpose(0, 2, 1, 3))
    for core in range(NCORES):
        b, g = divmod(core, 4)
        cs = slice(g * 256, (g + 1) * 256)
        in_maps.append({
            "xT": xtr[b],
            "wq": np.ascontiguousarray(W_qkv[:, 0 * D:1 * D][:, cs]),
            "wk": np.ascontiguousarray(W_qkv[:, 1 * D:2 * D][:, cs]),
            "wv": np.ascontiguousarray(W_qkv[:, 2 * D:3 * D][:, cs]),
            "wp": np.ascontiguousarray(W_proj[cs, :]),
        })
    return in_maps


def gather_output(results):
    out = np.empty((B, T, D), dtype=np.float32)
    for b in range(B):
        acc = results[4 * b]["yT"].astype(np.float32).copy()
        for g in range(1, 4):
            acc += results[4 * b + g]["yT"]
        # (ech, tch, p, t) -> yT (D, T) -> transpose to (T, D)
        yt = acc.transpose(0, 2, 1, 3).reshape(D, T)
        out[b] = yt.T
    return out


def kernel(x, W_qkv, W_proj, key_padding_mask=None, **_ignored):
    # key_padding_mask is all-True per the problem spec (fill: ones) -> no-op.
    from concourse.bass_utils import run_bass_kernel_spmd

    nc = get_program()
    in_maps = make_in_maps(x, W_qkv, W_proj)
    res = run_bass_kernel_spmd(nc, in_maps, list(range(NCORES)))
    return gather_output(res.results)



# revision 48
# speedup vs baseline: 1.1168x; 1.1168x over previous
"""Causal self-attention with RoPE on 8 trn2 NeuronCores.

Problem: B=2, T=2048, D=1024, H=16 heads, head_dim=64, fp32 in/out.
Sharding: core = b*4 + g  (data parallel over batch, tensor parallel over
head groups of 4). Each core computes its 4 heads' attention plus the
row-slice of the output projection; the host sums the 4 partial Y^T per
batch (bf16 partials, f32 accumulate) and transposes back.

All matmul inputs are bf16 (same PE rate as f32r at >=256 moving cols, no
4x penalty on the <256-col diagonal tiles, half the DMA bytes, and 2x DVE
throughput on the all-bf16 elementwise ops). PSUM accumulation stays f32.

Per-core dataflow (everything transposed so matmuls contract on partitions):
  xT (1024, 2048)  =  x[b].T
  QT/KT packs: qt_sb[128, 2, T] (pack p = 2 heads of 64 rows)
  RoPE: QT' = QT*cos + (R2 @ QT)*sin   (R2 = block-diag rotate-half matrix)
  V_aug [128, 16, 260]: V natural layout per key block, 4 heads x (64 dims
      + ones column) -> fused softmax denominator.
  S pair tile [keys 128, 2*512] = both heads of a pack in one 2-bank PSUM
      tile; ONE exp activation covers both halves (halves ACT op count).
  P = exp(S * 0.125) -> bf16 SBUF, diag blocks masked on GPSIMD.
  Oacc pair [65, 2*512] += V_aug^T @ P  (row 64 = denominator)
  norm: DVE reciprocal -> GPSIMD partition_broadcast -> DVE mul -> ot bf16
  Y^T partial = Wp-slice^T @ O^T packs -> bf16 DRAM out (SP-queue DMA).

Phases are interleaved: B(i)'s st->exp->PV stream is ACT-latency-bound, so
A(i+1) (projections) and C(i-1) (out-proj) PE work is sprinkled into its
kj slots to keep the PE busy while the scalar engine catches up.
"""

import sys
import numpy as np

sys.path.insert(0, "/opt/trn_rl_repo")

B, T, D, H = 2, 2048, 1024, 16
HD = 64          # head dim
HPC = 4          # heads per core
NCORES = 8
ROPE_BASE = 10000.0

_PROGRAM = None  # cached compiled program

# GPSIMD (Pool-engine) offloads: flipped on only after HW verifier approval,
# since the BIR verifier enforces rules CoreSim does not.
USE_POOL_COPIES = False   # PSUM->SBUF tensor copies on gpsimd
USE_POOL_MASKS = False    # SBUF tensor_mul (causal masks) on gpsimd
USE_POOL_PB = False       # gpsimd partition_broadcast for the softmax recip


def _bf16():
    import ml_dtypes
    return ml_dtypes.bfloat16


def _rope_tables_np():
    inv_freq = 1.0 / (ROPE_BASE ** (np.arange(0, HD, 2, dtype=np.float32) / np.float32(HD)))
    pos = np.arange(T, dtype=np.float32)
    freqs = np.outer(pos, inv_freq).astype(np.float32)          # (T, 32)
    emb = np.concatenate([freqs, freqs], axis=-1)               # (T, 64)
    cosT = np.cos(emb).T.astype(np.float32)                     # (64, T)
    sinT = np.sin(emb).T.astype(np.float32)
    cos2 = np.vstack([cosT, cosT]).copy()                       # (128, T) two heads
    sin2 = np.vstack([sinT, sinT]).copy()
    # sign-folded sin for the matmul-free rotate-half:
    #   rope(q)[d] = q[d]*cos[d] + rot(q)[d]*sin[d],
    #   rot(q)[base+d] = -q[base+32+d] (d<32) ; q[base+d-32] (d>=32)
    # so ts[base+0:32] = raw[base+32:64] * (-sin[base+0:32]) and
    #    ts[base+32:64] = raw[base+0:32] * (+sin[base+32:64]).
    sinadj = sin2.copy()
    for base in (0, 32, 64, 96):
        if (base // 32) % 2 == 0:
            sinadj[base:base + 32] = -sinadj[base:base + 32]
    return cos2, sin2, sinadj


def _r2_np():
    # qrot[d] = -q[d+32] (d<32) ; q[d-32] (d>=32), per 64-row block.
    # matmul computes out[d, t] = sum_k r2[k, d] q[k, t]
    r2 = np.zeros((128, 128), dtype=np.float32)
    for base in (0, 64):
        for d in range(32):
            r2[base + d + 32, base + d] = -1.0
            r2[base + d, base + d + 32] = 1.0
    return r2


def _masks_np():
    # tri[j, ql] = 1 if key j may attend query ql within a diagonal block
    j = np.arange(128)[:, None]
    ql = np.arange(128)[None, :]
    return (j <= ql).astype(np.float32)                         # [128, 128]


def build_program():
    import concourse.bass as bass
    import concourse.tile as tile
    from concourse import bacc, mybir
    from contextlib import ExitStack

    BF = _bf16()
    f32 = mybir.dt.float32
    bf16 = mybir.dt.bfloat16

    nc = bacc.Bacc(None, target_bir_lowering=False, debug=False)

    # xT pre-tiled on host: xTr[kc, tch, p, t] = x[b].T[kc*128+p, tch*512+t]
    xT = nc.declare_dram_parameter("xT", [D // 128, T // 512, 128, 512], bf16, isOutput=False)
    wq = nc.declare_dram_parameter("wq", [D, 256], bf16, isOutput=False)
    wk = nc.declare_dram_parameter("wk", [D, 256], bf16, isOutput=False)
    wv = nc.declare_dram_parameter("wv", [D, 256], bf16, isOutput=False)
    wp = nc.declare_dram_parameter("wp", [256, D], bf16, isOutput=False)
    # yT tiled: yTr[ech, tch, p, t] = yT_partial[ech*128+p, tch*512+t], bf16
    yT = nc.declare_dram_parameter("yT", [8, T // 512, 128, 512], bf16, isOutput=True)

    cos2_np, sin2_np, _sinadj_np = _rope_tables_np()
    cos_d = nc.inline_tensor(cos2_np.astype(BF), name="cos2")
    sin_d = nc.inline_tensor(sin2_np.astype(BF), name="sin2")
    r2_d = nc.inline_tensor(_r2_np().astype(BF), name="r2")
    masks_d = nc.inline_tensor(_masks_np().astype(BF), name="masks")

    NT = T // 512            # 4 t-chunks
    NJ = T // 128            # 16 key blocks
    KC = D // 128            # 8 contraction chunks
    Exp = mybir.ActivationFunctionType.Exp

    with tile.TileContext(nc) as tc, ExitStack() as ctx:
        # --- persistent SBUF ---
        wts = ctx.enter_context(tc.tile_pool(name="wts", bufs=1))
        packs = ctx.enter_context(tc.tile_pool(name="packs", bufs=1))
        consts = ctx.enter_context(tc.tile_pool(name="consts", bufs=1))

        # --- working pools (xts first: its loads gate the first matmuls) ---
        xts = ctx.enter_context(tc.tile_pool(name="xts", bufs=2))

        wq_sb = wts.tile([128, KC, 256], bf16, tag="wq")
        wk_sb = wts.tile([128, KC, 256], bf16, tag="wk")
        wv_sb = wts.tile([128, KC, 256], bf16, tag="wv")
        wp_sb = wts.tile([128, 2, 1024], bf16, tag="wp")
        xt_tiles = {}

        def prefetch_xt(tch):
            if tch >= NT or tch in xt_tiles:
                return
            xt = xts.tile([128, KC, 512], bf16, tag="xt", name=f"xt{tch}")
            # split so the first accumulation chunks start sooner
            nsplit = 4 if tch == 0 else 2
            step = KC // nsplit
            for s in range(nsplit):
                nc.sync.dma_start(
                    out=xt[:, s * step:(s + 1) * step, :],
                    in_=xT[s * step:(s + 1) * step, tch].rearrange("k p t -> p k t"))
            xt_tiles[tch] = xt

        cos_sb = consts.tile([128, T], bf16, tag="cos")
        sin_sb = consts.tile([128, T], bf16, tag="sin")
        r2_sb = consts.tile([128, 128], bf16, tag="r2")
        masks_sb = consts.tile([128, 128], bf16, tag="masks")
        # The head is HWDGE-throughput-bound: every A(0) group needs all of
        # xt0, so split the critical loads across BOTH descriptor generators
        # (sync -> HWDGE, gpsimd -> Pool SWDGE) in consumption order.
        wq_r = wq.rearrange("(k p) d -> p k d", p=128)
        wk_r = wk.rearrange("(k p) d -> p k d", p=128)
        xt0 = xts.tile([128, KC, 512], bf16, tag="xt", name="xt0")
        xt_tiles[0] = xt0
        nc.sync.dma_start(out=wq_sb[:, :, 0:128], in_=wq_r[:, :, 0:128])
        nc.gpsimd.dma_start(out=xt0[:, 4:6, :], in_=xT[4:6, 0].rearrange("k p t -> p k t"))
        nc.sync.dma_start(out=xt0[:, 0:2, :], in_=xT[0:2, 0].rearrange("k p t -> p k t"))
        nc.gpsimd.dma_start(out=xt0[:, 6:8, :], in_=xT[6:8, 0].rearrange("k p t -> p k t"))
        nc.sync.dma_start(out=wq_sb[:, :, 128:256], in_=wq_r[:, :, 128:256])
        nc.gpsimd.dma_start(out=wk_sb[:, :, 0:128], in_=wk_r[:, :, 0:128])
        nc.sync.dma_start(out=xt0[:, 2:4, :], in_=xT[2:4, 0].rearrange("k p t -> p k t"))
        nc.sync.dma_start(out=wk_sb[:, :, 128:256], in_=wk_r[:, :, 128:256])
        nc.gpsimd.dma_start(out=cos_sb[:], in_=cos_d[:, :])
        nc.gpsimd.dma_start(out=sin_sb[:], in_=sin_d[:, :])
        nc.gpsimd.dma_start(out=r2_sb[:], in_=r2_d[:, :])
        nc.gpsimd.dma_start(out=wv_sb[:], in_=wv.rearrange("(k p) d -> p k d", p=128))
        nc.gpsimd.dma_start(out=masks_sb[:], in_=masks_d[:, :])
        nc.gpsimd.dma_start(out=wp_sb[:], in_=wp.rearrange("(k p) d -> p k d", p=128))

        # packs: [128, pack p, T] so one rope add can write both packs' slices
        qt_sb = packs.tile([128, 2, T], bf16, tag="qt")
        kt_sb = packs.tile([128, 2, T], bf16, tag="kt")
        ot_sb = packs.tile([128, 2, T], bf16, tag="ot")
        v_sb = packs.tile([128, NJ, HPC * (HD + 1)], bf16, tag="vaug")

        # ones columns of v_aug (fused softmax denominator)
        v_view = v_sb[:].rearrange("p j (h c) -> p j h c", h=HPC)
        nc.vector.memset(v_view[:, :, :, HD:HD + 1], 1.0)
        ones64 = consts.tile([1, 64], bf16, tag="ones64")
        nc.vector.memset(ones64[:], 1.0)

        # --- working pools ---
        tmps = ctx.enter_context(tc.tile_pool(name="tmps", bufs=3))
        pts = ctx.enter_context(tc.tile_pool(name="pts", bufs=8))
        outs = ctx.enter_context(tc.tile_pool(name="outs", bufs=3))
        smalls = ctx.enter_context(tc.tile_pool(name="smalls", bufs=2))

        # 8 PSUM banks: stp pairs 2x2, oacc pair 2, aux (acc/rot/vacc/pc) 2x1
        psSTP = ctx.enter_context(tc.tile_pool(name="psSTP", bufs=2, space="PSUM"))
        psOAC = ctx.enter_context(tc.tile_pool(name="psOAC", bufs=1, space="PSUM"))
        psAUX = ctx.enter_context(tc.tile_pool(name="psAUX", bufs=2, space="PSUM"))

        def emit_rope(tch, raw, dst, p):
            # rope(q) = q*cos + rot(q)*sin; the rotate-half must cross
            # partitions, which only the PE (or DMA) may do on HW, so it is
            # a [128,128] block-diag matmul against the bf16 raw copy.
            ts = slice(tch * 512, (tch + 1) * 512)
            rot = psAUX.tile([128, 512], f32, tag="aux", name="rot")
            nc.tensor.matmul(rot[:], r2_sb[:], raw[:], start=True, stop=True)
            tc_t = tmps.tile([128, 512], bf16, tag="tc", name="tc_t")
            nc.vector.tensor_mul(tc_t[:], raw[:], cos_sb[:, ts])
            ts_t = tmps.tile([128, 512], bf16, tag="ts", name="ts_t")
            with nc.allow_low_precision(reason="bf16 rope"):
                nc.vector.tensor_mul(ts_t[:], rot[:], sin_sb[:, ts])
                nc.vector.tensor_add(dst[:, p, ts], tc_t[:], ts_t[:])

        def emit_o(oacc, p, kj_max, kj, pt, lo):
            for hh in range(2):
                hl = 2 * p + hh
                nc.tensor.matmul(
                    oacc[0:65, 512 * hh + lo:512 * hh + 512],
                    v_sb[:, kj, 65 * hl:65 * hl + 65],
                    pt[:, 512 * hh + lo:512 * hh + 512],
                    start=(kj == 0), stop=(kj == kj_max - 1),
                )

        def attn_step(qi, p, oacc, pending, kj, kj_max):
            """One kj iteration: st pair matmuls -> single exp -> (mask),
            software-pipelined so the PE never queues behind its own exp."""
            r = kj - 4 * qi          # >= 0 on the causal diagonal
            lo = max(r, 0) * 128     # first valid column
            stp = psSTP.tile([128, 1024], f32, tag="stp", name="stp")
            for hh in range(2):
                off = 64 * hh
                nc.tensor.matmul(
                    stp[:, 512 * hh + lo:512 * hh + 512],
                    kt_sb[off:off + 64, p, kj * 128:(kj + 1) * 128],
                    qt_sb[off:off + 64, p, qi * 512 + lo:(qi + 1) * 512],
                    start=True, stop=True,
                )
            pt = pts.tile([128, 1024], bf16, tag="pt", name="pt")
            w = 512 - lo
            st3 = stp[:].rearrange("p (h t) -> p h t", h=2)[:, :, lo:512]
            pt3 = pt[:].rearrange("p (h t) -> p h t", h=2)[:, :, lo:512]
            with nc.allow_low_precision(reason="bf16 softmax weights"):
                nc.scalar.activation(pt3, st3, Exp, scale=0.125)
            if r >= 0:
                meng = nc.gpsimd if (USE_POOL_MASKS and qi < 3) else nc.vector
                for hh in range(2):
                    s = slice(512 * hh + lo, 512 * hh + lo + 128)
                    with nc.allow_low_precision(reason="bf16 mask"):
                        meng.tensor_mul(pt[:, s], pt[:, s], masks_sb[:])
            pending.append((kj, pt, lo))
            # keep the window tight on the very last stream: its PVs gate the
            # final norm -> C(3) tail, so don't let them queue up
            window = 1 if (qi == NT - 1 and p == 1) else 4
            while len(pending) > window:
                kj0, pt0, lo0 = pending.pop(0)
                emit_o(oacc, p, kj_max, kj0, pt0, lo0)

        def attn_norm(qi, p, oacc):
            qs = slice(qi * 512, (qi + 1) * 512)
            # one bf16 copy releases the oacc PSUM pair early (the next
            # stream's PV accumulation is waiting on these 2 banks); the
            # ot mul then has exactly one PSUM operand (bc_ps), which is the
            # HW limit for TensorTensor.
            recip2 = smalls.tile([1, 1024], bf16, tag="recip2", name="recip2")
            with nc.allow_low_precision(reason="bf16 softmax recip"):
                nc.vector.reciprocal(recip2[:], oacc[64:65, :])
            oct = smalls.tile([65, 1024], bf16, tag="oct", name="oct")
            with nc.allow_low_precision(reason="bf16 attn out"):
                nc.vector.tensor_copy(oct[:], oacc[0:65, :])
            # partition-broadcast of the recip row: gpsimd custom op when
            # allowed, else a ones-column matmul on the PE (the only other
            # engine that may cross partitions on HW)
            if USE_POOL_PB:
                bc = smalls.tile([64, 1024], bf16, tag="bc", name="bc")
                for hh in range(2):
                    s = slice(512 * hh, 512 * hh + 512)
                    nc.gpsimd.partition_broadcast(bc[0:64, s], recip2[0:1, s])
                for hh in range(2):
                    off = 64 * hh
                    s = slice(512 * hh, 512 * hh + 512)
                    with nc.allow_low_precision(reason="bf16 attn out"):
                        nc.vector.tensor_mul(ot_sb[off:off + 64, p, qs],
                                             oct[0:64, s], bc[0:64, s])
            else:
                for hh in range(2):
                    s = slice(512 * hh, 512 * hh + 512)
                    bc_ps = psAUX.tile([64, 512], f32, tag="aux", name="bc_ps")
                    nc.tensor.matmul(bc_ps[:], ones64[:], recip2[0:1, s],
                                     start=True, stop=True)
                    off = 64 * hh
                    with nc.allow_low_precision(reason="bf16 attn out"):
                        nc.vector.tensor_mul(ot_sb[off:off + 64, p, qs],
                                             oct[0:64, s], bc_ps[:])

        def a_unit_list(tch):
            """A(tch) as a list of emission closures (proj groups, V blocks).
            The rope skew chains across units via `state`."""
            if tch >= NT:
                return []
            state = {"pend": None}

            def start():
                prefetch_xt(tch)
                prefetch_xt(tch + 1)

            def qk_group(w_sb, dst, p):
                def emit():
                    xt = xt_tiles[tch]
                    acc = psAUX.tile([128, 512], f32, tag="aux", name=f"acc{tch}_{p}")
                    for kc in range(KC):
                        nc.tensor.matmul(
                            acc[:],
                            w_sb[:, kc, 128 * p:128 * (p + 1)],
                            xt[:, kc, :],
                            start=(kc == 0), stop=(kc == KC - 1),
                        )
                    raw = tmps.tile([128, 512], bf16, tag="raw", name="raw")
                    ceng = nc.gpsimd if USE_POOL_COPIES else nc.vector
                    with nc.allow_low_precision(reason="bf16 q/k"):
                        ceng.tensor_copy(raw[:], acc[:])
                    if state["pend"] is not None:
                        emit_rope(*state["pend"])
                    state["pend"] = (tch, raw, dst, p)
                return emit

            def v_block(jb):
                def emit():
                    xt = xt_tiles[tch]
                    if state["pend"] is not None:
                        emit_rope(*state["pend"])
                        state["pend"] = None
                    jbg = tch * 4 + jb
                    vacc = psAUX.tile([128, 256], f32, tag="aux", name=f"vacc{jbg}")
                    for kc in range(KC):
                        nc.tensor.matmul(
                            vacc[:],
                            xt[:, kc, 128 * jb:128 * (jb + 1)],
                            wv_sb[:, kc, :],
                            start=(kc == 0), stop=(kc == KC - 1),
                        )
                    veng = nc.gpsimd if USE_POOL_COPIES else nc.vector
                    with nc.allow_low_precision(reason="bf16 v"):
                        veng.tensor_copy(
                            v_view[:, jbg, :, 0:HD],
                            vacc[:].rearrange("p (h c) -> p h c", h=HPC),
                        )
                    if jb == 3:
                        xt_tiles.pop(tch)
                return emit

            units = [start]
            units.append(qk_group(wq_sb, qt_sb, 0))
            units.append(qk_group(wq_sb, qt_sb, 1))
            units.append(qk_group(wk_sb, kt_sb, 0))
            units.append(qk_group(wk_sb, kt_sb, 1))
            for jb in range(4):
                units.append(v_block(jb))
            return units

        def c_unit_list(tch):
            if tch < 0:
                return []
            ts = slice(tch * 512, (tch + 1) * 512)

            def c_block(ech):
                def emit():
                    pc = psAUX.tile([128, 512], f32, tag="aux", name=f"pc{tch}_{ech}")
                    for kd in range(2):
                        nc.tensor.matmul(
                            pc[:],
                            wp_sb[:, kd, ech * 128:(ech + 1) * 128],
                            ot_sb[:, kd, ts],
                            start=(kd == 0), stop=(kd == 1),
                        )
                    oc = outs.tile([128, 512], bf16, tag="oc", name="oc")
                    oeng = nc.gpsimd if (USE_POOL_COPIES and tch == 0) else nc.vector
                    with nc.allow_low_precision(reason="bf16 partial out"):
                        oeng.tensor_copy(oc[:], pc[:])
                    nc.sync.dma_start(out=yT[ech, tch], in_=oc[:])
                return emit
            return [c_block(e) for e in range(8)]

        def c_final(tch):
            """Tail-specific C: nothing else is left for the PE, so pipeline
            pairs of out-proj blocks through the (now free) 2-bank stp slots,
            alternating the PSUM->SBUF copy between DVE and Pool."""
            ts = slice(tch * 512, (tch + 1) * 512)
            for pair in range(4):
                pc2 = psSTP.tile([128, 1024], f32, tag="stp", name=f"pcf{pair}")
                for half in range(2):
                    ech = 2 * pair + half
                    for kd in range(2):
                        nc.tensor.matmul(
                            pc2[:, 512 * half:512 * half + 512],
                            wp_sb[:, kd, ech * 128:(ech + 1) * 128],
                            ot_sb[:, kd, ts],
                            start=(kd == 0), stop=(kd == 1),
                        )
                oc2 = outs.tile([128, 1024], bf16, tag="oc2", name="oc2")
                eng2 = nc.gpsimd if USE_POOL_COPIES else nc.vector
                with nc.allow_low_precision(reason="bf16 partial out"):
                    nc.vector.tensor_copy(oc2[:, 0:512], pc2[:, 0:512])
                    eng2.tensor_copy(oc2[:, 512:1024], pc2[:, 512:1024])
                nc.sync.dma_start(out=yT[2 * pair, tch], in_=oc2[:, 0:512])
                nc.sync.dma_start(out=yT[2 * pair + 1, tch], in_=oc2[:, 512:1024])

        def interleave_fill(a_units, c_units):
            """start + qk units first (rope latency chain), C blocks woven
            between the chunky A units so aux-psum slots alternate."""
            fill = []
            a = list(a_units)
            c = list(c_units)
            if a:
                fill.append(a.pop(0))       # start (DMA prefetch) first
            while a or c:
                if a:
                    fill.append(a.pop(0))
                if c:
                    fill.append(c.pop(0))
            return fill

        def b_emit(qi, fill_units):
            """Emit B(qi)'s attention streams, sprinkling `fill_units`
            (A(qi+1) / C(qi-1) closures) between kj iterations so the PE
            always has independent matmul work queued behind exp waits."""
            kj_max = 4 * (qi + 1)
            n_slots = 2 * (kj_max + 1)
            fill = list(fill_units)
            n_fill = len(fill)
            slot = [0]

            def maybe_fill():
                # spread the n_fill units evenly across the n_slots slots
                k = (n_fill * (slot[0] + 1)) // n_slots - (n_fill * slot[0]) // n_slots
                for _ in range(k):
                    if fill:
                        fill.pop(0)()
                slot[0] += 1

            for p in range(2):
                oacc = psOAC.tile([128, 1024], f32, tag="oaccp", name=f"oacc{qi}_{p}")
                pending = []
                for kj in range(kj_max):
                    attn_step(qi, p, oacc, pending, kj, kj_max)
                    maybe_fill()
                while pending:
                    kj0, pt0, lo0 = pending.pop(0)
                    emit_o(oacc, p, kj_max, kj0, pt0, lo0)
                attn_norm(qi, p, oacc)
                maybe_fill()
            while fill:
                fill.pop(0)()

        # Fill rebalance: the later B(i) streams have the most ACT (exp) work
        # relative to their own PE work, so the out-proj C units are pushed as
        # late as their ot dependencies allow: B(3) gets C(1)+C(2) while the
        # A units go one chunk ahead as before.
        for u in a_unit_list(0):
            u()
        b_emit(0, interleave_fill(a_unit_list(1), []))
        b_emit(1, interleave_fill(a_unit_list(2), []))
        b_emit(2, interleave_fill(a_unit_list(3), c_unit_list(0)))
        b_emit(3, interleave_fill([], c_unit_list(1) + c_unit_list(2)))
        c_final(NT - 1)

    nc.compile()
    return nc


def get_program():
    global _PROGRAM
    if _PROGRAM is None:
        _PROGRAM = build_program()
    return _PROGRAM


def make_in_maps(x, W_qkv, W_proj):
    BF = _bf16()
    x = np.asarray(x, dtype=np.float32)
    W_qkv = np.asarray(W_qkv, dtype=np.float32)
    W_proj = np.asarray(W_proj, dtype=np.float32)
    in_maps = []
    xtr = {}
    for b in range(B):
        xt = x[b].T.reshape(D // 128, 128, T // 512, 512)
        xtr[b] = np.ascontiguousarray(xt.transpose(0, 2, 1, 3)).astype(BF)
    for core in range(NCORES):
        b, g = divmod(core, 4)
        cs = slice(g * 256, (g + 1) * 256)
        in_maps.append({
            "xT": xtr[b],
            "wq": np.ascontiguousarray(W_qkv[:, 0 * D:1 * D][:, cs]).astype(BF),
            "wk": np.ascontiguousarray(W_qkv[:, 1 * D:2 * D][:, cs]).astype(BF),
            "wv": np.ascontiguousarray(W_qkv[:, 2 * D:3 * D][:, cs]).astype(BF),
            "wp": np.ascontiguousarray(W_proj[cs, :]).astype(BF),
        })
    return in_maps


def gather_output(results):
    out = np.empty((B, T, D), dtype=np.float32)
    for b in range(B):
        acc = np.asarray(results[4 * b]["yT"]).astype(np.float32)
        for g in range(1, 4):
            acc += np.asarray(results[4 * b + g]["yT"]).astype(np.float32)
        # (ech, tch, p, t) -> yT (D, T) -> transpose to (T, D)
        yt = acc.transpose(0, 2, 1, 3).reshape(D, T)
        out[b] = yt.T
    return out


def kernel(x, W_qkv, W_proj, key_padding_mask=None, **_ignored):
    # key_padding_mask is all-True per the problem spec (fill: ones) -> no-op.
    from concourse.bass_utils import run_bass_kernel_spmd

    nc = get_program()
    in_maps = make_in_maps(x, W_qkv, W_proj)
    res = run_bass_kernel_spmd(nc, in_maps, list(range(NCORES)))
    return gather_output(res.results)


# revision 53
# speedup vs baseline: 1.1945x; 1.0696x over previous
"""Causal self-attention with RoPE on 8 trn2 NeuronCores.

Problem: B=2, T=2048, D=1024, H=16 heads, head_dim=64, fp32 in/out.
Sharding: core = b*4 + g  (data parallel over batch, tensor parallel over
head groups of 4). Each core computes its 4 heads' attention plus the
row-slice of the output projection; the host sums the 4 partial Y^T per
batch (bf16 partials, f32 accumulate) and transposes back.

All matmul inputs are bf16 (same PE rate as f32r at >=256 moving cols, no
4x penalty on the <256-col diagonal tiles, half the DMA bytes, and 2x DVE
throughput on the all-bf16 elementwise ops). PSUM accumulation stays f32.

Per-core dataflow (everything transposed so matmuls contract on partitions):
  xT (1024, 2048)  =  x[b].T
  QT/KT packs: qt_sb[128, 2, T] (pack p = 2 heads of 64 rows)
  RoPE: QT' = QT*cos + (R2 @ QT)*sin   (R2 = block-diag rotate-half matrix)
  V_aug [128, 16, 260]: V natural layout per key block, 4 heads x (64 dims
      + ones column) -> fused softmax denominator.
  S pair tile [keys 128, 2*512] = both heads of a pack in one 2-bank PSUM
      tile; ONE exp activation covers both halves (halves ACT op count).
  P = exp(S * 0.125) -> bf16 SBUF, diag blocks masked on GPSIMD.
  Oacc pair [65, 2*512] += V_aug^T @ P  (row 64 = denominator)
  norm: DVE reciprocal -> GPSIMD partition_broadcast -> DVE mul -> ot bf16
  Y^T partial = Wp-slice^T @ O^T packs -> bf16 DRAM out (SP-queue DMA).

Phases are interleaved: B(i)'s st->exp->PV stream is ACT-latency-bound, so
A(i+1) (projections) and C(i-1) (out-proj) PE work is sprinkled into its
kj slots to keep the PE busy while the scalar engine catches up.
"""

import sys
import numpy as np

sys.path.insert(0, "/opt/trn_rl_repo")

B, T, D, H = 2, 2048, 1024, 16
HD = 64          # head dim
HPC = 4          # heads per core
NCORES = 8
ROPE_BASE = 10000.0

_PROGRAM = None  # cached compiled program

# GPSIMD (Pool-engine) offloads: flipped on only after HW verifier approval,
# since the BIR verifier enforces rules CoreSim does not.
USE_POOL_COPIES = False   # dead: "GPSIMD Instructions cannot access PSUM"
USE_POOL_MASKS = True     # SBUF tensor_mul (causal masks) on gpsimd
USE_POOL_PB = True        # gpsimd partition_broadcast for the softmax recip


def _bf16():
    import ml_dtypes
    return ml_dtypes.bfloat16


def _rope_tables_np():
    inv_freq = 1.0 / (ROPE_BASE ** (np.arange(0, HD, 2, dtype=np.float32) / np.float32(HD)))
    pos = np.arange(T, dtype=np.float32)
    freqs = np.outer(pos, inv_freq).astype(np.float32)          # (T, 32)
    emb = np.concatenate([freqs, freqs], axis=-1)               # (T, 64)
    cosT = np.cos(emb).T.astype(np.float32)                     # (64, T)
    sinT = np.sin(emb).T.astype(np.float32)
    cos2 = np.vstack([cosT, cosT]).copy()                       # (128, T) two heads
    sin2 = np.vstack([sinT, sinT]).copy()
    # sign-folded sin for the matmul-free rotate-half:
    #   rope(q)[d] = q[d]*cos[d] + rot(q)[d]*sin[d],
    #   rot(q)[base+d] = -q[base+32+d] (d<32) ; q[base+d-32] (d>=32)
    # so ts[base+0:32] = raw[base+32:64] * (-sin[base+0:32]) and
    #    ts[base+32:64] = raw[base+0:32] * (+sin[base+32:64]).
    sinadj = sin2.copy()
    for base in (0, 32, 64, 96):
        if (base // 32) % 2 == 0:
            sinadj[base:base + 32] = -sinadj[base:base + 32]
    return cos2, sin2, sinadj


def _r2_np():
    # qrot[d] = -q[d+32] (d<32) ; q[d-32] (d>=32), per 64-row block.
    # matmul computes out[d, t] = sum_k r2[k, d] q[k, t]
    r2 = np.zeros((128, 128), dtype=np.float32)
    for base in (0, 64):
        for d in range(32):
            r2[base + d + 32, base + d] = -1.0
            r2[base + d, base + d + 32] = 1.0
    return r2


def _masks_np():
    # tri[j, ql] = 1 if key j may attend query ql within a diagonal block
    j = np.arange(128)[:, None]
    ql = np.arange(128)[None, :]
    return (j <= ql).astype(np.float32)                         # [128, 128]


def build_program():
    import concourse.bass as bass
    import concourse.tile as tile
    from concourse import bacc, mybir
    from contextlib import ExitStack

    BF = _bf16()
    f32 = mybir.dt.float32
    bf16 = mybir.dt.bfloat16

    nc = bacc.Bacc(None, target_bir_lowering=False, debug=False)

    # xT pre-tiled on host: xTr[kc, tch, p, t] = x[b].T[kc*128+p, tch*512+t]
    xT = nc.declare_dram_parameter("xT", [D // 128, T // 512, 128, 512], bf16, isOutput=False)
    wq = nc.declare_dram_parameter("wq", [D, 256], bf16, isOutput=False)
    wk = nc.declare_dram_parameter("wk", [D, 256], bf16, isOutput=False)
    wv = nc.declare_dram_parameter("wv", [D, 256], bf16, isOutput=False)
    wp = nc.declare_dram_parameter("wp", [256, D], bf16, isOutput=False)
    # yT tiled: yTr[ech, tch, p, t] = yT_partial[ech*128+p, tch*512+t], bf16
    yT = nc.declare_dram_parameter("yT", [8, T // 512, 128, 512], bf16, isOutput=True)

    cos2_np, sin2_np, _sinadj_np = _rope_tables_np()
    cos_d = nc.inline_tensor(cos2_np.astype(BF), name="cos2")
    sin_d = nc.inline_tensor(sin2_np.astype(BF), name="sin2")
    r2_d = nc.inline_tensor(_r2_np().astype(BF), name="r2")
    masks_d = nc.inline_tensor(_masks_np().astype(BF), name="masks")

    NT = T // 512            # 4 t-chunks
    NJ = T // 128            # 16 key blocks
    KC = D // 128            # 8 contraction chunks
    Exp = mybir.ActivationFunctionType.Exp

    with tile.TileContext(nc) as tc, ExitStack() as ctx:
        # --- persistent SBUF ---
        wts = ctx.enter_context(tc.tile_pool(name="wts", bufs=1))
        packs = ctx.enter_context(tc.tile_pool(name="packs", bufs=1))
        consts = ctx.enter_context(tc.tile_pool(name="consts", bufs=1))

        # --- working pools (xts first: its loads gate the first matmuls) ---
        xts = ctx.enter_context(tc.tile_pool(name="xts", bufs=2))

        wq_sb = wts.tile([128, KC, 256], bf16, tag="wq")
        wk_sb = wts.tile([128, KC, 256], bf16, tag="wk")
        wv_sb = wts.tile([128, KC, 256], bf16, tag="wv")
        wp_sb = wts.tile([128, 2, 1024], bf16, tag="wp")
        xt_tiles = {}

        def prefetch_xt(tch):
            if tch >= NT or tch in xt_tiles:
                return
            xt = xts.tile([128, KC, 512], bf16, tag="xt", name=f"xt{tch}")
            # split so the first accumulation chunks start sooner
            nsplit = 4 if tch == 0 else 2
            step = KC // nsplit
            for s in range(nsplit):
                nc.sync.dma_start(
                    out=xt[:, s * step:(s + 1) * step, :],
                    in_=xT[s * step:(s + 1) * step, tch].rearrange("k p t -> p k t"))
            xt_tiles[tch] = xt

        cos_sb = consts.tile([128, T], bf16, tag="cos")
        sin_sb = consts.tile([128, T], bf16, tag="sin")
        r2_sb = consts.tile([128, 128], bf16, tag="r2")
        masks_sb = consts.tile([128, 128], bf16, tag="masks")
        # The head is HWDGE-throughput-bound: every A(0) group needs all of
        # xt0, so split the critical loads across BOTH descriptor generators
        # (sync -> HWDGE, gpsimd -> Pool SWDGE) in consumption order.
        wq_r = wq.rearrange("(k p) d -> p k d", p=128)
        wk_r = wk.rearrange("(k p) d -> p k d", p=128)
        xt0 = xts.tile([128, KC, 512], bf16, tag="xt", name="xt0")
        xt_tiles[0] = xt0
        nc.sync.dma_start(out=wq_sb[:, :, 0:128], in_=wq_r[:, :, 0:128])
        nc.gpsimd.dma_start(out=r2_sb[:], in_=r2_d[:, :])
        nc.gpsimd.dma_start(out=xt0[:, 4:6, :], in_=xT[4:6, 0].rearrange("k p t -> p k t"))
        nc.sync.dma_start(out=xt0[:, 0:2, :], in_=xT[0:2, 0].rearrange("k p t -> p k t"))
        nc.gpsimd.dma_start(out=xt0[:, 6:8, :], in_=xT[6:8, 0].rearrange("k p t -> p k t"))
        nc.sync.dma_start(out=wq_sb[:, :, 128:256], in_=wq_r[:, :, 128:256])
        nc.gpsimd.dma_start(out=wk_sb[:, :, 0:128], in_=wk_r[:, :, 0:128])
        nc.sync.dma_start(out=xt0[:, 2:4, :], in_=xT[2:4, 0].rearrange("k p t -> p k t"))
        nc.sync.dma_start(out=wk_sb[:, :, 128:256], in_=wk_r[:, :, 128:256])
        nc.gpsimd.dma_start(out=cos_sb[:], in_=cos_d[:, :])
        nc.gpsimd.dma_start(out=sin_sb[:], in_=sin_d[:, :])
        nc.gpsimd.dma_start(out=wv_sb[:], in_=wv.rearrange("(k p) d -> p k d", p=128))
        nc.gpsimd.dma_start(out=masks_sb[:], in_=masks_d[:, :])
        nc.gpsimd.dma_start(out=wp_sb[:], in_=wp.rearrange("(k p) d -> p k d", p=128))

        # packs: [128, pack p, T] so one rope add can write both packs' slices
        qt_sb = packs.tile([128, 2, T], bf16, tag="qt")
        kt_sb = packs.tile([128, 2, T], bf16, tag="kt")
        ot_sb = packs.tile([128, 2, T], bf16, tag="ot")
        v_sb = packs.tile([128, NJ, HPC * (HD + 1)], bf16, tag="vaug")

        # ones columns of v_aug (fused softmax denominator)
        v_view = v_sb[:].rearrange("p j (h c) -> p j h c", h=HPC)
        nc.vector.memset(v_view[:, :, :, HD:HD + 1], 1.0)
        ones64 = consts.tile([1, 64], bf16, tag="ones64")
        nc.vector.memset(ones64[:], 1.0)

        # --- working pools ---
        tmps = ctx.enter_context(tc.tile_pool(name="tmps", bufs=3))
        pts = ctx.enter_context(tc.tile_pool(name="pts", bufs=8))
        outs = ctx.enter_context(tc.tile_pool(name="outs", bufs=3))
        smalls = ctx.enter_context(tc.tile_pool(name="smalls", bufs=2))

        # 8 PSUM banks: stp pairs 2x2, oacc pair 2, aux (acc/rot/vacc/pc) 2x1
        psSTP = ctx.enter_context(tc.tile_pool(name="psSTP", bufs=2, space="PSUM"))
        psOAC = ctx.enter_context(tc.tile_pool(name="psOAC", bufs=1, space="PSUM"))
        psAUX = ctx.enter_context(tc.tile_pool(name="psAUX", bufs=2, space="PSUM"))

        def emit_rope(tch, raw, dst, p):
            # rope(q) = q*cos + rot(q)*sin; the rotate-half must cross
            # partitions, which only the PE (or DMA) may do on HW, so it is
            # a [128,128] block-diag matmul against the bf16 raw copy.
            ts = slice(tch * 512, (tch + 1) * 512)
            rot = psAUX.tile([128, 512], f32, tag="aux", name="rot")
            nc.tensor.matmul(rot[:], r2_sb[:], raw[:], start=True, stop=True)
            tc_t = tmps.tile([128, 512], bf16, tag="tc", name="tc_t")
            nc.vector.tensor_mul(tc_t[:], raw[:], cos_sb[:, ts])
            ts_t = tmps.tile([128, 512], bf16, tag="ts", name="ts_t")
            with nc.allow_low_precision(reason="bf16 rope"):
                nc.vector.tensor_mul(ts_t[:], rot[:], sin_sb[:, ts])
                nc.vector.tensor_add(dst[:, p, ts], tc_t[:], ts_t[:])

        def emit_o(oacc, p, kj_max, kj, pt, lo):
            for hh in range(2):
                hl = 2 * p + hh
                nc.tensor.matmul(
                    oacc[0:65, 512 * hh + lo:512 * hh + 512],
                    v_sb[:, kj, 65 * hl:65 * hl + 65],
                    pt[:, 512 * hh + lo:512 * hh + 512],
                    start=(kj == 0), stop=(kj == kj_max - 1),
                )

        def attn_step(qi, p, oacc, pending, kj, kj_max):
            """One kj iteration: st pair matmuls -> single exp -> (mask),
            software-pipelined so the PE never queues behind its own exp."""
            r = kj - 4 * qi          # >= 0 on the causal diagonal
            lo = max(r, 0) * 128     # first valid column
            stp = psSTP.tile([128, 1024], f32, tag="stp", name="stp")
            for hh in range(2):
                off = 64 * hh
                nc.tensor.matmul(
                    stp[:, 512 * hh + lo:512 * hh + 512],
                    kt_sb[off:off + 64, p, kj * 128:(kj + 1) * 128],
                    qt_sb[off:off + 64, p, qi * 512 + lo:(qi + 1) * 512],
                    start=True, stop=True,
                )
            pt = pts.tile([128, 1024], bf16, tag="pt", name="pt")
            w = 512 - lo
            st3 = stp[:].rearrange("p (h t) -> p h t", h=2)[:, :, lo:512]
            pt3 = pt[:].rearrange("p (h t) -> p h t", h=2)[:, :, lo:512]
            with nc.allow_low_precision(reason="bf16 softmax weights"):
                nc.scalar.activation(pt3, st3, Exp, scale=0.125)
            if r >= 0:
                meng = nc.gpsimd if (USE_POOL_MASKS and qi < 3) else nc.vector
                for hh in range(2):
                    s = slice(512 * hh + lo, 512 * hh + lo + 128)
                    with nc.allow_low_precision(reason="bf16 mask"):
                        meng.tensor_mul(pt[:, s], pt[:, s], masks_sb[:])
            pending.append((kj, pt, lo))
            # keep the window tight on the very last stream: its PVs gate the
            # final norm -> C(3) tail, so don't let them queue up
            window = 1 if (qi == NT - 1 and p == 1) else 4
            while len(pending) > window:
                kj0, pt0, lo0 = pending.pop(0)
                emit_o(oacc, p, kj_max, kj0, pt0, lo0)

        def attn_norm(qi, p, oacc):
            qs = slice(qi * 512, (qi + 1) * 512)
            # one bf16 copy releases the oacc PSUM pair early (the next
            # stream's PV accumulation is waiting on these 2 banks); the
            # ot mul then has exactly one PSUM operand (bc_ps), which is the
            # HW limit for TensorTensor.
            recip2 = smalls.tile([1, 1024], bf16, tag="recip2", name="recip2")
            with nc.allow_low_precision(reason="bf16 softmax recip"):
                nc.vector.reciprocal(recip2[:], oacc[64:65, :])
            oct = smalls.tile([65, 1024], bf16, tag="oct", name="oct")
            with nc.allow_low_precision(reason="bf16 attn out"):
                nc.vector.tensor_copy(oct[:], oacc[0:65, :])
            # partition-broadcast of the recip row: gpsimd custom op when
            # allowed, else a ones-column matmul on the PE (the only other
            # engine that may cross partitions on HW)
            if USE_POOL_PB:
                bc = smalls.tile([64, 1024], bf16, tag="bc", name="bc")
                for hh in range(2):
                    s = slice(512 * hh, 512 * hh + 512)
                    nc.gpsimd.partition_broadcast(bc[0:64, s], recip2[0:1, s])
                for hh in range(2):
                    off = 64 * hh
                    s = slice(512 * hh, 512 * hh + 512)
                    with nc.allow_low_precision(reason="bf16 attn out"):
                        nc.vector.tensor_mul(ot_sb[off:off + 64, p, qs],
                                             oct[0:64, s], bc[0:64, s])
            else:
                for hh in range(2):
                    s = slice(512 * hh, 512 * hh + 512)
                    bc_ps = psAUX.tile([64, 512], f32, tag="aux", name="bc_ps")
                    nc.tensor.matmul(bc_ps[:], ones64[:], recip2[0:1, s],
                                     start=True, stop=True)
                    off = 64 * hh
                    with nc.allow_low_precision(reason="bf16 attn out"):
                        nc.vector.tensor_mul(ot_sb[off:off + 64, p, qs],
                                             oct[0:64, s], bc_ps[:])

        def a_unit_list(tch):
            """A(tch) as a list of emission closures (proj groups, V blocks).
            The rope skew chains across units via `state`."""
            if tch >= NT:
                return []
            state = {"pend": None}

            def start():
                prefetch_xt(tch)

            def qk_group(w_sb, dst, p):
                def emit():
                    xt = xt_tiles[tch]
                    acc = psAUX.tile([128, 512], f32, tag="aux", name=f"acc{tch}_{p}")
                    for kc in range(KC):
                        nc.tensor.matmul(
                            acc[:],
                            w_sb[:, kc, 128 * p:128 * (p + 1)],
                            xt[:, kc, :],
                            start=(kc == 0), stop=(kc == KC - 1),
                        )
                    raw = tmps.tile([128, 512], bf16, tag="raw", name="raw")
                    ceng = nc.gpsimd if USE_POOL_COPIES else nc.vector
                    with nc.allow_low_precision(reason="bf16 q/k"):
                        ceng.tensor_copy(raw[:], acc[:])
                    if state["pend"] is not None:
                        emit_rope(*state["pend"])
                    state["pend"] = (tch, raw, dst, p)
                return emit

            def v_block(jb):
                def emit():
                    xt = xt_tiles[tch]
                    if state["pend"] is not None:
                        emit_rope(*state["pend"])
                        state["pend"] = None
                    jbg = tch * 4 + jb
                    vacc = psAUX.tile([128, 256], f32, tag="aux", name=f"vacc{jbg}")
                    for kc in range(KC):
                        nc.tensor.matmul(
                            vacc[:],
                            xt[:, kc, 128 * jb:128 * (jb + 1)],
                            wv_sb[:, kc, :],
                            start=(kc == 0), stop=(kc == KC - 1),
                        )
                    veng = nc.gpsimd if USE_POOL_COPIES else nc.vector
                    with nc.allow_low_precision(reason="bf16 v"):
                        veng.tensor_copy(
                            v_view[:, jbg, :, 0:HD],
                            vacc[:].rearrange("p (h c) -> p h c", h=HPC),
                        )
                    if jb == 3:
                        xt_tiles.pop(tch)
                        # prefetch the next chunk only now: issuing it any
                        # earlier puts its transfers ahead of the (critical)
                        # cos/sin/wv const loads in the serial DMA queue
                        prefetch_xt(tch + 1)
                return emit

            units = [start]
            units.append(qk_group(wq_sb, qt_sb, 0))
            units.append(qk_group(wq_sb, qt_sb, 1))
            units.append(qk_group(wk_sb, kt_sb, 0))
            units.append(qk_group(wk_sb, kt_sb, 1))
            for jb in range(4):
                units.append(v_block(jb))
            return units

        def c_unit_list(tch):
            if tch < 0:
                return []
            ts = slice(tch * 512, (tch + 1) * 512)

            def c_block(ech):
                def emit():
                    pc = psAUX.tile([128, 512], f32, tag="aux", name=f"pc{tch}_{ech}")
                    for kd in range(2):
                        nc.tensor.matmul(
                            pc[:],
                            wp_sb[:, kd, ech * 128:(ech + 1) * 128],
                            ot_sb[:, kd, ts],
                            start=(kd == 0), stop=(kd == 1),
                        )
                    oc = outs.tile([128, 512], bf16, tag="oc", name="oc")
                    oeng = nc.gpsimd if (USE_POOL_COPIES and tch == 0) else nc.vector
                    with nc.allow_low_precision(reason="bf16 partial out"):
                        oeng.tensor_copy(oc[:], pc[:])
                    nc.sync.dma_start(out=yT[ech, tch], in_=oc[:])
                return emit
            return [c_block(e) for e in range(8)]

        def c_final(tch):
            """Tail-specific C: nothing else is left for the PE, so pipeline
            pairs of out-proj blocks through the (now free) 2-bank stp slots,
            alternating the PSUM->SBUF copy between DVE and Pool."""
            ts = slice(tch * 512, (tch + 1) * 512)
            for pair in range(4):
                pc2 = psSTP.tile([128, 1024], f32, tag="stp", name=f"pcf{pair}")
                for half in range(2):
                    ech = 2 * pair + half
                    for kd in range(2):
                        nc.tensor.matmul(
                            pc2[:, 512 * half:512 * half + 512],
                            wp_sb[:, kd, ech * 128:(ech + 1) * 128],
                            ot_sb[:, kd, ts],
                            start=(kd == 0), stop=(kd == 1),
                        )
                oc2 = outs.tile([128, 1024], bf16, tag="oc2", name="oc2")
                eng2 = nc.gpsimd if USE_POOL_COPIES else nc.vector
                with nc.allow_low_precision(reason="bf16 partial out"):
                    nc.vector.tensor_copy(oc2[:, 0:512], pc2[:, 0:512])
                    eng2.tensor_copy(oc2[:, 512:1024], pc2[:, 512:1024])
                nc.sync.dma_start(out=yT[2 * pair, tch], in_=oc2[:, 0:512])
                nc.sync.dma_start(out=yT[2 * pair + 1, tch], in_=oc2[:, 512:1024])

        def interleave_fill(a_units, c_units):
            """start + qk units first (rope latency chain), C blocks woven
            between the chunky A units so aux-psum slots alternate."""
            fill = []
            a = list(a_units)
            c = list(c_units)
            if a:
                fill.append(a.pop(0))       # start (DMA prefetch) first
            while a or c:
                if a:
                    fill.append(a.pop(0))
                if c:
                    fill.append(c.pop(0))
            return fill

        def b_emit(qi, fill_units):
            """Emit B(qi)'s attention streams, sprinkling `fill_units`
            (A(qi+1) / C(qi-1) closures) between kj iterations so the PE
            always has independent matmul work queued behind exp waits."""
            kj_max = 4 * (qi + 1)
            n_slots = 2 * (kj_max + 1)
            fill = list(fill_units)
            n_fill = len(fill)
            slot = [0]

            def maybe_fill():
                # spread the n_fill units evenly across the n_slots slots
                k = (n_fill * (slot[0] + 1)) // n_slots - (n_fill * slot[0]) // n_slots
                for _ in range(k):
                    if fill:
                        fill.pop(0)()
                slot[0] += 1

            for p in range(2):
                oacc = psOAC.tile([128, 1024], f32, tag="oaccp", name=f"oacc{qi}_{p}")
                pending = []
                for kj in range(kj_max):
                    attn_step(qi, p, oacc, pending, kj, kj_max)
                    maybe_fill()
                while pending:
                    kj0, pt0, lo0 = pending.pop(0)
                    emit_o(oacc, p, kj_max, kj0, pt0, lo0)
                attn_norm(qi, p, oacc)
                maybe_fill()
            while fill:
                fill.pop(0)()

        # Fill rebalance: the later B(i) streams have the most ACT (exp) work
        # relative to their own PE work, so the out-proj C units are pushed as
        # late as their ot dependencies allow: B(3) gets C(1)+C(2) while the
        # A units go one chunk ahead as before.
        for u in a_unit_list(0):
            u()
        b_emit(0, interleave_fill(a_unit_list(1), []))
        b_emit(1, interleave_fill(a_unit_list(2), []))
        b_emit(2, interleave_fill(a_unit_list(3), c_unit_list(0)))
        b_emit(3, interleave_fill([], c_unit_list(1) + c_unit_list(2)))
        c_final(NT - 1)

    nc.compile()
    return nc


def get_program():
    global _PROGRAM
    if _PROGRAM is None:
        _PROGRAM = build_program()
    return _PROGRAM


def make_in_maps(x, W_qkv, W_proj):
    BF = _bf16()
    x = np.asarray(x, dtype=np.float32)
    W_qkv = np.asarray(W_qkv, dtype=np.float32)
    W_proj = np.asarray(W_proj, dtype=np.float32)
    in_maps = []
    xtr = {}
    for b in range(B):
        xt = x[b].T.reshape(D // 128, 128, T // 512, 512)
        xtr[b] = np.ascontiguousarray(xt.transpose(0, 2, 1, 3)).astype(BF)
    for core in range(NCORES):
        b, g = divmod(core, 4)
        cs = slice(g * 256, (g + 1) * 256)
        in_maps.append({
            "xT": xtr[b],
            "wq": np.ascontiguousarray(W_qkv[:, 0 * D:1 * D][:, cs]).astype(BF),
            "wk": np.ascontiguousarray(W_qkv[:, 1 * D:2 * D][:, cs]).astype(BF),
            "wv": np.ascontiguousarray(W_qkv[:, 2 * D:3 * D][:, cs]).astype(BF),
            "wp": np.ascontiguousarray(W_proj[cs, :]).astype(BF),
        })
    return in_maps


def gather_output(results):
    out = np.empty((B, T, D), dtype=np.float32)
    for b in range(B):
        acc = np.asarray(results[4 * b]["yT"]).astype(np.float32)
        for g in range(1, 4):
            acc += np.asarray(results[4 * b + g]["yT"]).astype(np.float32)
        # (ech, tch, p, t) -> yT (D, T) -> transpose to (T, D)
        yt = acc.transpose(0, 2, 1, 3).reshape(D, T)
        out[b] = yt.T
    return out


def kernel(x, W_qkv, W_proj, key_padding_mask=None, **_ignored):
    # key_padding_mask is all-True per the problem spec (fill: ones) -> no-op.
    from concourse.bass_utils import run_bass_kernel_spmd

    nc = get_program()
    in_maps = make_in_maps(x, W_qkv, W_proj)
    res = run_bass_kernel_spmd(nc, in_maps, list(range(NCORES)))
    return gather_output(res.results)


# revision 65
# speedup vs baseline: 1.2537x; 1.0496x over previous
"""Causal self-attention with RoPE on 8 trn2 NeuronCores.

Problem: B=2, T=2048, D=1024, H=16 heads, head_dim=64, fp32 in/out.
Sharding: core = b*4 + g  (data parallel over batch, tensor parallel over
head groups of 4). Each core computes its 4 heads' attention plus the
row-slice of the output projection; the host sums the 4 partial Y^T per
batch (bf16 partials, f32 accumulate) and transposes back.

All matmul inputs are bf16 (same PE rate as f32r at >=256 moving cols, no
4x penalty on the <256-col diagonal tiles, half the DMA bytes, and 2x DVE
throughput on the all-bf16 elementwise ops). PSUM accumulation stays f32.

Per-core dataflow (everything transposed so matmuls contract on partitions):
  xT (1024, 2048)  =  x[b].T
  QT/KT packs: qt_sb[128, 2, T] (pack p = 2 heads of 64 rows)
  RoPE: QT' = QT*cos + (R2 @ QT)*sin   (R2 = block-diag rotate-half matrix)
  V_aug [128, 16, 260]: V natural layout per key block, 4 heads x (64 dims
      + ones column) -> fused softmax denominator.
  S pair tile [keys 128, 2*512] = both heads of a pack in one 2-bank PSUM
      tile; ONE exp activation covers both halves (halves ACT op count).
  P = exp(S * 0.125) -> bf16 SBUF, diag blocks masked on GPSIMD.
  Oacc pair [65, 2*512] += V_aug^T @ P  (row 64 = denominator)
  norm: DVE reciprocal -> GPSIMD partition_broadcast -> DVE mul -> ot bf16
  Y^T partial = Wp-slice^T @ O^T packs -> bf16 DRAM out (SP-queue DMA).

Phases are interleaved: B(i)'s st->exp->PV stream is ACT-latency-bound, so
A(i+1) (projections) and C(i-1) (out-proj) PE work is sprinkled into its
kj slots to keep the PE busy while the scalar engine catches up.
"""

import sys
import numpy as np

sys.path.insert(0, "/opt/trn_rl_repo")

B, T, D, H = 2, 2048, 1024, 16
HD = 64          # head dim
HPC = 4          # heads per core
NCORES = 8
ROPE_BASE = 10000.0

_PROGRAM = None  # cached compiled program

# GPSIMD (Pool-engine) offloads: flipped on only after HW verifier approval,
# since the BIR verifier enforces rules CoreSim does not.
USE_POOL_COPIES = False   # dead: "GPSIMD Instructions cannot access PSUM"
USE_POOL_MASKS = True     # SBUF tensor_mul (causal masks) on gpsimd
USE_POOL_PB = True        # gpsimd partition_broadcast for the softmax recip


def _bf16():
    import ml_dtypes
    return ml_dtypes.bfloat16


def _rope_tables_np():
    inv_freq = 1.0 / (ROPE_BASE ** (np.arange(0, HD, 2, dtype=np.float32) / np.float32(HD)))
    pos = np.arange(T, dtype=np.float32)
    freqs = np.outer(pos, inv_freq).astype(np.float32)          # (T, 32)
    emb = np.concatenate([freqs, freqs], axis=-1)               # (T, 64)
    cosT = np.cos(emb).T.astype(np.float32)                     # (64, T)
    sinT = np.sin(emb).T.astype(np.float32)
    cos2 = np.vstack([cosT, cosT]).copy()                       # (128, T) two heads
    sin2 = np.vstack([sinT, sinT]).copy()
    # sign-folded sin for the matmul-free rotate-half:
    #   rope(q)[d] = q[d]*cos[d] + rot(q)[d]*sin[d],
    #   rot(q)[base+d] = -q[base+32+d] (d<32) ; q[base+d-32] (d>=32)
    # so ts[base+0:32] = raw[base+32:64] * (-sin[base+0:32]) and
    #    ts[base+32:64] = raw[base+0:32] * (+sin[base+32:64]).
    sinadj = sin2.copy()
    for base in (0, 32, 64, 96):
        if (base // 32) % 2 == 0:
            sinadj[base:base + 32] = -sinadj[base:base + 32]
    return cos2, sin2, sinadj


def _r2_np():
    # qrot[d] = -q[d+32] (d<32) ; q[d-32] (d>=32), per 64-row block.
    # matmul computes out[d, t] = sum_k r2[k, d] q[k, t]
    r2 = np.zeros((128, 128), dtype=np.float32)
    for base in (0, 64):
        for d in range(32):
            r2[base + d + 32, base + d] = -1.0
            r2[base + d, base + d + 32] = 1.0
    return r2


def _masks_np():
    # tri[j, ql] = 1 if key j may attend query ql within a diagonal block
    j = np.arange(128)[:, None]
    ql = np.arange(128)[None, :]
    return (j <= ql).astype(np.float32)                         # [128, 128]


def build_program():
    import concourse.bass as bass
    import concourse.tile as tile
    from concourse import bacc, mybir
    from contextlib import ExitStack

    BF = _bf16()
    f32 = mybir.dt.float32
    bf16 = mybir.dt.bfloat16

    nc = bacc.Bacc(None, target_bir_lowering=False, debug=False)

    # xT pre-tiled on host: xTr[kc, tch, p, t] = x[b].T[kc*128+p, tch*512+t]
    xT = nc.declare_dram_parameter("xT", [D // 128, T // 512, 128, 512], bf16, isOutput=False)
    # weights pre-tiled on host to partition-major so each load is one
    # contiguous descriptor per partition (4x fewer descriptors -> 2x faster
    # serial DMA at the head): wq[p, kc, c] = W_qkv[kc*128+p, c]
    wq = nc.declare_dram_parameter("wq", [128, D // 128, 256], bf16, isOutput=False)
    wk = nc.declare_dram_parameter("wk", [128, D // 128, 256], bf16, isOutput=False)
    wv = nc.declare_dram_parameter("wv", [128, D // 128, 256], bf16, isOutput=False)
    wp = nc.declare_dram_parameter("wp", [128, 2, D], bf16, isOutput=False)
    # yT tiled: yTr[ech, tch, p, t] = yT_partial[ech*128+p, tch*512+t], bf16
    yT = nc.declare_dram_parameter("yT", [8, T // 512, 128, 512], bf16, isOutput=True)

    cos2_np, sin2_np, _sinadj_np = _rope_tables_np()
    cos_d = nc.inline_tensor(cos2_np.astype(BF), name="cos2")
    sin_d = nc.inline_tensor(sin2_np.astype(BF), name="sin2")
    r2_d = nc.inline_tensor(_r2_np().astype(BF), name="r2")
    masks_d = nc.inline_tensor(_masks_np().astype(BF), name="masks")

    NT = T // 512            # 4 t-chunks
    NJ = T // 128            # 16 key blocks
    KC = D // 128            # 8 contraction chunks
    Exp = mybir.ActivationFunctionType.Exp

    with tile.TileContext(nc) as tc, ExitStack() as ctx:
        # --- persistent SBUF ---
        wts = ctx.enter_context(tc.tile_pool(name="wts", bufs=1))
        packs = ctx.enter_context(tc.tile_pool(name="packs", bufs=1))
        consts = ctx.enter_context(tc.tile_pool(name="consts", bufs=1))

        # --- working pools (xts first: its loads gate the first matmuls) ---
        xts = ctx.enter_context(tc.tile_pool(name="xts", bufs=2))

        wq_sb = wts.tile([128, KC, 256], bf16, tag="wq")
        wk_sb = wts.tile([128, KC, 256], bf16, tag="wk")
        wv_sb = wts.tile([128, KC, 256], bf16, tag="wv")
        wp_sb = wts.tile([128, 2, 1024], bf16, tag="wp")
        xt_tiles = {}

        def prefetch_xt(tch):
            if tch >= NT or tch in xt_tiles:
                return
            xt = xts.tile([128, KC, 512], bf16, tag="xt", name=f"xt{tch}")
            # split so the first accumulation chunks start sooner
            nsplit = 4 if tch == 0 else 2
            step = KC // nsplit
            for s in range(nsplit):
                nc.sync.dma_start(
                    out=xt[:, s * step:(s + 1) * step, :],
                    in_=xT[s * step:(s + 1) * step, tch].rearrange("k p t -> p k t"))
            xt_tiles[tch] = xt

        cos_sb = consts.tile([128, T], bf16, tag="cos")
        sin_sb = consts.tile([128, T], bf16, tag="sin")
        r2_sb = consts.tile([128, 128], bf16, tag="r2")
        masks_sb = consts.tile([128, 128], bf16, tag="masks")
        # The head is serial-DMA-bound: issue in consumption order across
        # BOTH descriptor generators (sync -> HWDGE, gpsimd -> Pool SWDGE).
        xt0 = xts.tile([128, KC, 512], bf16, tag="xt", name="xt0")
        xt_tiles[0] = xt0
        nc.sync.dma_start(out=wq_sb[:, 0:2, :], in_=wq[:, 0:2, :])
        nc.gpsimd.dma_start(out=r2_sb[:], in_=r2_d[:, :])
        nc.sync.dma_start(out=wq_sb[:, 2:8, :], in_=wq[:, 2:8, :])
        nc.gpsimd.dma_start(out=xt0[:, 4:6, :], in_=xT[4:6, 0].rearrange("k p t -> p k t"))
        nc.sync.dma_start(out=xt0[:, 0:2, :], in_=xT[0:2, 0].rearrange("k p t -> p k t"))
        nc.gpsimd.dma_start(out=xt0[:, 6:8, :], in_=xT[6:8, 0].rearrange("k p t -> p k t"))
        nc.sync.dma_start(out=xt0[:, 2:4, :], in_=xT[2:4, 0].rearrange("k p t -> p k t"))
        nc.sync.dma_start(out=wk_sb[:], in_=wk[:, :, :])
        nc.gpsimd.dma_start(out=cos_sb[:], in_=cos_d[:, :])
        nc.gpsimd.dma_start(out=sin_sb[:], in_=sin_d[:, :])
        nc.gpsimd.dma_start(out=wv_sb[:], in_=wv[:, :, :])
        nc.gpsimd.dma_start(out=masks_sb[:], in_=masks_d[:, :])
        nc.gpsimd.dma_start(out=wp_sb[:], in_=wp[:, :, :])
        # xt1 queued last on the Pool SWDGE: the serial DMA engine drains in
        # arrival order, so issuing it any earlier starves the const loads
        xt1 = xts.tile([128, KC, 512], bf16, tag="xt", name="xt1")
        xt_tiles[1] = xt1
        for s in range(2):
            nc.gpsimd.dma_start(
                out=xt1[:, 4 * s:4 * (s + 1), :],
                in_=xT[4 * s:4 * (s + 1), 1].rearrange("k p t -> p k t"))

        # packs: [128, pack p, T] so one rope add can write both packs' slices
        qt_sb = packs.tile([128, 2, T], bf16, tag="qt")
        kt_sb = packs.tile([128, 2, T], bf16, tag="kt")
        ot_sb = packs.tile([128, 2, T], bf16, tag="ot")
        v_sb = packs.tile([128, NJ, HPC * (HD + 1)], bf16, tag="vaug")

        # ones columns of v_aug (fused softmax denominator)
        v_view = v_sb[:].rearrange("p j (h c) -> p j h c", h=HPC)
        nc.vector.memset(v_view[:, :, :, HD:HD + 1], 1.0)
        ones64 = consts.tile([1, 64], bf16, tag="ones64")
        nc.vector.memset(ones64[:], 1.0)

        # --- working pools ---
        tmps = ctx.enter_context(tc.tile_pool(name="tmps", bufs=3))
        pts = ctx.enter_context(tc.tile_pool(name="pts", bufs=8))
        outs = ctx.enter_context(tc.tile_pool(name="outs", bufs=3))
        smalls = ctx.enter_context(tc.tile_pool(name="smalls", bufs=2))

        # 8 PSUM banks: stp pairs 2x2, oacc pair 2, aux (acc/rot/vacc/pc) 2x1
        psSTP = ctx.enter_context(tc.tile_pool(name="psSTP", bufs=2, space="PSUM"))
        psOAC = ctx.enter_context(tc.tile_pool(name="psOAC", bufs=1, space="PSUM"))
        psAUX = ctx.enter_context(tc.tile_pool(name="psAUX", bufs=2, space="PSUM"))

        def emit_rope(tch, raw, dst, p):
            # rope(q) = q*cos + rot(q)*sin; the rotate-half must cross
            # partitions, which only the PE (or DMA) may do on HW, so it is
            # a [128,128] block-diag matmul against the bf16 raw copy.
            ts = slice(tch * 512, (tch + 1) * 512)
            rot = psAUX.tile([128, 512], f32, tag="aux", name="rot")
            nc.tensor.matmul(rot[:], r2_sb[:], raw[:], start=True, stop=True)
            tc_t = tmps.tile([128, 512], bf16, tag="tc", name="tc_t")
            nc.vector.tensor_mul(tc_t[:], raw[:], cos_sb[:, ts])
            ts_t = tmps.tile([128, 512], bf16, tag="ts", name="ts_t")
            with nc.allow_low_precision(reason="bf16 rope"):
                nc.vector.tensor_mul(ts_t[:], rot[:], sin_sb[:, ts])
                nc.vector.tensor_add(dst[:, p, ts], tc_t[:], ts_t[:])

        def emit_o(oacc, p, kj_max, kj, pt, lo):
            for hh in range(2):
                hl = 2 * p + hh
                nc.tensor.matmul(
                    oacc[0:65, 512 * hh + lo:512 * hh + 512],
                    v_sb[:, kj, 65 * hl:65 * hl + 65],
                    pt[:, 512 * hh + lo:512 * hh + 512],
                    start=(kj == 0), stop=(kj == kj_max - 1),
                )

        def attn_step(qi, p, oacc, pending, kj, kj_max):
            """One kj iteration: st pair matmuls -> single exp -> (mask),
            software-pipelined so the PE never queues behind its own exp."""
            r = kj - 4 * qi          # >= 0 on the causal diagonal
            lo = max(r, 0) * 128     # first valid column
            stp = psSTP.tile([128, 1024], f32, tag="stp", name="stp")
            for hh in range(2):
                off = 64 * hh
                nc.tensor.matmul(
                    stp[:, 512 * hh + lo:512 * hh + 512],
                    kt_sb[off:off + 64, p, kj * 128:(kj + 1) * 128],
                    qt_sb[off:off + 64, p, qi * 512 + lo:(qi + 1) * 512],
                    start=True, stop=True,
                )
            pt = pts.tile([128, 1024], bf16, tag="pt", name="pt")
            w = 512 - lo
            st3 = stp[:].rearrange("p (h t) -> p h t", h=2)[:, :, lo:512]
            pt3 = pt[:].rearrange("p (h t) -> p h t", h=2)[:, :, lo:512]
            with nc.allow_low_precision(reason="bf16 softmax weights"):
                nc.scalar.activation(pt3, st3, Exp, scale=0.125)
            if r >= 0:
                meng = nc.gpsimd if (USE_POOL_MASKS and qi < 3) else nc.vector
                for hh in range(2):
                    s = slice(512 * hh + lo, 512 * hh + lo + 128)
                    with nc.allow_low_precision(reason="bf16 mask"):
                        meng.tensor_mul(pt[:, s], pt[:, s], masks_sb[:])
            pending.append((kj, pt, lo))
            # keep the window tight on the very last stream: its PVs gate the
            # final norm -> C(3) tail, so don't let them queue up
            window = 1 if (qi == NT - 1 and p == 1) else 4
            while len(pending) > window:
                kj0, pt0, lo0 = pending.pop(0)
                emit_o(oacc, p, kj_max, kj0, pt0, lo0)

        def attn_norm(qi, p, oacc):
            qs = slice(qi * 512, (qi + 1) * 512)
            if qi == NT - 1 and p == 1:
                # tail stream: C(3) waits on this chain and nothing waits on
                # the oacc banks, so skip oct; emit recips, then pbs, then
                # muls so the per-head chains pipeline across DVE and Pool
                # instead of serializing on the in-order DVE queue
                rcps, bcls = [], []
                for hh in range(2):
                    s = slice(512 * hh, 512 * hh + 512)
                    rcp = smalls.tile([1, 512], bf16, tag="rcp", name="rcp")
                    with nc.allow_low_precision(reason="bf16 softmax recip"):
                        nc.vector.reciprocal(rcp[:], oacc[64:65, s])
                    rcps.append(rcp)
                for hh in range(2):
                    bcl = smalls.tile([64, 512], bf16, tag="bcl", name="bcl")
                    nc.gpsimd.partition_broadcast(bcl[0:64, :], rcps[hh][0:1, :])
                    bcls.append(bcl)
                for hh in range(2):
                    s = slice(512 * hh, 512 * hh + 512)
                    off = 64 * hh
                    with nc.allow_low_precision(reason="bf16 attn out"):
                        nc.vector.tensor_mul(ot_sb[off:off + 64, p, qs],
                                             oacc[0:64, s], bcls[hh][0:64, :])
                return
            # one bf16 copy releases the oacc PSUM pair early (the next
            # stream's PV accumulation is waiting on these 2 banks); the
            # ot mul then has exactly one PSUM operand (bc_ps), which is the
            # HW limit for TensorTensor.
            recip2 = smalls.tile([1, 1024], bf16, tag="recip2", name="recip2")
            with nc.allow_low_precision(reason="bf16 softmax recip"):
                nc.vector.reciprocal(recip2[:], oacc[64:65, :])
            oct = smalls.tile([65, 1024], bf16, tag="oct", name="oct")
            with nc.allow_low_precision(reason="bf16 attn out"):
                nc.vector.tensor_copy(oct[:], oacc[0:65, :])
            # partition-broadcast of the recip row: gpsimd custom op when
            # allowed, else a ones-column matmul on the PE (the only other
            # engine that may cross partitions on HW)
            if USE_POOL_PB:
                bc = smalls.tile([64, 1024], bf16, tag="bc", name="bc")
                for hh in range(2):
                    s = slice(512 * hh, 512 * hh + 512)
                    nc.gpsimd.partition_broadcast(bc[0:64, s], recip2[0:1, s])
                for hh in range(2):
                    off = 64 * hh
                    s = slice(512 * hh, 512 * hh + 512)
                    with nc.allow_low_precision(reason="bf16 attn out"):
                        nc.vector.tensor_mul(ot_sb[off:off + 64, p, qs],
                                             oct[0:64, s], bc[0:64, s])
            else:
                for hh in range(2):
                    s = slice(512 * hh, 512 * hh + 512)
                    bc_ps = psAUX.tile([64, 512], f32, tag="aux", name="bc_ps")
                    nc.tensor.matmul(bc_ps[:], ones64[:], recip2[0:1, s],
                                     start=True, stop=True)
                    off = 64 * hh
                    with nc.allow_low_precision(reason="bf16 attn out"):
                        nc.vector.tensor_mul(ot_sb[off:off + 64, p, qs],
                                             oct[0:64, s], bc_ps[:])

        def a_unit_list(tch):
            """A(tch) as a list of emission closures (proj groups, V blocks).
            The rope skew chains across units via `state`."""
            if tch >= NT:
                return []
            state = {"pend": None}

            def start():
                prefetch_xt(tch)

            def qk_group(w_sb, dst, p):
                def emit():
                    xt = xt_tiles[tch]
                    acc = psAUX.tile([128, 512], f32, tag="aux", name=f"acc{tch}_{p}")
                    for kc in range(KC):
                        nc.tensor.matmul(
                            acc[:],
                            w_sb[:, kc, 128 * p:128 * (p + 1)],
                            xt[:, kc, :],
                            start=(kc == 0), stop=(kc == KC - 1),
                        )
                    raw = tmps.tile([128, 512], bf16, tag="raw", name="raw")
                    ceng = nc.gpsimd if USE_POOL_COPIES else nc.vector
                    with nc.allow_low_precision(reason="bf16 q/k"):
                        ceng.tensor_copy(raw[:], acc[:])
                    if state["pend"] is not None:
                        emit_rope(*state["pend"])
                    state["pend"] = (tch, raw, dst, p)
                return emit

            def v_block(jb):
                def emit():
                    xt = xt_tiles[tch]
                    if state["pend"] is not None:
                        emit_rope(*state["pend"])
                        state["pend"] = None
                    jbg = tch * 4 + jb
                    vacc = psAUX.tile([128, 256], f32, tag="aux", name=f"vacc{jbg}")
                    for kc in range(KC):
                        nc.tensor.matmul(
                            vacc[:],
                            xt[:, kc, 128 * jb:128 * (jb + 1)],
                            wv_sb[:, kc, :],
                            start=(kc == 0), stop=(kc == KC - 1),
                        )
                    veng = nc.gpsimd if USE_POOL_COPIES else nc.vector
                    with nc.allow_low_precision(reason="bf16 v"):
                        veng.tensor_copy(
                            v_view[:, jbg, :, 0:HD],
                            vacc[:].rearrange("p (h c) -> p h c", h=HPC),
                        )
                    if jb == 3:
                        xt_tiles.pop(tch)
                        # prefetch the next chunk only now: issuing it any
                        # earlier puts its transfers ahead of the (critical)
                        # cos/sin/wv const loads in the serial DMA queue
                        prefetch_xt(tch + 1)
                return emit

            units = [start]
            units.append(qk_group(wq_sb, qt_sb, 0))
            units.append(qk_group(wq_sb, qt_sb, 1))
            units.append(qk_group(wk_sb, kt_sb, 0))
            units.append(qk_group(wk_sb, kt_sb, 1))
            for jb in range(4):
                units.append(v_block(jb))
            return units

        def c_unit_list(tch):
            if tch < 0:
                return []
            ts = slice(tch * 512, (tch + 1) * 512)

            def c_block(ech):
                def emit():
                    pc = psAUX.tile([128, 512], f32, tag="aux", name=f"pc{tch}_{ech}")
                    for kd in range(2):
                        nc.tensor.matmul(
                            pc[:],
                            wp_sb[:, kd, ech * 128:(ech + 1) * 128],
                            ot_sb[:, kd, ts],
                            start=(kd == 0), stop=(kd == 1),
                        )
                    oc = outs.tile([128, 512], bf16, tag="oc", name="oc")
                    oeng = nc.gpsimd if (USE_POOL_COPIES and tch == 0) else nc.vector
                    with nc.allow_low_precision(reason="bf16 partial out"):
                        oeng.tensor_copy(oc[:], pc[:])
                    nc.sync.dma_start(out=yT[ech, tch], in_=oc[:])
                return emit
            return [c_block(e) for e in range(8)]

        def c_final(tch):
            """Tail-specific C: nothing else is left for the PE, so pipeline
            pairs of out-proj blocks through the (now free) 2-bank stp slots,
            alternating the PSUM->SBUF copy between DVE and Pool."""
            ts = slice(tch * 512, (tch + 1) * 512)
            # contract per (pack, head-half) in 64-partition sub-matmuls:
            # twice the PE cycles (it is idle here anyway), but each sub-mm
            # only waits on ONE norm mul, so the final ot write gates just
            # the 4 (pair, kd=1, hh=1) matmuls instead of everything.
            for pair in range(4):
                pc2 = psSTP.tile([128, 1024], f32, tag="stp", name=f"pcf{pair}")
                for kd in range(2):
                    for hh in range(2):
                        off = 64 * hh
                        for half in range(2):
                            ech = 2 * pair + half
                            nc.tensor.matmul(
                                pc2[:, 512 * half:512 * half + 512],
                                wp_sb[off:off + 64, kd, ech * 128:(ech + 1) * 128],
                                ot_sb[off:off + 64, kd, ts],
                                start=(kd == 0 and hh == 0),
                                stop=(kd == 1 and hh == 1),
                            )
                oc2 = outs.tile([128, 1024], bf16, tag="oc2", name="oc2")
                # split the PSUM->SBUF copy between DVE and the (idle at the
                # tail) scalar engine so the pcf pipeline is not
                # copy-throughput-limited; the second half is finished by the
                # later stop-matmul, so it goes on the lower-latency DVE
                with nc.allow_low_precision(reason="bf16 partial out"):
                    nc.scalar.activation(oc2[:, 0:512], pc2[:, 0:512],
                                         mybir.ActivationFunctionType.Copy)
                    nc.vector.tensor_copy(oc2[:, 512:1024], pc2[:, 512:1024])
                nc.sync.dma_start(out=yT[2 * pair, tch], in_=oc2[:, 0:512])
                nc.sync.dma_start(out=yT[2 * pair + 1, tch], in_=oc2[:, 512:1024])

        def interleave_fill(a_units, c_units):
            """start + qk units first (rope latency chain), C blocks woven
            between the chunky A units so aux-psum slots alternate."""
            fill = []
            a = list(a_units)
            c = list(c_units)
            if a:
                fill.append(a.pop(0))       # start (DMA prefetch) first
            while a or c:
                if a:
                    fill.append(a.pop(0))
                if c:
                    fill.append(c.pop(0))
            return fill

        def b_emit(qi, fill_units):
            """Emit B(qi)'s attention streams, sprinkling `fill_units`
            (A(qi+1) / C(qi-1) closures) between kj iterations so the PE
            always has independent matmul work queued behind exp waits."""
            kj_max = 4 * (qi + 1)
            n_slots = 2 * (kj_max + 1)
            fill = list(fill_units)
            n_fill = len(fill)
            slot = [0]

            def maybe_fill():
                # spread the n_fill units evenly across the n_slots slots
                k = (n_fill * (slot[0] + 1)) // n_slots - (n_fill * slot[0]) // n_slots
                for _ in range(k):
                    if fill:
                        fill.pop(0)()
                slot[0] += 1

            for p in range(2):
                oacc = psOAC.tile([128, 1024], f32, tag="oaccp", name=f"oacc{qi}_{p}")
                pending = []
                for kj in range(kj_max):
                    attn_step(qi, p, oacc, pending, kj, kj_max)
                    maybe_fill()
                while pending:
                    kj0, pt0, lo0 = pending.pop(0)
                    emit_o(oacc, p, kj_max, kj0, pt0, lo0)
                attn_norm(qi, p, oacc)
                maybe_fill()
            while fill:
                fill.pop(0)()

        # Fill rebalance: the later B(i) streams have the most ACT (exp) work
        # relative to their own PE work, so the out-proj C units are pushed as
        # late as their ot dependencies allow: B(3) gets C(1)+C(2) while the
        # A units go one chunk ahead as before.
        for u in a_unit_list(0):
            u()
        b_emit(0, interleave_fill(a_unit_list(1), []))
        b_emit(1, interleave_fill(a_unit_list(2), []))
        b_emit(2, interleave_fill(a_unit_list(3), c_unit_list(0)))
        b_emit(3, interleave_fill([], c_unit_list(1) + c_unit_list(2)))
        c_final(NT - 1)

    nc.compile()
    return nc


def get_program():
    global _PROGRAM
    if _PROGRAM is None:
        _PROGRAM = build_program()
    return _PROGRAM


def make_in_maps(x, W_qkv, W_proj):
    BF = _bf16()
    x = np.asarray(x, dtype=np.float32)
    W_qkv = np.asarray(W_qkv, dtype=np.float32)
    W_proj = np.asarray(W_proj, dtype=np.float32)
    in_maps = []
    xtr = {}
    for b in range(B):
        xt = x[b].T.reshape(D // 128, 128, T // 512, 512)
        xtr[b] = np.ascontiguousarray(xt.transpose(0, 2, 1, 3)).astype(BF)
    def tile_w(w):  # [D, 256] -> [128, D//128, 256] partition-major
        return np.ascontiguousarray(
            w.reshape(D // 128, 128, w.shape[1]).transpose(1, 0, 2)).astype(BF)

    for core in range(NCORES):
        b, g = divmod(core, 4)
        cs = slice(g * 256, (g + 1) * 256)
        in_maps.append({
            "xT": xtr[b],
            "wq": tile_w(W_qkv[:, 0 * D:1 * D][:, cs]),
            "wk": tile_w(W_qkv[:, 1 * D:2 * D][:, cs]),
            "wv": tile_w(W_qkv[:, 2 * D:3 * D][:, cs]),
            "wp": np.ascontiguousarray(
                W_proj[cs, :].reshape(2, 128, D).transpose(1, 0, 2)).astype(BF),
        })
    return in_maps


def gather_output(results):
    out = np.empty((B, T, D), dtype=np.float32)
    for b in range(B):
        acc = np.asarray(results[4 * b]["yT"]).astype(np.float32)
        for g in range(1, 4):
            acc += np.asarray(results[4 * b + g]["yT"]).astype(np.float32)
        # (ech, tch, p, t) -> yT (D, T) -> transpose to (T, D)
        yt = acc.transpose(0, 2, 1, 3).reshape(D, T)
        out[b] = yt.T
    return out


def kernel(x, W_qkv, W_proj, key_padding_mask=None, **_ignored):
    # key_padding_mask is all-True per the problem spec (fill: ones) -> no-op.
    from concourse.bass_utils import run_bass_kernel_spmd

    nc = get_program()
    in_maps = make_in_maps(x, W_qkv, W_proj)
    res = run_bass_kernel_spmd(nc, in_maps, list(range(NCORES)))
    return gather_output(res.results)


# revision 68
# speedup vs baseline: 1.2686x; 1.0119x over previous
"""Causal self-attention with RoPE on 8 trn2 NeuronCores.

Problem: B=2, T=2048, D=1024, H=16 heads, head_dim=64, fp32 in/out.
Sharding: core = b*4 + g  (data parallel over batch, tensor parallel over
head groups of 4). Each core computes its 4 heads' attention plus the
row-slice of the output projection; the host sums the 4 partial Y^T per
batch (bf16 partials, f32 accumulate) and transposes back.

All matmul inputs are bf16 (same PE rate as f32r at >=256 moving cols, no
4x penalty on the <256-col diagonal tiles, half the DMA bytes, and 2x DVE
throughput on the all-bf16 elementwise ops). PSUM accumulation stays f32.

Per-core dataflow (everything transposed so matmuls contract on partitions):
  xT (1024, 2048)  =  x[b].T
  QT/KT packs: qt_sb[128, 2, T] (pack p = 2 heads of 64 rows)
  RoPE: QT' = QT*cos + (R2 @ QT)*sin   (R2 = block-diag rotate-half matrix)
  V_aug [128, 16, 260]: V natural layout per key block, 4 heads x (64 dims
      + ones column) -> fused softmax denominator.
  S pair tile [keys 128, 2*512] = both heads of a pack in one 2-bank PSUM
      tile; ONE exp activation covers both halves (halves ACT op count).
  P = exp(S * 0.125) -> bf16 SBUF, diag blocks masked on GPSIMD.
  Oacc pair [65, 2*512] += V_aug^T @ P  (row 64 = denominator)
  norm: DVE reciprocal -> GPSIMD partition_broadcast -> DVE mul -> ot bf16
  Y^T partial = Wp-slice^T @ O^T packs -> bf16 DRAM out (SP-queue DMA).

Phases are interleaved: B(i)'s st->exp->PV stream is ACT-latency-bound, so
A(i+1) (projections) and C(i-1) (out-proj) PE work is sprinkled into its
kj slots to keep the PE busy while the scalar engine catches up.
"""

import sys
import numpy as np

sys.path.insert(0, "/opt/trn_rl_repo")

B, T, D, H = 2, 2048, 1024, 16
HD = 64          # head dim
HPC = 4          # heads per core
NCORES = 8
ROPE_BASE = 10000.0

_PROGRAM = None  # cached compiled program

# GPSIMD (Pool-engine) offloads: flipped on only after HW verifier approval,
# since the BIR verifier enforces rules CoreSim does not.
USE_POOL_COPIES = False   # dead: "GPSIMD Instructions cannot access PSUM"
USE_POOL_MASKS = True     # SBUF tensor_mul (causal masks) on gpsimd
USE_POOL_PB = True        # gpsimd partition_broadcast for the softmax recip


def _bf16():
    import ml_dtypes
    return ml_dtypes.bfloat16


def _rope_tables_np():
    inv_freq = 1.0 / (ROPE_BASE ** (np.arange(0, HD, 2, dtype=np.float32) / np.float32(HD)))
    pos = np.arange(T, dtype=np.float32)
    freqs = np.outer(pos, inv_freq).astype(np.float32)          # (T, 32)
    emb = np.concatenate([freqs, freqs], axis=-1)               # (T, 64)
    cosT = np.cos(emb).T.astype(np.float32)                     # (64, T)
    sinT = np.sin(emb).T.astype(np.float32)
    cos2 = np.vstack([cosT, cosT]).copy()                       # (128, T) two heads
    sin2 = np.vstack([sinT, sinT]).copy()
    # sign-folded sin for the matmul-free rotate-half:
    #   rope(q)[d] = q[d]*cos[d] + rot(q)[d]*sin[d],
    #   rot(q)[base+d] = -q[base+32+d] (d<32) ; q[base+d-32] (d>=32)
    # so ts[base+0:32] = raw[base+32:64] * (-sin[base+0:32]) and
    #    ts[base+32:64] = raw[base+0:32] * (+sin[base+32:64]).
    sinadj = sin2.copy()
    for base in (0, 32, 64, 96):
        if (base // 32) % 2 == 0:
            sinadj[base:base + 32] = -sinadj[base:base + 32]
    return cos2, sin2, sinadj


def _r2_np():
    # qrot[d] = -q[d+32] (d<32) ; q[d-32] (d>=32), per 64-row block.
    # matmul computes out[d, t] = sum_k r2[k, d] q[k, t]
    r2 = np.zeros((128, 128), dtype=np.float32)
    for base in (0, 64):
        for d in range(32):
            r2[base + d + 32, base + d] = -1.0
            r2[base + d, base + d + 32] = 1.0
    return r2


def _masks_np():
    # tri[j, ql] = 1 if key j may attend query ql within a diagonal block
    j = np.arange(128)[:, None]
    ql = np.arange(128)[None, :]
    return (j <= ql).astype(np.float32)                         # [128, 128]


def build_program():
    import concourse.bass as bass
    import concourse.tile as tile
    from concourse import bacc, mybir
    from contextlib import ExitStack

    BF = _bf16()
    f32 = mybir.dt.float32
    bf16 = mybir.dt.bfloat16

    nc = bacc.Bacc(None, target_bir_lowering=False, debug=False)

    # xT pre-tiled on host: xTr[kc, tch, p, t] = x[b].T[kc*128+p, tch*512+t]
    xT = nc.declare_dram_parameter("xT", [D // 128, T // 512, 128, 512], bf16, isOutput=False)
    # weights pre-tiled on host to partition-major so each load is one
    # contiguous descriptor per partition (4x fewer descriptors -> 2x faster
    # serial DMA at the head): wq[p, kc, c] = W_qkv[kc*128+p, c]
    wq = nc.declare_dram_parameter("wq", [128, D // 128, 256], bf16, isOutput=False)
    wk = nc.declare_dram_parameter("wk", [128, D // 128, 256], bf16, isOutput=False)
    wv = nc.declare_dram_parameter("wv", [128, D // 128, 256], bf16, isOutput=False)
    wp = nc.declare_dram_parameter("wp", [128, 2, D], bf16, isOutput=False)
    # yT tiled: yTr[ech, tch, p, t] = yT_partial[ech*128+p, tch*512+t], bf16
    yT = nc.declare_dram_parameter("yT", [8, T // 512, 128, 512], bf16, isOutput=True)

    cos2_np, sin2_np, _sinadj_np = _rope_tables_np()
    cos_d = nc.inline_tensor(cos2_np.astype(BF), name="cos2")
    sin_d = nc.inline_tensor(sin2_np.astype(BF), name="sin2")
    r2_d = nc.inline_tensor(_r2_np().astype(BF), name="r2")
    masks_d = nc.inline_tensor(_masks_np().astype(BF), name="masks")

    NT = T // 512            # 4 t-chunks
    NJ = T // 128            # 16 key blocks
    KC = D // 128            # 8 contraction chunks
    Exp = mybir.ActivationFunctionType.Exp

    with tile.TileContext(nc) as tc, ExitStack() as ctx:
        # --- persistent SBUF ---
        wts = ctx.enter_context(tc.tile_pool(name="wts", bufs=1))
        packs = ctx.enter_context(tc.tile_pool(name="packs", bufs=1))
        consts = ctx.enter_context(tc.tile_pool(name="consts", bufs=1))

        # --- working pools (xts first: its loads gate the first matmuls) ---
        xts = ctx.enter_context(tc.tile_pool(name="xts", bufs=2))

        wq_sb = wts.tile([128, KC, 256], bf16, tag="wq")
        wk_sb = wts.tile([128, KC, 256], bf16, tag="wk")
        wv_sb = wts.tile([128, KC, 256], bf16, tag="wv")
        wp_sb = wts.tile([128, 2, 1024], bf16, tag="wp")
        xt_tiles = {}

        def prefetch_xt(tch):
            if tch >= NT or tch in xt_tiles:
                return
            xt = xts.tile([128, KC, 512], bf16, tag="xt", name=f"xt{tch}")
            # split so the first accumulation chunks start sooner
            nsplit = 4 if tch == 0 else 2
            step = KC // nsplit
            for s in range(nsplit):
                nc.sync.dma_start(
                    out=xt[:, s * step:(s + 1) * step, :],
                    in_=xT[s * step:(s + 1) * step, tch].rearrange("k p t -> p k t"))
            xt_tiles[tch] = xt

        cos_sb = consts.tile([128, T], bf16, tag="cos")
        sin_sb = consts.tile([128, T], bf16, tag="sin")
        r2_sb = consts.tile([128, 128], bf16, tag="r2")
        masks_sb = consts.tile([128, 128], bf16, tag="masks")
        # The head is serial-DMA-bound: issue in consumption order across
        # BOTH descriptor generators (sync -> HWDGE, gpsimd -> Pool SWDGE).
        xt0 = xts.tile([128, KC, 512], bf16, tag="xt", name="xt0")
        xt_tiles[0] = xt0
        nc.sync.dma_start(out=wq_sb[:, 0:2, :], in_=wq[:, 0:2, :])
        nc.gpsimd.dma_start(out=r2_sb[:], in_=r2_d[:, :])
        nc.sync.dma_start(out=wq_sb[:, 2:8, :], in_=wq[:, 2:8, :])
        nc.gpsimd.dma_start(out=xt0[:, 4:6, :], in_=xT[4:6, 0].rearrange("k p t -> p k t"))
        nc.sync.dma_start(out=xt0[:, 0:2, :], in_=xT[0:2, 0].rearrange("k p t -> p k t"))
        nc.gpsimd.dma_start(out=xt0[:, 6:8, :], in_=xT[6:8, 0].rearrange("k p t -> p k t"))
        nc.sync.dma_start(out=xt0[:, 2:4, :], in_=xT[2:4, 0].rearrange("k p t -> p k t"))
        nc.sync.dma_start(out=wk_sb[:], in_=wk[:, :, :])
        nc.gpsimd.dma_start(out=cos_sb[:], in_=cos_d[:, :])
        nc.gpsimd.dma_start(out=sin_sb[:], in_=sin_d[:, :])
        nc.gpsimd.dma_start(out=wv_sb[:], in_=wv[:, :, :])
        nc.gpsimd.dma_start(out=masks_sb[:], in_=masks_d[:, :])
        nc.gpsimd.dma_start(out=wp_sb[:], in_=wp[:, :, :])
        # xt1 queued last on the Pool SWDGE: the serial DMA engine drains in
        # arrival order, so issuing it any earlier starves the const loads
        xt1 = xts.tile([128, KC, 512], bf16, tag="xt", name="xt1")
        xt_tiles[1] = xt1
        for s in range(2):
            nc.gpsimd.dma_start(
                out=xt1[:, 4 * s:4 * (s + 1), :],
                in_=xT[4 * s:4 * (s + 1), 1].rearrange("k p t -> p k t"))

        # packs: [128, pack p, T] so one rope add can write both packs' slices
        qt_sb = packs.tile([128, 2, T], bf16, tag="qt")
        kt_sb = packs.tile([128, 2, T], bf16, tag="kt")
        ot_sb = packs.tile([128, 2, T], bf16, tag="ot")
        v_sb = packs.tile([128, NJ, HPC * (HD + 1)], bf16, tag="vaug")

        # ones columns of v_aug (fused softmax denominator)
        v_view = v_sb[:].rearrange("p j (h c) -> p j h c", h=HPC)
        nc.vector.memset(v_view[:, :, :, HD:HD + 1], 1.0)
        ones64 = consts.tile([1, 64], bf16, tag="ones64")
        nc.vector.memset(ones64[:], 1.0)
        # dummy activation pulls the Exp table load (1.3us) into the
        # DMA-bound head instead of B(0)'s first softmax
        warm = consts.tile([1, 2], bf16, tag="warm")
        with nc.allow_low_precision(reason="act table warmup"):
            nc.scalar.activation(warm[:], ones64[0:1, 0:2], Exp)

        # --- working pools ---
        tmps = ctx.enter_context(tc.tile_pool(name="tmps", bufs=3))
        pts = ctx.enter_context(tc.tile_pool(name="pts", bufs=8))
        outs = ctx.enter_context(tc.tile_pool(name="outs", bufs=3))
        smalls = ctx.enter_context(tc.tile_pool(name="smalls", bufs=2))

        # 8 PSUM banks: stp pairs 2x2, oacc pair 2, aux (acc/rot/vacc/pc) 2x1
        psSTP = ctx.enter_context(tc.tile_pool(name="psSTP", bufs=2, space="PSUM"))
        psOAC = ctx.enter_context(tc.tile_pool(name="psOAC", bufs=1, space="PSUM"))
        psAUX = ctx.enter_context(tc.tile_pool(name="psAUX", bufs=2, space="PSUM"))

        def emit_rope(tch, raw, dst, p):
            # rope(q) = q*cos + rot(q)*sin; the rotate-half must cross
            # partitions, which only the PE (or DMA) may do on HW, so it is
            # a [128,128] block-diag matmul against the bf16 raw copy.
            ts = slice(tch * 512, (tch + 1) * 512)
            rot = psAUX.tile([128, 512], f32, tag="aux", name="rot")
            nc.tensor.matmul(rot[:], r2_sb[:], raw[:], start=True, stop=True)
            tc_t = tmps.tile([128, 512], bf16, tag="tc", name="tc_t")
            nc.vector.tensor_mul(tc_t[:], raw[:], cos_sb[:, ts])
            ts_t = tmps.tile([128, 512], bf16, tag="ts", name="ts_t")
            with nc.allow_low_precision(reason="bf16 rope"):
                nc.vector.tensor_mul(ts_t[:], rot[:], sin_sb[:, ts])
                nc.vector.tensor_add(dst[:, p, ts], tc_t[:], ts_t[:])

        def emit_o(oacc, p, kj_max, kj, pt, lo):
            for hh in range(2):
                hl = 2 * p + hh
                nc.tensor.matmul(
                    oacc[0:65, 512 * hh + lo:512 * hh + 512],
                    v_sb[:, kj, 65 * hl:65 * hl + 65],
                    pt[:, 512 * hh + lo:512 * hh + 512],
                    start=(kj == 0), stop=(kj == kj_max - 1),
                )

        def attn_step(qi, p, oacc, pending, kj, kj_max):
            """One kj iteration: st pair matmuls -> single exp -> (mask),
            software-pipelined so the PE never queues behind its own exp."""
            r = kj - 4 * qi          # >= 0 on the causal diagonal
            lo = max(r, 0) * 128     # first valid column
            stp = psSTP.tile([128, 1024], f32, tag="stp", name="stp")
            for hh in range(2):
                off = 64 * hh
                nc.tensor.matmul(
                    stp[:, 512 * hh + lo:512 * hh + 512],
                    kt_sb[off:off + 64, p, kj * 128:(kj + 1) * 128],
                    qt_sb[off:off + 64, p, qi * 512 + lo:(qi + 1) * 512],
                    start=True, stop=True,
                )
            pt = pts.tile([128, 1024], bf16, tag="pt", name="pt")
            w = 512 - lo
            st3 = stp[:].rearrange("p (h t) -> p h t", h=2)[:, :, lo:512]
            pt3 = pt[:].rearrange("p (h t) -> p h t", h=2)[:, :, lo:512]
            with nc.allow_low_precision(reason="bf16 softmax weights"):
                nc.scalar.activation(pt3, st3, Exp, scale=0.125)
            if r >= 0:
                meng = nc.gpsimd if (USE_POOL_MASKS and qi < 3) else nc.vector
                for hh in range(2):
                    s = slice(512 * hh + lo, 512 * hh + lo + 128)
                    with nc.allow_low_precision(reason="bf16 mask"):
                        meng.tensor_mul(pt[:, s], pt[:, s], masks_sb[:])
            pending.append((kj, pt, lo))
            # keep the window tight on the very last stream: its PVs gate the
            # final norm -> C(3) tail, so don't let them queue up
            window = 1 if (qi == NT - 1 and p == 1) else 4
            while len(pending) > window:
                kj0, pt0, lo0 = pending.pop(0)
                emit_o(oacc, p, kj_max, kj0, pt0, lo0)

        def attn_norm(qi, p, oacc):
            qs = slice(qi * 512, (qi + 1) * 512)
            if qi == NT - 1 and p == 1:
                # tail stream: C(3) waits on this chain and nothing waits on
                # the oacc banks, so skip oct; emit recips, then pbs, then
                # muls so the per-head chains pipeline across DVE and Pool
                # instead of serializing on the in-order DVE queue
                rcps, bcls = [], []
                for hh in range(2):
                    s = slice(512 * hh, 512 * hh + 512)
                    rcp = smalls.tile([1, 512], bf16, tag="rcp", name="rcp")
                    with nc.allow_low_precision(reason="bf16 softmax recip"):
                        nc.vector.reciprocal(rcp[:], oacc[64:65, s])
                    rcps.append(rcp)
                for hh in range(2):
                    bcl = smalls.tile([64, 512], bf16, tag="bcl", name="bcl")
                    nc.gpsimd.partition_broadcast(bcl[0:64, :], rcps[hh][0:1, :])
                    bcls.append(bcl)
                for hh in range(2):
                    s = slice(512 * hh, 512 * hh + 512)
                    off = 64 * hh
                    with nc.allow_low_precision(reason="bf16 attn out"):
                        nc.vector.tensor_mul(ot_sb[off:off + 64, p, qs],
                                             oacc[0:64, s], bcls[hh][0:64, :])
                return
            # one bf16 copy releases the oacc PSUM pair early (the next
            # stream's PV accumulation is waiting on these 2 banks); the
            # ot mul then has exactly one PSUM operand (bc_ps), which is the
            # HW limit for TensorTensor.
            recip2 = smalls.tile([1, 1024], bf16, tag="recip2", name="recip2")
            with nc.allow_low_precision(reason="bf16 softmax recip"):
                nc.vector.reciprocal(recip2[:], oacc[64:65, :])
            oct = smalls.tile([65, 1024], bf16, tag="oct", name="oct")
            with nc.allow_low_precision(reason="bf16 attn out"):
                nc.vector.tensor_copy(oct[:], oacc[0:65, :])
            # partition-broadcast of the recip row: gpsimd custom op when
            # allowed, else a ones-column matmul on the PE (the only other
            # engine that may cross partitions on HW)
            if USE_POOL_PB:
                bc = smalls.tile([64, 1024], bf16, tag="bc", name="bc")
                for hh in range(2):
                    s = slice(512 * hh, 512 * hh + 512)
                    nc.gpsimd.partition_broadcast(bc[0:64, s], recip2[0:1, s])
                for hh in range(2):
                    off = 64 * hh
                    s = slice(512 * hh, 512 * hh + 512)
                    with nc.allow_low_precision(reason="bf16 attn out"):
                        nc.vector.tensor_mul(ot_sb[off:off + 64, p, qs],
                                             oct[0:64, s], bc[0:64, s])
            else:
                for hh in range(2):
                    s = slice(512 * hh, 512 * hh + 512)
                    bc_ps = psAUX.tile([64, 512], f32, tag="aux", name="bc_ps")
                    nc.tensor.matmul(bc_ps[:], ones64[:], recip2[0:1, s],
                                     start=True, stop=True)
                    off = 64 * hh
                    with nc.allow_low_precision(reason="bf16 attn out"):
                        nc.vector.tensor_mul(ot_sb[off:off + 64, p, qs],
                                             oct[0:64, s], bc_ps[:])

        def a_unit_list(tch):
            """A(tch) as a list of emission closures (proj groups, V blocks).
            The rope skew chains across units via `state`."""
            if tch >= NT:
                return []
            state = {"pend": None}

            def start():
                prefetch_xt(tch)

            def qk_group(w_sb, dst, p):
                def emit():
                    xt = xt_tiles[tch]
                    acc = psAUX.tile([128, 512], f32, tag="aux", name=f"acc{tch}_{p}")
                    for kc in range(KC):
                        nc.tensor.matmul(
                            acc[:],
                            w_sb[:, kc, 128 * p:128 * (p + 1)],
                            xt[:, kc, :],
                            start=(kc == 0), stop=(kc == KC - 1),
                        )
                    raw = tmps.tile([128, 512], bf16, tag="raw", name="raw")
                    ceng = nc.gpsimd if USE_POOL_COPIES else nc.vector
                    with nc.allow_low_precision(reason="bf16 q/k"):
                        ceng.tensor_copy(raw[:], acc[:])
                    if state["pend"] is not None:
                        emit_rope(*state["pend"])
                    state["pend"] = (tch, raw, dst, p)
                return emit

            def v_block(jb):
                def emit():
                    xt = xt_tiles[tch]
                    if state["pend"] is not None:
                        emit_rope(*state["pend"])
                        state["pend"] = None
                    jbg = tch * 4 + jb
                    vacc = psAUX.tile([128, 256], f32, tag="aux", name=f"vacc{jbg}")
                    for kc in range(KC):
                        nc.tensor.matmul(
                            vacc[:],
                            xt[:, kc, 128 * jb:128 * (jb + 1)],
                            wv_sb[:, kc, :],
                            start=(kc == 0), stop=(kc == KC - 1),
                        )
                    veng = nc.gpsimd if USE_POOL_COPIES else nc.vector
                    with nc.allow_low_precision(reason="bf16 v"):
                        veng.tensor_copy(
                            v_view[:, jbg, :, 0:HD],
                            vacc[:].rearrange("p (h c) -> p h c", h=HPC),
                        )
                    if jb == 3:
                        xt_tiles.pop(tch)
                        # prefetch the next chunk only now: issuing it any
                        # earlier puts its transfers ahead of the (critical)
                        # cos/sin/wv const loads in the serial DMA queue
                        prefetch_xt(tch + 1)
                return emit

            units = [start]
            units.append(qk_group(wq_sb, qt_sb, 0))
            units.append(qk_group(wq_sb, qt_sb, 1))
            units.append(qk_group(wk_sb, kt_sb, 0))
            units.append(qk_group(wk_sb, kt_sb, 1))
            for jb in range(4):
                units.append(v_block(jb))
            return units

        def c_unit_list(tch):
            if tch < 0:
                return []
            ts = slice(tch * 512, (tch + 1) * 512)

            def c_block(ech):
                def emit():
                    pc = psAUX.tile([128, 512], f32, tag="aux", name=f"pc{tch}_{ech}")
                    for kd in range(2):
                        nc.tensor.matmul(
                            pc[:],
                            wp_sb[:, kd, ech * 128:(ech + 1) * 128],
                            ot_sb[:, kd, ts],
                            start=(kd == 0), stop=(kd == 1),
                        )
                    oc = outs.tile([128, 512], bf16, tag="oc", name="oc")
                    oeng = nc.gpsimd if (USE_POOL_COPIES and tch == 0) else nc.vector
                    with nc.allow_low_precision(reason="bf16 partial out"):
                        oeng.tensor_copy(oc[:], pc[:])
                    nc.sync.dma_start(out=yT[ech, tch], in_=oc[:])
                return emit
            return [c_block(e) for e in range(8)]

        def c_final(tch):
            """Tail-specific C: nothing else is left for the PE, so pipeline
            pairs of out-proj blocks through the (now free) 2-bank stp slots,
            alternating the PSUM->SBUF copy between DVE and Pool."""
            ts = slice(tch * 512, (tch + 1) * 512)
            for pair in range(4):
                pc2 = psSTP.tile([128, 1024], f32, tag="stp", name=f"pcf{pair}")
                # kd-major: both halves' kd=0 matmuls only need ot pack 0
                # (ready mid-B(3)), so the in-order PE runs them during the
                # final norm chain instead of blocking behind kd=1
                for kd in range(2):
                    for half in range(2):
                        ech = 2 * pair + half
                        nc.tensor.matmul(
                            pc2[:, 512 * half:512 * half + 512],
                            wp_sb[:, kd, ech * 128:(ech + 1) * 128],
                            ot_sb[:, kd, ts],
                            start=(kd == 0), stop=(kd == 1),
                        )
                oc2 = outs.tile([128, 1024], bf16, tag="oc2", name="oc2")
                # split the PSUM->SBUF copy between DVE and the (idle at the
                # tail) scalar engine so the pcf pipeline is not
                # copy-throughput-limited; the second half is finished by the
                # later stop-matmul, so it goes on the lower-latency DVE
                with nc.allow_low_precision(reason="bf16 partial out"):
                    nc.scalar.activation(oc2[:, 0:512], pc2[:, 0:512],
                                         mybir.ActivationFunctionType.Copy)
                    nc.vector.tensor_copy(oc2[:, 512:1024], pc2[:, 512:1024])
                nc.sync.dma_start(out=yT[2 * pair, tch], in_=oc2[:, 0:512])
                nc.sync.dma_start(out=yT[2 * pair + 1, tch], in_=oc2[:, 512:1024])

        def interleave_fill(a_units, c_units):
            """start + qk units first (rope latency chain), C blocks woven
            between the chunky A units so aux-psum slots alternate."""
            fill = []
            a = list(a_units)
            c = list(c_units)
            if a:
                fill.append(a.pop(0))       # start (DMA prefetch) first
            while a or c:
                if a:
                    fill.append(a.pop(0))
                if c:
                    fill.append(c.pop(0))
            return fill

        def b_emit(qi, fill_units):
            """Emit B(qi)'s attention streams, sprinkling `fill_units`
            (A(qi+1) / C(qi-1) closures) between kj iterations so the PE
            always has independent matmul work queued behind exp waits."""
            kj_max = 4 * (qi + 1)
            n_slots = 2 * (kj_max + 1)
            fill = list(fill_units)
            n_fill = len(fill)
            slot = [0]

            def maybe_fill():
                # spread the n_fill units evenly across the n_slots slots
                k = (n_fill * (slot[0] + 1)) // n_slots - (n_fill * slot[0]) // n_slots
                for _ in range(k):
                    if fill:
                        fill.pop(0)()
                slot[0] += 1

            for p in range(2):
                oacc = psOAC.tile([128, 1024], f32, tag="oaccp", name=f"oacc{qi}_{p}")
                pending = []
                for kj in range(kj_max):
                    attn_step(qi, p, oacc, pending, kj, kj_max)
                    maybe_fill()
                while pending:
                    kj0, pt0, lo0 = pending.pop(0)
                    emit_o(oacc, p, kj_max, kj0, pt0, lo0)
                attn_norm(qi, p, oacc)
                maybe_fill()
            while fill:
                fill.pop(0)()

        # Fill rebalance: the later B(i) streams have the most ACT (exp) work
        # relative to their own PE work, so the out-proj C units are pushed as
        # late as their ot dependencies allow: B(3) gets C(1)+C(2) while the
        # A units go one chunk ahead as before.
        for u in a_unit_list(0):
            u()
        b_emit(0, interleave_fill(a_unit_list(1), []))
        b_emit(1, interleave_fill(a_unit_list(2), []))
        b_emit(2, interleave_fill(a_unit_list(3), c_unit_list(0)))
        b_emit(3, interleave_fill([], c_unit_list(1) + c_unit_list(2)))
        c_final(NT - 1)

    nc.compile()
    return nc


def get_program():
    global _PROGRAM
    if _PROGRAM is None:
        _PROGRAM = build_program()
    return _PROGRAM


def make_in_maps(x, W_qkv, W_proj):
    BF = _bf16()
    x = np.asarray(x, dtype=np.float32)
    W_qkv = np.asarray(W_qkv, dtype=np.float32)
    W_proj = np.asarray(W_proj, dtype=np.float32)
    in_maps = []
    xtr = {}
    for b in range(B):
        xt = x[b].T.reshape(D // 128, 128, T // 512, 512)
        xtr[b] = np.ascontiguousarray(xt.transpose(0, 2, 1, 3)).astype(BF)
    def tile_w(w):  # [D, 256] -> [128, D//128, 256] partition-major
        return np.ascontiguousarray(
            w.reshape(D // 128, 128, w.shape[1]).transpose(1, 0, 2)).astype(BF)

    for core in range(NCORES):
        b, g = divmod(core, 4)
        cs = slice(g * 256, (g + 1) * 256)
        in_maps.append({
            "xT": xtr[b],
            "wq": tile_w(W_qkv[:, 0 * D:1 * D][:, cs]),
            "wk": tile_w(W_qkv[:, 1 * D:2 * D][:, cs]),
            "wv": tile_w(W_qkv[:, 2 * D:3 * D][:, cs]),
            "wp": np.ascontiguousarray(
                W_proj[cs, :].reshape(2, 128, D).transpose(1, 0, 2)).astype(BF),
        })
    return in_maps


def gather_output(results):
    out = np.empty((B, T, D), dtype=np.float32)
    for b in range(B):
        acc = np.asarray(results[4 * b]["yT"]).astype(np.float32)
        for g in range(1, 4):
            acc += np.asarray(results[4 * b + g]["yT"]).astype(np.float32)
        # (ech, tch, p, t) -> yT (D, T) -> transpose to (T, D)
        yt = acc.transpose(0, 2, 1, 3).reshape(D, T)
        out[b] = yt.T
    return out


def kernel(x, W_qkv, W_proj, key_padding_mask=None, **_ignored):
    # key_padding_mask is all-True per the problem spec (fill: ones) -> no-op.
    from concourse.bass_utils import run_bass_kernel_spmd

    nc = get_program()
    in_maps = make_in_maps(x, W_qkv, W_proj)
    res = run_bass_kernel_spmd(nc, in_maps, list(range(NCORES)))
    return gather_output(res.results)


# revision 74
# speedup vs baseline: 1.2787x; 1.0080x over previous
"""Causal self-attention with RoPE on 8 trn2 NeuronCores.

Problem: B=2, T=2048, D=1024, H=16 heads, head_dim=64, fp32 in/out.
Sharding: core = b*4 + g  (data parallel over batch, tensor parallel over
head groups of 4). Each core computes its 4 heads' attention plus the
row-slice of the output projection; the host sums the 4 partial Y^T per
batch (bf16 partials, f32 accumulate) and transposes back.

All matmul inputs are bf16 (same PE rate as f32r at >=256 moving cols, no
4x penalty on the <256-col diagonal tiles, half the DMA bytes, and 2x DVE
throughput on the all-bf16 elementwise ops). PSUM accumulation stays f32.

Per-core dataflow (everything transposed so matmuls contract on partitions):
  xT (1024, 2048)  =  x[b].T
  QT/KT packs: qt_sb[128, 2, T] (pack p = 2 heads of 64 rows)
  RoPE: QT' = QT*cos + (R2 @ QT)*sin   (R2 = block-diag rotate-half matrix)
  V_aug [128, 16, 260]: V natural layout per key block, 4 heads x (64 dims
      + ones column) -> fused softmax denominator.
  S pair tile [keys 128, 2*512] = both heads of a pack in one 2-bank PSUM
      tile; ONE exp activation covers both halves (halves ACT op count).
  P = exp(S * 0.125) -> bf16 SBUF, diag blocks masked on GPSIMD.
  Oacc pair [65, 2*512] += V_aug^T @ P  (row 64 = denominator)
  norm: DVE reciprocal -> GPSIMD partition_broadcast -> DVE mul -> ot bf16
  Y^T partial = Wp-slice^T @ O^T packs -> bf16 DRAM out (SP-queue DMA).

Phases are interleaved: B(i)'s st->exp->PV stream is ACT-latency-bound, so
A(i+1) (projections) and C(i-1) (out-proj) PE work is sprinkled into its
kj slots to keep the PE busy while the scalar engine catches up.
"""

import sys
import numpy as np

sys.path.insert(0, "/opt/trn_rl_repo")

B, T, D, H = 2, 2048, 1024, 16
HD = 64          # head dim
HPC = 4          # heads per core
NCORES = 8
ROPE_BASE = 10000.0

_PROGRAM = None  # cached compiled program

# GPSIMD (Pool-engine) offloads: flipped on only after HW verifier approval,
# since the BIR verifier enforces rules CoreSim does not.
USE_POOL_COPIES = False   # dead: "GPSIMD Instructions cannot access PSUM"
USE_POOL_MASKS = True     # SBUF tensor_mul (causal masks) on gpsimd
USE_POOL_PB = True        # gpsimd partition_broadcast for the softmax recip


def _bf16():
    import ml_dtypes
    return ml_dtypes.bfloat16


def _rope_tables_np():
    inv_freq = 1.0 / (ROPE_BASE ** (np.arange(0, HD, 2, dtype=np.float32) / np.float32(HD)))
    pos = np.arange(T, dtype=np.float32)
    freqs = np.outer(pos, inv_freq).astype(np.float32)          # (T, 32)
    emb = np.concatenate([freqs, freqs], axis=-1)               # (T, 64)
    cosT = np.cos(emb).T.astype(np.float32)                     # (64, T)
    sinT = np.sin(emb).T.astype(np.float32)
    cos2 = np.vstack([cosT, cosT]).copy()                       # (128, T) two heads
    sin2 = np.vstack([sinT, sinT]).copy()
    # sign-folded sin for the matmul-free rotate-half:
    #   rope(q)[d] = q[d]*cos[d] + rot(q)[d]*sin[d],
    #   rot(q)[base+d] = -q[base+32+d] (d<32) ; q[base+d-32] (d>=32)
    # so ts[base+0:32] = raw[base+32:64] * (-sin[base+0:32]) and
    #    ts[base+32:64] = raw[base+0:32] * (+sin[base+32:64]).
    sinadj = sin2.copy()
    for base in (0, 32, 64, 96):
        if (base // 32) % 2 == 0:
            sinadj[base:base + 32] = -sinadj[base:base + 32]
    return cos2, sin2, sinadj


def _r2_np():
    # qrot[d] = -q[d+32] (d<32) ; q[d-32] (d>=32), per 64-row block.
    # matmul computes out[d, t] = sum_k r2[k, d] q[k, t]
    r2 = np.zeros((128, 128), dtype=np.float32)
    for base in (0, 64):
        for d in range(32):
            r2[base + d + 32, base + d] = -1.0
            r2[base + d, base + d + 32] = 1.0
    return r2


def _masks_np():
    # tri[j, ql] = 1 if key j may attend query ql within a diagonal block
    j = np.arange(128)[:, None]
    ql = np.arange(128)[None, :]
    return (j <= ql).astype(np.float32)                         # [128, 128]


def build_program():
    import concourse.bass as bass
    import concourse.tile as tile
    from concourse import bacc, mybir
    from contextlib import ExitStack

    BF = _bf16()
    f32 = mybir.dt.float32
    bf16 = mybir.dt.bfloat16

    nc = bacc.Bacc(None, target_bir_lowering=False, debug=False)

    # xT pre-tiled on host: xTr[kc, tch, p, t] = x[b].T[kc*128+p, tch*512+t]
    xT = nc.declare_dram_parameter("xT", [D // 128, T // 512, 128, 512], bf16, isOutput=False)
    # weights pre-tiled on host to partition-major so each load is one
    # contiguous descriptor per partition (4x fewer descriptors -> 2x faster
    # serial DMA at the head): wq[p, kc, c] = W_qkv[kc*128+p, c]
    wq = nc.declare_dram_parameter("wq", [128, D // 128, 256], bf16, isOutput=False)
    wk = nc.declare_dram_parameter("wk", [128, D // 128, 256], bf16, isOutput=False)
    wv = nc.declare_dram_parameter("wv", [128, D // 128, 256], bf16, isOutput=False)
    wp = nc.declare_dram_parameter("wp", [128, 2, D], bf16, isOutput=False)
    # yT tiled: yTr[ech, tch, p, t] = yT_partial[ech*128+p, tch*512+t], bf16
    yT = nc.declare_dram_parameter("yT", [8, T // 512, 128, 512], bf16, isOutput=True)

    cos2_np, sin2_np, _sinadj_np = _rope_tables_np()
    cos_d = nc.inline_tensor(cos2_np.astype(BF), name="cos2")
    sin_d = nc.inline_tensor(sin2_np.astype(BF), name="sin2")
    r2_d = nc.inline_tensor(_r2_np().astype(BF), name="r2")
    masks_d = nc.inline_tensor(_masks_np().astype(BF), name="masks")

    NT = T // 512            # 4 t-chunks
    NJ = T // 128            # 16 key blocks
    KC = D // 128            # 8 contraction chunks
    Exp = mybir.ActivationFunctionType.Exp

    with tile.TileContext(nc) as tc, ExitStack() as ctx:
        # --- persistent SBUF ---
        wts = ctx.enter_context(tc.tile_pool(name="wts", bufs=1))
        packs = ctx.enter_context(tc.tile_pool(name="packs", bufs=1))
        consts = ctx.enter_context(tc.tile_pool(name="consts", bufs=1))

        # --- working pools (xts first: its loads gate the first matmuls) ---
        xts = ctx.enter_context(tc.tile_pool(name="xts", bufs=2))

        wq_sb = wts.tile([128, KC, 256], bf16, tag="wq")
        wk_sb = wts.tile([128, KC, 256], bf16, tag="wk")
        wv_sb = wts.tile([128, KC, 256], bf16, tag="wv")
        wp_sb = wts.tile([128, 2, 1024], bf16, tag="wp")
        xt_tiles = {}

        def prefetch_xt(tch):
            if tch >= NT or tch in xt_tiles:
                return
            xt = xts.tile([128, KC, 512], bf16, tag="xt", name=f"xt{tch}")
            # split so the first accumulation chunks start sooner
            nsplit = 4 if tch == 0 else 2
            step = KC // nsplit
            for s in range(nsplit):
                nc.sync.dma_start(
                    out=xt[:, s * step:(s + 1) * step, :],
                    in_=xT[s * step:(s + 1) * step, tch].rearrange("k p t -> p k t"))
            xt_tiles[tch] = xt

        cos_sb = consts.tile([128, T], bf16, tag="cos")
        sin_sb = consts.tile([128, T], bf16, tag="sin")
        r2_sb = consts.tile([128, 128], bf16, tag="r2")
        masks_sb = consts.tile([128, 128], bf16, tag="masks")
        # The head is serial-DMA-bound: issue in consumption order across
        # BOTH descriptor generators (sync -> HWDGE, gpsimd -> Pool SWDGE).
        xt0 = xts.tile([128, KC, 512], bf16, tag="xt", name="xt0")
        xt_tiles[0] = xt0
        nc.sync.dma_start(out=wq_sb[:, 0:2, :], in_=wq[:, 0:2, :])
        nc.gpsimd.dma_start(out=r2_sb[:], in_=r2_d[:, :])
        nc.sync.dma_start(out=wq_sb[:, 2:8, :], in_=wq[:, 2:8, :])
        nc.gpsimd.dma_start(out=xt0[:, 4:6, :], in_=xT[4:6, 0].rearrange("k p t -> p k t"))
        nc.sync.dma_start(out=xt0[:, 0:2, :], in_=xT[0:2, 0].rearrange("k p t -> p k t"))
        nc.gpsimd.dma_start(out=xt0[:, 6:8, :], in_=xT[6:8, 0].rearrange("k p t -> p k t"))
        nc.sync.dma_start(out=xt0[:, 2:4, :], in_=xT[2:4, 0].rearrange("k p t -> p k t"))
        nc.sync.dma_start(out=wk_sb[:], in_=wk[:, :, :])
        nc.gpsimd.dma_start(out=cos_sb[:], in_=cos_d[:, :])
        nc.gpsimd.dma_start(out=sin_sb[:], in_=sin_d[:, :])
        nc.gpsimd.dma_start(out=wv_sb[:], in_=wv[:, :, :])
        nc.gpsimd.dma_start(out=masks_sb[:], in_=masks_d[:, :])
        nc.gpsimd.dma_start(out=wp_sb[:], in_=wp[:, :, :])
        # xt1 queued last on the Pool SWDGE: the serial DMA engine drains in
        # arrival order, so issuing it any earlier starves the const loads
        xt1 = xts.tile([128, KC, 512], bf16, tag="xt", name="xt1")
        xt_tiles[1] = xt1
        for s in range(2):
            nc.gpsimd.dma_start(
                out=xt1[:, 4 * s:4 * (s + 1), :],
                in_=xT[4 * s:4 * (s + 1), 1].rearrange("k p t -> p k t"))

        # packs: [128, pack p, T] so one rope add can write both packs' slices
        qt_sb = packs.tile([128, 2, T], bf16, tag="qt")
        kt_sb = packs.tile([128, 2, T], bf16, tag="kt")
        ot_sb = packs.tile([128, 2, T], bf16, tag="ot")
        v_sb = packs.tile([128, NJ, HPC * (HD + 1)], bf16, tag="vaug")

        # ones columns of v_aug (fused softmax denominator)
        v_view = v_sb[:].rearrange("p j (h c) -> p j h c", h=HPC)
        nc.vector.memset(v_view[:, :, :, HD:HD + 1], 1.0)
        ones64 = consts.tile([1, 64], bf16, tag="ones64")
        nc.vector.memset(ones64[:], 1.0)
        # dummy activation pulls the Exp table load (1.3us) into the
        # DMA-bound head instead of B(0)'s first softmax
        warm = consts.tile([1, 2], bf16, tag="warm")
        with nc.allow_low_precision(reason="act table warmup"):
            nc.scalar.activation(warm[:], ones64[0:1, 0:2], Exp)

        # --- working pools ---
        tmps = ctx.enter_context(tc.tile_pool(name="tmps", bufs=3))
        pts = ctx.enter_context(tc.tile_pool(name="pts", bufs=8))
        outs = ctx.enter_context(tc.tile_pool(name="outs", bufs=3))
        smalls = ctx.enter_context(tc.tile_pool(name="smalls", bufs=2))

        # 8 PSUM banks: stp pairs 2x2, oacc pair 2, aux (acc/rot/vacc/pc) 2x1
        psSTP = ctx.enter_context(tc.tile_pool(name="psSTP", bufs=2, space="PSUM"))
        psOAC = ctx.enter_context(tc.tile_pool(name="psOAC", bufs=1, space="PSUM"))
        psAUX = ctx.enter_context(tc.tile_pool(name="psAUX", bufs=2, space="PSUM"))

        def emit_rope(tch, raw, dst, p):
            # rope(q) = q*cos + rot(q)*sin; the rotate-half must cross
            # partitions, which only the PE (or DMA) may do on HW, so it is
            # a [128,128] block-diag matmul against the bf16 raw copy.
            ts = slice(tch * 512, (tch + 1) * 512)
            rot = psAUX.tile([128, 512], f32, tag="aux", name="rot")
            nc.tensor.matmul(rot[:], r2_sb[:], raw[:], start=True, stop=True)
            tc_t = tmps.tile([128, 512], bf16, tag="tc", name="tc_t")
            nc.vector.tensor_mul(tc_t[:], raw[:], cos_sb[:, ts])
            ts_t = tmps.tile([128, 512], bf16, tag="ts", name="ts_t")
            with nc.allow_low_precision(reason="bf16 rope"):
                nc.vector.tensor_mul(ts_t[:], rot[:], sin_sb[:, ts])
                nc.vector.tensor_add(dst[:, p, ts], tc_t[:], ts_t[:])

        def emit_o(oacc, p, kj_max, kj, pt, lo):
            for hh in range(2):
                hl = 2 * p + hh
                nc.tensor.matmul(
                    oacc[0:65, 512 * hh + lo:512 * hh + 512],
                    v_sb[:, kj, 65 * hl:65 * hl + 65],
                    pt[:, 512 * hh + lo:512 * hh + 512],
                    start=(kj == 0), stop=(kj == kj_max - 1),
                )

        def attn_step(qi, p, oacc, pending, kj, kj_max):
            """One kj iteration: st pair matmuls -> single exp -> (mask),
            software-pipelined so the PE never queues behind its own exp."""
            r = kj - 4 * qi          # >= 0 on the causal diagonal
            lo = max(r, 0) * 128     # first valid column
            stp = psSTP.tile([128, 1024], f32, tag="stp", name="stp")
            for hh in range(2):
                off = 64 * hh
                nc.tensor.matmul(
                    stp[:, 512 * hh + lo:512 * hh + 512],
                    kt_sb[off:off + 64, p, kj * 128:(kj + 1) * 128],
                    qt_sb[off:off + 64, p, qi * 512 + lo:(qi + 1) * 512],
                    start=True, stop=True,
                )
            pt = pts.tile([128, 1024], bf16, tag="pt", name="pt")
            w = 512 - lo
            st3 = stp[:].rearrange("p (h t) -> p h t", h=2)[:, :, lo:512]
            pt3 = pt[:].rearrange("p (h t) -> p h t", h=2)[:, :, lo:512]
            with nc.allow_low_precision(reason="bf16 softmax weights"):
                nc.scalar.activation(pt3, st3, Exp, scale=0.125)
            if r >= 0:
                meng = nc.gpsimd if (USE_POOL_MASKS and qi < 3) else nc.vector
                for hh in range(2):
                    s = slice(512 * hh + lo, 512 * hh + lo + 128)
                    with nc.allow_low_precision(reason="bf16 mask"):
                        meng.tensor_mul(pt[:, s], pt[:, s], masks_sb[:])
            pending.append((kj, pt, lo))
            # keep the window tight on the very last stream: its PVs gate the
            # final norm -> C(3) tail, so don't let them queue up
            window = 1 if (qi == NT - 1 and p == 1) else 4
            while len(pending) > window:
                kj0, pt0, lo0 = pending.pop(0)
                emit_o(oacc, p, kj_max, kj0, pt0, lo0)

        def attn_norm(qi, p, oacc):
            qs = slice(qi * 512, (qi + 1) * 512)
            if qi == NT - 1 and p == 1:
                # tail stream: C(3) waits on this chain and nothing waits on
                # the oacc banks, so skip oct; emit recips, then pbs, then
                # muls so the per-head chains pipeline across DVE and Pool
                # instead of serializing on the in-order DVE queue
                rcps, bcls = [], []
                for hh in range(2):
                    s = slice(512 * hh, 512 * hh + 512)
                    rcp = smalls.tile([1, 512], bf16, tag="rcp", name="rcp")
                    with nc.allow_low_precision(reason="bf16 softmax recip"):
                        nc.vector.reciprocal(rcp[:], oacc[64:65, s])
                    rcps.append(rcp)
                for hh in range(2):
                    bcl = smalls.tile([64, 512], bf16, tag="bcl", name="bcl")
                    nc.gpsimd.partition_broadcast(bcl[0:64, :], rcps[hh][0:1, :])
                    bcls.append(bcl)
                for hh in range(2):
                    s = slice(512 * hh, 512 * hh + 512)
                    off = 64 * hh
                    with nc.allow_low_precision(reason="bf16 attn out"):
                        nc.vector.tensor_mul(ot_sb[off:off + 64, p, qs],
                                             oacc[0:64, s], bcls[hh][0:64, :])
                return
            # one bf16 copy releases the oacc PSUM pair early (the next
            # stream's PV accumulation is waiting on these 2 banks); the
            # ot mul then has exactly one PSUM operand (bc_ps), which is the
            # HW limit for TensorTensor.
            recip2 = smalls.tile([1, 1024], bf16, tag="recip2", name="recip2")
            with nc.allow_low_precision(reason="bf16 softmax recip"):
                nc.vector.reciprocal(recip2[:], oacc[64:65, :])
            oct = smalls.tile([65, 1024], bf16, tag="oct", name="oct")
            with nc.allow_low_precision(reason="bf16 attn out"):
                nc.vector.tensor_copy(oct[:], oacc[0:65, :])
            # partition-broadcast of the recip row: gpsimd custom op when
            # allowed, else a ones-column matmul on the PE (the only other
            # engine that may cross partitions on HW)
            if USE_POOL_PB:
                bc = smalls.tile([64, 1024], bf16, tag="bc", name="bc")
                for hh in range(2):
                    s = slice(512 * hh, 512 * hh + 512)
                    nc.gpsimd.partition_broadcast(bc[0:64, s], recip2[0:1, s])
                for hh in range(2):
                    off = 64 * hh
                    s = slice(512 * hh, 512 * hh + 512)
                    with nc.allow_low_precision(reason="bf16 attn out"):
                        nc.vector.tensor_mul(ot_sb[off:off + 64, p, qs],
                                             oct[0:64, s], bc[0:64, s])
            else:
                for hh in range(2):
                    s = slice(512 * hh, 512 * hh + 512)
                    bc_ps = psAUX.tile([64, 512], f32, tag="aux", name="bc_ps")
                    nc.tensor.matmul(bc_ps[:], ones64[:], recip2[0:1, s],
                                     start=True, stop=True)
                    off = 64 * hh
                    with nc.allow_low_precision(reason="bf16 attn out"):
                        nc.vector.tensor_mul(ot_sb[off:off + 64, p, qs],
                                             oct[0:64, s], bc_ps[:])

        def emit_rope_pair(raw2, dst):
            """Head-only rope over a pack-pair tile (tch 0): one rot matmul
            in a borrowed stp slot, per-pack DVE muls."""
            ts0 = slice(0, 512)
            rot2 = psSTP.tile([128, 1024], f32, tag="stp", name="rot2")
            for p in range(2):
                s = slice(512 * p, 512 * p + 512)
                nc.tensor.matmul(rot2[:, s], r2_sb[:], raw2[:, s],
                                 start=True, stop=True)
            for p in range(2):
                s = slice(512 * p, 512 * p + 512)
                tc_t = tmps.tile([128, 512], bf16, tag="tc", name="tc_t")
                nc.vector.tensor_mul(tc_t[:], raw2[:, s], cos_sb[:, ts0])
                ts_t = tmps.tile([128, 512], bf16, tag="ts", name="ts_t")
                with nc.allow_low_precision(reason="bf16 rope"):
                    nc.vector.tensor_mul(ts_t[:], rot2[:, s], sin_sb[:, ts0])
                    nc.vector.tensor_add(dst[:, p, ts0], tc_t[:], ts_t[:])

        def a_unit_list(tch):
            """A(tch) as a list of emission closures (proj groups, V blocks).
            The rope skew chains across units via `state`.

            tch 0 is special: B isn't running yet, so the stp PSUM slots are
            free and the two packs of each projection accumulate side by side
            in one 2-bank tile with kc-interleaved matmuls -- every arriving
            xt0 chunk immediately feeds 4 matmuls instead of 2, halving the
            delivery-bound head stall."""
            if tch >= NT:
                return []
            state = {"pend": None}

            def start():
                prefetch_xt(tch)

            def qk_pair_head(w_sb, dst, nm):
                def emit():
                    xt = xt_tiles[0]
                    accp = psSTP.tile([128, 1024], f32, tag="stp",
                                      name=f"accp_{nm}")
                    for kc in range(KC):
                        for p in range(2):
                            nc.tensor.matmul(
                                accp[:, 512 * p:512 * p + 512],
                                w_sb[:, kc, 128 * p:128 * (p + 1)],
                                xt[:, kc, :],
                                start=(kc == 0), stop=(kc == KC - 1),
                            )
                    raw2 = tmps.tile([128, 1024], bf16, tag="raw2", name="raw2")
                    with nc.allow_low_precision(reason="bf16 q/k"):
                        nc.vector.tensor_copy(raw2[:], accp[:])
                    if state["pend"] is not None:
                        state["pend"]()
                    state["pend"] = lambda r=raw2, d=dst: emit_rope_pair(r, d)
                return emit

            def qk_group(w_sb, dst, p):
                def emit():
                    xt = xt_tiles[tch]
                    acc = psAUX.tile([128, 512], f32, tag="aux", name=f"acc{tch}_{p}")
                    for kc in range(KC):
                        nc.tensor.matmul(
                            acc[:],
                            w_sb[:, kc, 128 * p:128 * (p + 1)],
                            xt[:, kc, :],
                            start=(kc == 0), stop=(kc == KC - 1),
                        )
                    raw = tmps.tile([128, 512], bf16, tag="raw", name="raw")
                    ceng = nc.gpsimd if USE_POOL_COPIES else nc.vector
                    with nc.allow_low_precision(reason="bf16 q/k"):
                        ceng.tensor_copy(raw[:], acc[:])
                    if state["pend"] is not None:
                        state["pend"]()
                    state["pend"] = (lambda t=tch, r=raw, d=dst, pp=p:
                                     emit_rope(t, r, d, pp))
                return emit

            def v_block(jb):
                def emit():
                    xt = xt_tiles[tch]
                    if state["pend"] is not None:
                        state["pend"]()
                        state["pend"] = None
                    jbg = tch * 4 + jb
                    vacc = psAUX.tile([128, 256], f32, tag="aux", name=f"vacc{jbg}")
                    for kc in range(KC):
                        nc.tensor.matmul(
                            vacc[:],
                            xt[:, kc, 128 * jb:128 * (jb + 1)],
                            wv_sb[:, kc, :],
                            start=(kc == 0), stop=(kc == KC - 1),
                        )
                    veng = nc.gpsimd if USE_POOL_COPIES else nc.vector
                    with nc.allow_low_precision(reason="bf16 v"):
                        veng.tensor_copy(
                            v_view[:, jbg, :, 0:HD],
                            vacc[:].rearrange("p (h c) -> p h c", h=HPC),
                        )
                    if jb == 3:
                        xt_tiles.pop(tch)
                        # prefetch the next chunk only now: issuing it any
                        # earlier puts its transfers ahead of the (critical)
                        # cos/sin/wv const loads in the serial DMA queue
                        prefetch_xt(tch + 1)
                return emit

            units = [start]
            if tch == 0:
                units.append(qk_pair_head(wq_sb, qt_sb, "q"))
                units.append(qk_pair_head(wk_sb, kt_sb, "k"))
            else:
                units.append(qk_group(wq_sb, qt_sb, 0))
                units.append(qk_group(wq_sb, qt_sb, 1))
                units.append(qk_group(wk_sb, kt_sb, 0))
                units.append(qk_group(wk_sb, kt_sb, 1))
            for jb in range(4):
                units.append(v_block(jb))
            return units

        def c_unit_list(tch):
            if tch < 0:
                return []
            ts = slice(tch * 512, (tch + 1) * 512)

            def c_block(ech):
                def emit():
                    pc = psAUX.tile([128, 512], f32, tag="aux", name=f"pc{tch}_{ech}")
                    for kd in range(2):
                        nc.tensor.matmul(
                            pc[:],
                            wp_sb[:, kd, ech * 128:(ech + 1) * 128],
                            ot_sb[:, kd, ts],
                            start=(kd == 0), stop=(kd == 1),
                        )
                    oc = outs.tile([128, 512], bf16, tag="oc", name="oc")
                    oeng = nc.gpsimd if (USE_POOL_COPIES and tch == 0) else nc.vector
                    with nc.allow_low_precision(reason="bf16 partial out"):
                        oeng.tensor_copy(oc[:], pc[:])
                    nc.sync.dma_start(out=yT[ech, tch], in_=oc[:])
                return emit
            return [c_block(e) for e in range(8)]

        def c_final(tch):
            """Tail-specific C: nothing else is left for the PE, so pipeline
            pairs of out-proj blocks through the (now free) 2-bank stp slots,
            alternating the PSUM->SBUF copy between DVE and Pool."""
            ts = slice(tch * 512, (tch + 1) * 512)
            # two waves of two pairs: BOTH in-flight pairs' kd=0 matmuls
            # (which only need ot pack 0, ready mid-B(3)) are emitted before
            # any norm-gated kd=1, so the in-order PE chews through them
            # during the final norm chain
            for wave in range(2):
                tiles = []
                for pair in (2 * wave, 2 * wave + 1):
                    pc2 = psSTP.tile([128, 1024], f32, tag="stp", name=f"pcf{pair}")
                    for half in range(2):
                        ech = 2 * pair + half
                        nc.tensor.matmul(
                            pc2[:, 512 * half:512 * half + 512],
                            wp_sb[:, 0, ech * 128:(ech + 1) * 128],
                            ot_sb[:, 0, ts],
                            start=True, stop=False,
                        )
                    tiles.append((pair, pc2))
                for pair, pc2 in tiles:
                    for half in range(2):
                        ech = 2 * pair + half
                        nc.tensor.matmul(
                            pc2[:, 512 * half:512 * half + 512],
                            wp_sb[:, 1, ech * 128:(ech + 1) * 128],
                            ot_sb[:, 1, ts],
                            start=False, stop=True,
                        )
                    oc2 = outs.tile([128, 1024], bf16, tag="oc2", name="oc2")
                    # split the PSUM->SBUF copy between DVE and the (idle at
                    # the tail) scalar engine so the pipeline is not
                    # copy-throughput-limited
                    with nc.allow_low_precision(reason="bf16 partial out"):
                        nc.scalar.activation(oc2[:, 0:512], pc2[:, 0:512],
                                             mybir.ActivationFunctionType.Copy)
                        nc.vector.tensor_copy(oc2[:, 512:1024], pc2[:, 512:1024])
                    nc.sync.dma_start(out=yT[2 * pair, tch], in_=oc2[:, 0:512])
                    nc.sync.dma_start(out=yT[2 * pair + 1, tch], in_=oc2[:, 512:1024])

        def interleave_fill(a_units, c_units):
            """start + qk units first (rope latency chain), C blocks woven
            between the chunky A units so aux-psum slots alternate."""
            fill = []
            a = list(a_units)
            c = list(c_units)
            if a:
                fill.append(a.pop(0))       # start (DMA prefetch) first
            while a or c:
                if a:
                    fill.append(a.pop(0))
                if c:
                    fill.append(c.pop(0))
            return fill

        def b_emit(qi, fill_units):
            """Emit B(qi)'s attention streams, sprinkling `fill_units`
            (A(qi+1) / C(qi-1) closures) between kj iterations so the PE
            always has independent matmul work queued behind exp waits."""
            kj_max = 4 * (qi + 1)
            n_slots = 2 * (kj_max + 1)
            fill = list(fill_units)
            n_fill = len(fill)
            slot = [0]

            def maybe_fill():
                # spread the n_fill units evenly across the n_slots slots
                k = (n_fill * (slot[0] + 1)) // n_slots - (n_fill * slot[0]) // n_slots
                for _ in range(k):
                    if fill:
                        fill.pop(0)()
                slot[0] += 1

            for p in range(2):
                oacc = psOAC.tile([128, 1024], f32, tag="oaccp", name=f"oacc{qi}_{p}")
                pending = []
                for kj in range(kj_max):
                    attn_step(qi, p, oacc, pending, kj, kj_max)
                    maybe_fill()
                while pending:
                    kj0, pt0, lo0 = pending.pop(0)
                    emit_o(oacc, p, kj_max, kj0, pt0, lo0)
                attn_norm(qi, p, oacc)
                maybe_fill()
            while fill:
                fill.pop(0)()

        # Fill rebalance: the later B(i) streams have the most ACT (exp) work
        # relative to their own PE work, so the out-proj C units are pushed as
        # late as their ot dependencies allow: B(3) gets C(1)+C(2) while the
        # A units go one chunk ahead as before.
        for u in a_unit_list(0):
            u()
        b_emit(0, interleave_fill(a_unit_list(1), []))
        b_emit(1, interleave_fill(a_unit_list(2), []))
        b_emit(2, interleave_fill(a_unit_list(3), c_unit_list(0)))
        b_emit(3, interleave_fill([], c_unit_list(1) + c_unit_list(2)))
        c_final(NT - 1)

    nc.compile()
    return nc


def get_program():
    global _PROGRAM
    if _PROGRAM is None:
        _PROGRAM = build_program()
    return _PROGRAM


def make_in_maps(x, W_qkv, W_proj):
    BF = _bf16()
    x = np.asarray(x, dtype=np.float32)
    W_qkv = np.asarray(W_qkv, dtype=np.float32)
    W_proj = np.asarray(W_proj, dtype=np.float32)
    in_maps = []
    xtr = {}
    for b in range(B):
        xt = x[b].T.reshape(D // 128, 128, T // 512, 512)
        xtr[b] = np.ascontiguousarray(xt.transpose(0, 2, 1, 3)).astype(BF)
    def tile_w(w):  # [D, 256] -> [128, D//128, 256] partition-major
        return np.ascontiguousarray(
            w.reshape(D // 128, 128, w.shape[1]).transpose(1, 0, 2)).astype(BF)

    for core in range(NCORES):
        b, g = divmod(core, 4)
        cs = slice(g * 256, (g + 1) * 256)
        in_maps.append({
            "xT": xtr[b],
            "wq": tile_w(W_qkv[:, 0 * D:1 * D][:, cs]),
            "wk": tile_w(W_qkv[:, 1 * D:2 * D][:, cs]),
            "wv": tile_w(W_qkv[:, 2 * D:3 * D][:, cs]),
            "wp": np.ascontiguousarray(
                W_proj[cs, :].reshape(2, 128, D).transpose(1, 0, 2)).astype(BF),
        })
    return in_maps


def gather_output(results):
    out = np.empty((B, T, D), dtype=np.float32)
    for b in range(B):
        acc = np.asarray(results[4 * b]["yT"]).astype(np.float32)
        for g in range(1, 4):
            acc += np.asarray(results[4 * b + g]["yT"]).astype(np.float32)
        # (ech, tch, p, t) -> yT (D, T) -> transpose to (T, D)
        yt = acc.transpose(0, 2, 1, 3).reshape(D, T)
        out[b] = yt.T
    return out


def kernel(x, W_qkv, W_proj, key_padding_mask=None, **_ignored):
    # key_padding_mask is all-True per the problem spec (fill: ones) -> no-op.
    from concourse.bass_utils import run_bass_kernel_spmd

    nc = get_program()
    in_maps = make_in_maps(x, W_qkv, W_proj)
    res = run_bass_kernel_spmd(nc, in_maps, list(range(NCORES)))
    return gather_output(res.results)


# revision 77
# speedup vs baseline: 1.2837x; 1.0039x over previous
"""Causal self-attention with RoPE on 8 trn2 NeuronCores.

Problem: B=2, T=2048, D=1024, H=16 heads, head_dim=64, fp32 in/out.
Sharding: core = b*4 + g  (data parallel over batch, tensor parallel over
head groups of 4). Each core computes its 4 heads' attention plus the
row-slice of the output projection; the host sums the 4 partial Y^T per
batch (bf16 partials, f32 accumulate) and transposes back.

All matmul inputs are bf16 (same PE rate as f32r at >=256 moving cols, no
4x penalty on the <256-col diagonal tiles, half the DMA bytes, and 2x DVE
throughput on the all-bf16 elementwise ops). PSUM accumulation stays f32.

Per-core dataflow (everything transposed so matmuls contract on partitions):
  xT (1024, 2048)  =  x[b].T
  QT/KT packs: qt_sb[128, 2, T] (pack p = 2 heads of 64 rows)
  RoPE: QT' = QT*cos + (R2 @ QT)*sin   (R2 = block-diag rotate-half matrix)
  V_aug [128, 16, 260]: V natural layout per key block, 4 heads x (64 dims
      + ones column) -> fused softmax denominator.
  S pair tile [keys 128, 2*512] = both heads of a pack in one 2-bank PSUM
      tile; ONE exp activation covers both halves (halves ACT op count).
  P = exp(S * 0.125) -> bf16 SBUF, diag blocks masked on GPSIMD.
  Oacc pair [65, 2*512] += V_aug^T @ P  (row 64 = denominator)
  norm: DVE reciprocal -> GPSIMD partition_broadcast -> DVE mul -> ot bf16
  Y^T partial = Wp-slice^T @ O^T packs -> bf16 DRAM out (SP-queue DMA).

Phases are interleaved: B(i)'s st->exp->PV stream is ACT-latency-bound, so
A(i+1) (projections) and C(i-1) (out-proj) PE work is sprinkled into its
kj slots to keep the PE busy while the scalar engine catches up.
"""

import sys
import numpy as np

sys.path.insert(0, "/opt/trn_rl_repo")

B, T, D, H = 2, 2048, 1024, 16
HD = 64          # head dim
HPC = 4          # heads per core
NCORES = 8
ROPE_BASE = 10000.0

_PROGRAM = None  # cached compiled program

# GPSIMD (Pool-engine) offloads: flipped on only after HW verifier approval,
# since the BIR verifier enforces rules CoreSim does not.
USE_POOL_COPIES = False   # dead: "GPSIMD Instructions cannot access PSUM"
USE_POOL_MASKS = True     # SBUF tensor_mul (causal masks) on gpsimd
USE_POOL_PB = True        # gpsimd partition_broadcast for the softmax recip


def _bf16():
    import ml_dtypes
    return ml_dtypes.bfloat16


def _rope_tables_np():
    inv_freq = 1.0 / (ROPE_BASE ** (np.arange(0, HD, 2, dtype=np.float32) / np.float32(HD)))
    pos = np.arange(T, dtype=np.float32)
    freqs = np.outer(pos, inv_freq).astype(np.float32)          # (T, 32)
    emb = np.concatenate([freqs, freqs], axis=-1)               # (T, 64)
    cosT = np.cos(emb).T.astype(np.float32)                     # (64, T)
    sinT = np.sin(emb).T.astype(np.float32)
    cos2 = np.vstack([cosT, cosT]).copy()                       # (128, T) two heads
    sin2 = np.vstack([sinT, sinT]).copy()
    # sign-folded sin for the matmul-free rotate-half:
    #   rope(q)[d] = q[d]*cos[d] + rot(q)[d]*sin[d],
    #   rot(q)[base+d] = -q[base+32+d] (d<32) ; q[base+d-32] (d>=32)
    # so ts[base+0:32] = raw[base+32:64] * (-sin[base+0:32]) and
    #    ts[base+32:64] = raw[base+0:32] * (+sin[base+32:64]).
    sinadj = sin2.copy()
    for base in (0, 32, 64, 96):
        if (base // 32) % 2 == 0:
            sinadj[base:base + 32] = -sinadj[base:base + 32]
    return cos2, sin2, sinadj


def _r2_np():
    # qrot[d] = -q[d+32] (d<32) ; q[d-32] (d>=32), per 64-row block.
    # matmul computes out[d, t] = sum_k r2[k, d] q[k, t]
    r2 = np.zeros((128, 128), dtype=np.float32)
    for base in (0, 64):
        for d in range(32):
            r2[base + d + 32, base + d] = -1.0
            r2[base + d, base + d + 32] = 1.0
    return r2


def _masks_np():
    # tri[j, ql] = 1 if key j may attend query ql within a diagonal block
    j = np.arange(128)[:, None]
    ql = np.arange(128)[None, :]
    return (j <= ql).astype(np.float32)                         # [128, 128]


def build_program():
    import concourse.bass as bass
    import concourse.tile as tile
    from concourse import bacc, mybir
    from contextlib import ExitStack

    BF = _bf16()
    f32 = mybir.dt.float32
    bf16 = mybir.dt.bfloat16

    nc = bacc.Bacc(None, target_bir_lowering=False, debug=False)

    # xT pre-tiled on host: xTr[kc, tch, p, t] = x[b].T[kc*128+p, tch*512+t]
    xT = nc.declare_dram_parameter("xT", [D // 128, T // 512, 128, 512], bf16, isOutput=False)
    # weights pre-tiled on host to partition-major so each load is one
    # contiguous descriptor per partition (4x fewer descriptors -> 2x faster
    # serial DMA at the head): wq[p, kc, c] = W_qkv[kc*128+p, c]
    wq = nc.declare_dram_parameter("wq", [128, D // 128, 256], bf16, isOutput=False)
    wk = nc.declare_dram_parameter("wk", [128, D // 128, 256], bf16, isOutput=False)
    wv = nc.declare_dram_parameter("wv", [128, D // 128, 256], bf16, isOutput=False)
    wp = nc.declare_dram_parameter("wp", [128, 2, D], bf16, isOutput=False)
    # yT tiled: yTr[ech, tch, p, t] = yT_partial[ech*128+p, tch*512+t], bf16
    yT = nc.declare_dram_parameter("yT", [8, T // 512, 128, 512], bf16, isOutput=True)

    cos2_np, sin2_np, _sinadj_np = _rope_tables_np()
    cos_d = nc.inline_tensor(cos2_np.astype(BF), name="cos2")
    sin_d = nc.inline_tensor(sin2_np.astype(BF), name="sin2")
    r2_d = nc.inline_tensor(_r2_np().astype(BF), name="r2")
    masks_d = nc.inline_tensor(_masks_np().astype(BF), name="masks")

    NT = T // 512            # 4 t-chunks
    NJ = T // 128            # 16 key blocks
    KC = D // 128            # 8 contraction chunks
    Exp = mybir.ActivationFunctionType.Exp

    with tile.TileContext(nc) as tc, ExitStack() as ctx:
        # --- persistent SBUF ---
        wts = ctx.enter_context(tc.tile_pool(name="wts", bufs=1))
        packs = ctx.enter_context(tc.tile_pool(name="packs", bufs=1))
        consts = ctx.enter_context(tc.tile_pool(name="consts", bufs=1))

        # --- working pools (xts first: its loads gate the first matmuls) ---
        xts = ctx.enter_context(tc.tile_pool(name="xts", bufs=2))

        wq_sb = wts.tile([128, KC, 256], bf16, tag="wq")
        wk_sb = wts.tile([128, KC, 256], bf16, tag="wk")
        wv_sb = wts.tile([128, KC, 256], bf16, tag="wv")
        wp_sb = wts.tile([128, 2, 1024], bf16, tag="wp")
        xt_tiles = {}

        def prefetch_xt(tch):
            if tch >= NT or tch in xt_tiles:
                return
            xt = xts.tile([128, KC, 512], bf16, tag="xt", name=f"xt{tch}")
            # split so the first accumulation chunks start sooner
            nsplit = 4 if tch == 0 else 2
            step = KC // nsplit
            for s in range(nsplit):
                nc.sync.dma_start(
                    out=xt[:, s * step:(s + 1) * step, :],
                    in_=xT[s * step:(s + 1) * step, tch].rearrange("k p t -> p k t"))
            xt_tiles[tch] = xt

        cos_sb = consts.tile([128, T], bf16, tag="cos")
        sin_sb = consts.tile([128, T], bf16, tag="sin")
        r2_sb = consts.tile([128, 128], bf16, tag="r2")
        masks_sb = consts.tile([128, 128], bf16, tag="masks")
        # The head is serial-DMA-bound: issue in consumption order across
        # BOTH descriptor generators (sync -> HWDGE, gpsimd -> Pool SWDGE).
        xt0 = xts.tile([128, KC, 512], bf16, tag="xt", name="xt0")
        xt_tiles[0] = xt0
        nc.sync.dma_start(out=wq_sb[:, 0:2, :], in_=wq[:, 0:2, :])
        nc.gpsimd.dma_start(out=r2_sb[:], in_=r2_d[:, :])
        nc.sync.dma_start(out=wq_sb[:, 2:8, :], in_=wq[:, 2:8, :])
        nc.gpsimd.dma_start(out=xt0[:, 4:6, :], in_=xT[4:6, 0].rearrange("k p t -> p k t"))
        nc.sync.dma_start(out=xt0[:, 0:2, :], in_=xT[0:2, 0].rearrange("k p t -> p k t"))
        nc.gpsimd.dma_start(out=xt0[:, 6:8, :], in_=xT[6:8, 0].rearrange("k p t -> p k t"))
        nc.sync.dma_start(out=xt0[:, 2:4, :], in_=xT[2:4, 0].rearrange("k p t -> p k t"))
        nc.sync.dma_start(out=wk_sb[:], in_=wk[:, :, :])
        # wv before cos/sin: the PE's V blocks consume wv directly, while
        # cos/sin only gate the DVE-side rope chain (hidden behind V's mms)
        nc.gpsimd.dma_start(out=wv_sb[:], in_=wv[:, :, :])
        nc.gpsimd.dma_start(out=cos_sb[:], in_=cos_d[:, :])
        nc.gpsimd.dma_start(out=sin_sb[:], in_=sin_d[:, :])
        # xt1 ahead of masks/wp: A(1)'s fill matmuls consume it at ~15us,
        # while the masks gate only B(0)'s Pool muls and wp only C(0) (~40us)
        xt1 = xts.tile([128, KC, 512], bf16, tag="xt", name="xt1")
        xt_tiles[1] = xt1
        for s in range(2):
            nc.gpsimd.dma_start(
                out=xt1[:, 4 * s:4 * (s + 1), :],
                in_=xT[4 * s:4 * (s + 1), 1].rearrange("k p t -> p k t"))
        nc.gpsimd.dma_start(out=masks_sb[:], in_=masks_d[:, :])
        nc.gpsimd.dma_start(out=wp_sb[:], in_=wp[:, :, :])

        # packs: [128, pack p, T] so one rope add can write both packs' slices
        qt_sb = packs.tile([128, 2, T], bf16, tag="qt")
        kt_sb = packs.tile([128, 2, T], bf16, tag="kt")
        ot_sb = packs.tile([128, 2, T], bf16, tag="ot")
        v_sb = packs.tile([128, NJ, HPC * (HD + 1)], bf16, tag="vaug")

        # ones columns of v_aug (fused softmax denominator)
        v_view = v_sb[:].rearrange("p j (h c) -> p j h c", h=HPC)
        nc.vector.memset(v_view[:, :, :, HD:HD + 1], 1.0)
        ones64 = consts.tile([1, 64], bf16, tag="ones64")
        nc.vector.memset(ones64[:], 1.0)
        # dummy activation pulls the Exp table load (1.3us) into the
        # DMA-bound head instead of B(0)'s first softmax
        warm = consts.tile([1, 2], bf16, tag="warm")
        with nc.allow_low_precision(reason="act table warmup"):
            nc.scalar.activation(warm[:], ones64[0:1, 0:2], Exp)

        # --- working pools ---
        tmps = ctx.enter_context(tc.tile_pool(name="tmps", bufs=3))
        pts = ctx.enter_context(tc.tile_pool(name="pts", bufs=8))
        outs = ctx.enter_context(tc.tile_pool(name="outs", bufs=3))
        smalls = ctx.enter_context(tc.tile_pool(name="smalls", bufs=2))

        # 8 PSUM banks: stp pairs 2x2, oacc pair 2, aux (acc/rot/vacc/pc) 2x1
        psSTP = ctx.enter_context(tc.tile_pool(name="psSTP", bufs=2, space="PSUM"))
        psOAC = ctx.enter_context(tc.tile_pool(name="psOAC", bufs=1, space="PSUM"))
        psAUX = ctx.enter_context(tc.tile_pool(name="psAUX", bufs=2, space="PSUM"))

        def emit_rope(tch, raw, dst, p):
            # rope(q) = q*cos + rot(q)*sin; the rotate-half must cross
            # partitions, which only the PE (or DMA) may do on HW, so it is
            # a [128,128] block-diag matmul against the bf16 raw copy.
            ts = slice(tch * 512, (tch + 1) * 512)
            rot = psAUX.tile([128, 512], f32, tag="aux", name="rot")
            nc.tensor.matmul(rot[:], r2_sb[:], raw[:], start=True, stop=True)
            tc_t = tmps.tile([128, 512], bf16, tag="tc", name="tc_t")
            nc.vector.tensor_mul(tc_t[:], raw[:], cos_sb[:, ts])
            ts_t = tmps.tile([128, 512], bf16, tag="ts", name="ts_t")
            with nc.allow_low_precision(reason="bf16 rope"):
                nc.vector.tensor_mul(ts_t[:], rot[:], sin_sb[:, ts])
                nc.vector.tensor_add(dst[:, p, ts], tc_t[:], ts_t[:])

        def emit_o(oacc, p, kj_max, kj, pt, lo):
            for hh in range(2):
                hl = 2 * p + hh
                nc.tensor.matmul(
                    oacc[0:65, 512 * hh + lo:512 * hh + 512],
                    v_sb[:, kj, 65 * hl:65 * hl + 65],
                    pt[:, 512 * hh + lo:512 * hh + 512],
                    start=(kj == 0), stop=(kj == kj_max - 1),
                )

        def attn_step(qi, p, oacc, pending, kj, kj_max):
            """One kj iteration: st pair matmuls -> single exp -> (mask),
            software-pipelined so the PE never queues behind its own exp."""
            r = kj - 4 * qi          # >= 0 on the causal diagonal
            lo = max(r, 0) * 128     # first valid column
            stp = psSTP.tile([128, 1024], f32, tag="stp", name="stp")
            for hh in range(2):
                off = 64 * hh
                nc.tensor.matmul(
                    stp[:, 512 * hh + lo:512 * hh + 512],
                    kt_sb[off:off + 64, p, kj * 128:(kj + 1) * 128],
                    qt_sb[off:off + 64, p, qi * 512 + lo:(qi + 1) * 512],
                    start=True, stop=True,
                )
            pt = pts.tile([128, 1024], bf16, tag="pt", name="pt")
            w = 512 - lo
            st3 = stp[:].rearrange("p (h t) -> p h t", h=2)[:, :, lo:512]
            pt3 = pt[:].rearrange("p (h t) -> p h t", h=2)[:, :, lo:512]
            with nc.allow_low_precision(reason="bf16 softmax weights"):
                nc.scalar.activation(pt3, st3, Exp, scale=0.125)
            if r >= 0:
                meng = nc.gpsimd if (USE_POOL_MASKS and qi < 3) else nc.vector
                for hh in range(2):
                    s = slice(512 * hh + lo, 512 * hh + lo + 128)
                    with nc.allow_low_precision(reason="bf16 mask"):
                        meng.tensor_mul(pt[:, s], pt[:, s], masks_sb[:])
            pending.append((kj, pt, lo))
            # keep the window tight on the very last stream: its PVs gate the
            # final norm -> C(3) tail, so don't let them queue up
            window = 1 if (qi == NT - 1 and p == 1) else 4
            while len(pending) > window:
                kj0, pt0, lo0 = pending.pop(0)
                emit_o(oacc, p, kj_max, kj0, pt0, lo0)

        def attn_norm(qi, p, oacc):
            qs = slice(qi * 512, (qi + 1) * 512)
            if qi == NT - 1 and p == 1:
                # tail stream: C(3) waits on this chain and nothing waits on
                # the oacc banks, so skip oct; emit recips, then pbs, then
                # muls so the per-head chains pipeline across DVE and Pool
                # instead of serializing on the in-order DVE queue
                rcps, bcls = [], []
                for hh in range(2):
                    s = slice(512 * hh, 512 * hh + 512)
                    rcp = smalls.tile([1, 512], bf16, tag="rcp", name="rcp")
                    with nc.allow_low_precision(reason="bf16 softmax recip"):
                        nc.vector.reciprocal(rcp[:], oacc[64:65, s])
                    rcps.append(rcp)
                for hh in range(2):
                    bcl = smalls.tile([64, 512], bf16, tag="bcl", name="bcl")
                    nc.gpsimd.partition_broadcast(bcl[0:64, :], rcps[hh][0:1, :])
                    bcls.append(bcl)
                for hh in range(2):
                    s = slice(512 * hh, 512 * hh + 512)
                    off = 64 * hh
                    with nc.allow_low_precision(reason="bf16 attn out"):
                        nc.vector.tensor_mul(ot_sb[off:off + 64, p, qs],
                                             oacc[0:64, s], bcls[hh][0:64, :])
                return
            # one bf16 copy releases the oacc PSUM pair early (the next
            # stream's PV accumulation is waiting on these 2 banks); the
            # ot mul then has exactly one PSUM operand (bc_ps), which is the
            # HW limit for TensorTensor.
            recip2 = smalls.tile([1, 1024], bf16, tag="recip2", name="recip2")
            with nc.allow_low_precision(reason="bf16 softmax recip"):
                nc.vector.reciprocal(recip2[:], oacc[64:65, :])
            oct = smalls.tile([65, 1024], bf16, tag="oct", name="oct")
            with nc.allow_low_precision(reason="bf16 attn out"):
                nc.vector.tensor_copy(oct[:], oacc[0:65, :])
            # partition-broadcast of the recip row: gpsimd custom op when
            # allowed, else a ones-column matmul on the PE (the only other
            # engine that may cross partitions on HW)
            if USE_POOL_PB:
                bc = smalls.tile([64, 1024], bf16, tag="bc", name="bc")
                for hh in range(2):
                    s = slice(512 * hh, 512 * hh + 512)
                    nc.gpsimd.partition_broadcast(bc[0:64, s], recip2[0:1, s])
                for hh in range(2):
                    off = 64 * hh
                    s = slice(512 * hh, 512 * hh + 512)
                    with nc.allow_low_precision(reason="bf16 attn out"):
                        nc.vector.tensor_mul(ot_sb[off:off + 64, p, qs],
                                             oct[0:64, s], bc[0:64, s])
            else:
                for hh in range(2):
                    s = slice(512 * hh, 512 * hh + 512)
                    bc_ps = psAUX.tile([64, 512], f32, tag="aux", name="bc_ps")
                    nc.tensor.matmul(bc_ps[:], ones64[:], recip2[0:1, s],
                                     start=True, stop=True)
                    off = 64 * hh
                    with nc.allow_low_precision(reason="bf16 attn out"):
                        nc.vector.tensor_mul(ot_sb[off:off + 64, p, qs],
                                             oct[0:64, s], bc_ps[:])

        def emit_rope_pair(raw2, dst):
            """Head-only rope over a pack-pair tile (tch 0): one rot matmul
            in a borrowed stp slot, per-pack DVE muls."""
            ts0 = slice(0, 512)
            rot2 = psSTP.tile([128, 1024], f32, tag="stp", name="rot2")
            for p in range(2):
                s = slice(512 * p, 512 * p + 512)
                nc.tensor.matmul(rot2[:, s], r2_sb[:], raw2[:, s],
                                 start=True, stop=True)
            for p in range(2):
                s = slice(512 * p, 512 * p + 512)
                tc_t = tmps.tile([128, 512], bf16, tag="tc", name="tc_t")
                nc.vector.tensor_mul(tc_t[:], raw2[:, s], cos_sb[:, ts0])
                ts_t = tmps.tile([128, 512], bf16, tag="ts", name="ts_t")
                with nc.allow_low_precision(reason="bf16 rope"):
                    nc.vector.tensor_mul(ts_t[:], rot2[:, s], sin_sb[:, ts0])
                    nc.vector.tensor_add(dst[:, p, ts0], tc_t[:], ts_t[:])

        def a_unit_list(tch):
            """A(tch) as a list of emission closures (proj groups, V blocks).
            The rope skew chains across units via `state`.

            tch 0 is special: B isn't running yet, so the stp PSUM slots are
            free and the two packs of each projection accumulate side by side
            in one 2-bank tile with kc-interleaved matmuls -- every arriving
            xt0 chunk immediately feeds 4 matmuls instead of 2, halving the
            delivery-bound head stall."""
            if tch >= NT:
                return []
            state = {"pend": None}

            def start():
                prefetch_xt(tch)

            def qk_pair_head(w_sb, dst, nm):
                def emit():
                    xt = xt_tiles[0]
                    accp = psSTP.tile([128, 1024], f32, tag="stp",
                                      name=f"accp_{nm}")
                    for kc in range(KC):
                        for p in range(2):
                            nc.tensor.matmul(
                                accp[:, 512 * p:512 * p + 512],
                                w_sb[:, kc, 128 * p:128 * (p + 1)],
                                xt[:, kc, :],
                                start=(kc == 0), stop=(kc == KC - 1),
                            )
                    raw2 = tmps.tile([128, 1024], bf16, tag="raw2", name="raw2")
                    with nc.allow_low_precision(reason="bf16 q/k"):
                        nc.vector.tensor_copy(raw2[:], accp[:])
                    if state["pend"] is not None:
                        state["pend"]()
                    state["pend"] = lambda r=raw2, d=dst: emit_rope_pair(r, d)
                return emit

            def qk_group(w_sb, dst, p):
                def emit():
                    xt = xt_tiles[tch]
                    acc = psAUX.tile([128, 512], f32, tag="aux", name=f"acc{tch}_{p}")
                    for kc in range(KC):
                        nc.tensor.matmul(
                            acc[:],
                            w_sb[:, kc, 128 * p:128 * (p + 1)],
                            xt[:, kc, :],
                            start=(kc == 0), stop=(kc == KC - 1),
                        )
                    raw = tmps.tile([128, 512], bf16, tag="raw", name="raw")
                    ceng = nc.gpsimd if USE_POOL_COPIES else nc.vector
                    with nc.allow_low_precision(reason="bf16 q/k"):
                        ceng.tensor_copy(raw[:], acc[:])
                    if state["pend"] is not None:
                        state["pend"]()
                    state["pend"] = (lambda t=tch, r=raw, d=dst, pp=p:
                                     emit_rope(t, r, d, pp))
                return emit

            def v_block(jb):
                def emit():
                    xt = xt_tiles[tch]
                    jbg = tch * 4 + jb
                    vacc = psAUX.tile([128, 256], f32, tag="aux", name=f"vacc{jbg}")
                    for kc in range(KC):
                        nc.tensor.matmul(
                            vacc[:],
                            xt[:, kc, 128 * jb:128 * (jb + 1)],
                            wv_sb[:, kc, :],
                            start=(kc == 0), stop=(kc == KC - 1),
                        )
                    # flush the pending rope AFTER these matmuls: its rot
                    # matmul waits on a DVE raw copy, and the in-order PE
                    # would head-of-line stall on it otherwise
                    if state["pend"] is not None:
                        state["pend"]()
                        state["pend"] = None
                    veng = nc.gpsimd if USE_POOL_COPIES else nc.vector
                    with nc.allow_low_precision(reason="bf16 v"):
                        veng.tensor_copy(
                            v_view[:, jbg, :, 0:HD],
                            vacc[:].rearrange("p (h c) -> p h c", h=HPC),
                        )
                    if jb == 3:
                        xt_tiles.pop(tch)
                        # prefetch the next chunk only now: issuing it any
                        # earlier puts its transfers ahead of the (critical)
                        # cos/sin/wv const loads in the serial DMA queue
                        prefetch_xt(tch + 1)
                return emit

            units = [start]
            if tch == 0:
                units.append(qk_pair_head(wq_sb, qt_sb, "q"))
                units.append(qk_pair_head(wk_sb, kt_sb, "k"))
            else:
                units.append(qk_group(wq_sb, qt_sb, 0))
                units.append(qk_group(wq_sb, qt_sb, 1))
                units.append(qk_group(wk_sb, kt_sb, 0))
                units.append(qk_group(wk_sb, kt_sb, 1))
            for jb in range(4):
                units.append(v_block(jb))
            return units

        def c_unit_list(tch):
            if tch < 0:
                return []
            ts = slice(tch * 512, (tch + 1) * 512)

            def c_block(ech):
                def emit():
                    pc = psAUX.tile([128, 512], f32, tag="aux", name=f"pc{tch}_{ech}")
                    for kd in range(2):
                        nc.tensor.matmul(
                            pc[:],
                            wp_sb[:, kd, ech * 128:(ech + 1) * 128],
                            ot_sb[:, kd, ts],
                            start=(kd == 0), stop=(kd == 1),
                        )
                    oc = outs.tile([128, 512], bf16, tag="oc", name="oc")
                    oeng = nc.gpsimd if (USE_POOL_COPIES and tch == 0) else nc.vector
                    with nc.allow_low_precision(reason="bf16 partial out"):
                        oeng.tensor_copy(oc[:], pc[:])
                    nc.sync.dma_start(out=yT[ech, tch], in_=oc[:])
                return emit
            return [c_block(e) for e in range(8)]

        def c_final(tch):
            """Tail-specific C: nothing else is left for the PE, so pipeline
            pairs of out-proj blocks through the (now free) 2-bank stp slots,
            alternating the PSUM->SBUF copy between DVE and Pool."""
            ts = slice(tch * 512, (tch + 1) * 512)
            # two waves of two pairs: BOTH in-flight pairs' kd=0 matmuls
            # (which only need ot pack 0, ready mid-B(3)) are emitted before
            # any norm-gated kd=1, so the in-order PE chews through them
            # during the final norm chain
            for wave in range(2):
                tiles = []
                for pair in (2 * wave, 2 * wave + 1):
                    pc2 = psSTP.tile([128, 1024], f32, tag="stp", name=f"pcf{pair}")
                    for half in range(2):
                        ech = 2 * pair + half
                        nc.tensor.matmul(
                            pc2[:, 512 * half:512 * half + 512],
                            wp_sb[:, 0, ech * 128:(ech + 1) * 128],
                            ot_sb[:, 0, ts],
                            start=True, stop=False,
                        )
                    tiles.append((pair, pc2))
                for pair, pc2 in tiles:
                    for half in range(2):
                        ech = 2 * pair + half
                        nc.tensor.matmul(
                            pc2[:, 512 * half:512 * half + 512],
                            wp_sb[:, 1, ech * 128:(ech + 1) * 128],
                            ot_sb[:, 1, ts],
                            start=False, stop=True,
                        )
                    oc2 = outs.tile([128, 1024], bf16, tag="oc2", name="oc2")
                    # split the PSUM->SBUF copy between DVE and the (idle at
                    # the tail) scalar engine so the pipeline is not
                    # copy-throughput-limited
                    with nc.allow_low_precision(reason="bf16 partial out"):
                        nc.scalar.activation(oc2[:, 0:512], pc2[:, 0:512],
                                             mybir.ActivationFunctionType.Copy)
                        nc.vector.tensor_copy(oc2[:, 512:1024], pc2[:, 512:1024])
                    nc.sync.dma_start(out=yT[2 * pair, tch], in_=oc2[:, 0:512])
                    nc.sync.dma_start(out=yT[2 * pair + 1, tch], in_=oc2[:, 512:1024])

        def interleave_fill(a_units, c_units):
            """start + qk units first (rope latency chain), C blocks woven
            between the chunky A units so aux-psum slots alternate."""
            fill = []
            a = list(a_units)
            c = list(c_units)
            if a:
                fill.append(a.pop(0))       # start (DMA prefetch) first
            while a or c:
                if a:
                    fill.append(a.pop(0))
                if c:
                    fill.append(c.pop(0))
            return fill

        def b_emit(qi, fill_units):
            """Emit B(qi)'s attention streams, sprinkling `fill_units`
            (A(qi+1) / C(qi-1) closures) between kj iterations so the PE
            always has independent matmul work queued behind exp waits."""
            kj_max = 4 * (qi + 1)
            n_slots = 2 * (kj_max + 1)
            fill = list(fill_units)
            n_fill = len(fill)
            slot = [0]

            def maybe_fill():
                # spread the n_fill units evenly across the n_slots slots
                k = (n_fill * (slot[0] + 1)) // n_slots - (n_fill * slot[0]) // n_slots
                for _ in range(k):
                    if fill:
                        fill.pop(0)()
                slot[0] += 1

            for p in range(2):
                oacc = psOAC.tile([128, 1024], f32, tag="oaccp", name=f"oacc{qi}_{p}")
                pending = []
                for kj in range(kj_max):
                    attn_step(qi, p, oacc, pending, kj, kj_max)
                    maybe_fill()
                while pending:
                    kj0, pt0, lo0 = pending.pop(0)
                    emit_o(oacc, p, kj_max, kj0, pt0, lo0)
                attn_norm(qi, p, oacc)
                maybe_fill()
            while fill:
                fill.pop(0)()

        # Fill rebalance: the later B(i) streams have the most ACT (exp) work
        # relative to their own PE work, so the out-proj C units are pushed as
        # late as their ot dependencies allow: B(3) gets C(1)+C(2) while the
        # A units go one chunk ahead as before.
        for u in a_unit_list(0):
            u()
        b_emit(0, interleave_fill(a_unit_list(1), []))
        b_emit(1, interleave_fill(a_unit_list(2), []))
        b_emit(2, interleave_fill(a_unit_list(3), c_unit_list(0)))
        b_emit(3, interleave_fill([], c_unit_list(1) + c_unit_list(2)))
        c_final(NT - 1)

    nc.compile()
    return nc


def get_program():
    global _PROGRAM
    if _PROGRAM is None:
        _PROGRAM = build_program()
    return _PROGRAM


def make_in_maps(x, W_qkv, W_proj):
    BF = _bf16()
    x = np.asarray(x, dtype=np.float32)
    W_qkv = np.asarray(W_qkv, dtype=np.float32)
    W_proj = np.asarray(W_proj, dtype=np.float32)
    in_maps = []
    xtr = {}
    for b in range(B):
        xt = x[b].T.reshape(D // 128, 128, T // 512, 512)
        xtr[b] = np.ascontiguousarray(xt.transpose(0, 2, 1, 3)).astype(BF)
    def tile_w(w):  # [D, 256] -> [128, D//128, 256] partition-major
        return np.ascontiguousarray(
            w.reshape(D // 128, 128, w.shape[1]).transpose(1, 0, 2)).astype(BF)

    for core in range(NCORES):
        b, g = divmod(core, 4)
        cs = slice(g * 256, (g + 1) * 256)
        in_maps.append({
            "xT": xtr[b],
            "wq": tile_w(W_qkv[:, 0 * D:1 * D][:, cs]),
            "wk": tile_w(W_qkv[:, 1 * D:2 * D][:, cs]),
            "wv": tile_w(W_qkv[:, 2 * D:3 * D][:, cs]),
            "wp": np.ascontiguousarray(
                W_proj[cs, :].reshape(2, 128, D).transpose(1, 0, 2)).astype(BF),
        })
    return in_maps


def gather_output(results):
    out = np.empty((B, T, D), dtype=np.float32)
    for b in range(B):
        acc = np.asarray(results[4 * b]["yT"]).astype(np.float32)
        for g in range(1, 4):
            acc += np.asarray(results[4 * b + g]["yT"]).astype(np.float32)
        # (ech, tch, p, t) -> yT (D, T) -> transpose to (T, D)
        yt = acc.transpose(0, 2, 1, 3).reshape(D, T)
        out[b] = yt.T
    return out


def kernel(x, W_qkv, W_proj, key_padding_mask=None, **_ignored):
    # key_padding_mask is all-True per the problem spec (fill: ones) -> no-op.
    from concourse.bass_utils import run_bass_kernel_spmd

    nc = get_program()
    in_maps = make_in_maps(x, W_qkv, W_proj)
    res = run_bass_kernel_spmd(nc, in_maps, list(range(NCORES)))
    return gather_output(res.results)


# revision 88
# speedup vs baseline: 1.2894x; 1.0045x over previous
"""Causal self-attention with RoPE on 8 trn2 NeuronCores.

Problem: B=2, T=2048, D=1024, H=16 heads, head_dim=64, fp32 in/out.
Sharding: core = b*4 + g  (data parallel over batch, tensor parallel over
head groups of 4). Each core computes its 4 heads' attention plus the
row-slice of the output projection; the host sums the 4 partial Y^T per
batch (bf16 partials, f32 accumulate) and transposes back.

All matmul inputs are bf16 (same PE rate as f32r at >=256 moving cols, no
4x penalty on the <256-col diagonal tiles, half the DMA bytes, and 2x DVE
throughput on the all-bf16 elementwise ops). PSUM accumulation stays f32.

Per-core dataflow (everything transposed so matmuls contract on partitions):
  xT (1024, 2048)  =  x[b].T
  QT/KT packs: qt_sb[128, 2, T] (pack p = 2 heads of 64 rows)
  RoPE: QT' = QT*cos + (R2 @ QT)*sin   (R2 = block-diag rotate-half matrix)
  V_aug [128, 16, 260]: V natural layout per key block, 4 heads x (64 dims
      + ones column) -> fused softmax denominator.
  S pair tile [keys 128, 2*512] = both heads of a pack in one 2-bank PSUM
      tile; ONE exp activation covers both halves (halves ACT op count).
  P = exp(S * 0.125) -> bf16 SBUF, diag blocks masked on GPSIMD.
  Oacc pair [65, 2*512] += V_aug^T @ P  (row 64 = denominator)
  norm: DVE reciprocal -> GPSIMD partition_broadcast -> DVE mul -> ot bf16
  Y^T partial = Wp-slice^T @ O^T packs -> bf16 DRAM out (SP-queue DMA).

Phases are interleaved: B(i)'s st->exp->PV stream is ACT-latency-bound, so
A(i+1) (projections) and C(i-1) (out-proj) PE work is sprinkled into its
kj slots to keep the PE busy while the scalar engine catches up.
"""

import sys
import numpy as np

sys.path.insert(0, "/opt/trn_rl_repo")

B, T, D, H = 2, 2048, 1024, 16
HD = 64          # head dim
HPC = 4          # heads per core
NCORES = 8
ROPE_BASE = 10000.0

_PROGRAM = None  # cached compiled program

# GPSIMD (Pool-engine) offloads: flipped on only after HW verifier approval,
# since the BIR verifier enforces rules CoreSim does not.
USE_POOL_COPIES = False   # dead: "GPSIMD Instructions cannot access PSUM"
USE_POOL_MASKS = True     # SBUF tensor_mul (causal masks) on gpsimd
USE_POOL_PB = True        # gpsimd partition_broadcast for the softmax recip


def _bf16():
    import ml_dtypes
    return ml_dtypes.bfloat16


def _rope_tables_np():
    inv_freq = 1.0 / (ROPE_BASE ** (np.arange(0, HD, 2, dtype=np.float32) / np.float32(HD)))
    pos = np.arange(T, dtype=np.float32)
    freqs = np.outer(pos, inv_freq).astype(np.float32)          # (T, 32)
    emb = np.concatenate([freqs, freqs], axis=-1)               # (T, 64)
    cosT = np.cos(emb).T.astype(np.float32)                     # (64, T)
    sinT = np.sin(emb).T.astype(np.float32)
    cos2 = np.vstack([cosT, cosT]).copy()                       # (128, T) two heads
    sin2 = np.vstack([sinT, sinT]).copy()
    # sign-folded sin for the matmul-free rotate-half:
    #   rope(q)[d] = q[d]*cos[d] + rot(q)[d]*sin[d],
    #   rot(q)[base+d] = -q[base+32+d] (d<32) ; q[base+d-32] (d>=32)
    # so ts[base+0:32] = raw[base+32:64] * (-sin[base+0:32]) and
    #    ts[base+32:64] = raw[base+0:32] * (+sin[base+32:64]).
    sinadj = sin2.copy()
    for base in (0, 32, 64, 96):
        if (base // 32) % 2 == 0:
            sinadj[base:base + 32] = -sinadj[base:base + 32]
    return cos2, sin2, sinadj


def _r2_np():
    # qrot[d] = -q[d+32] (d<32) ; q[d-32] (d>=32), per 64-row block.
    # matmul computes out[d, t] = sum_k r2[k, d] q[k, t]
    r2 = np.zeros((128, 128), dtype=np.float32)
    for base in (0, 64):
        for d in range(32):
            r2[base + d + 32, base + d] = -1.0
            r2[base + d, base + d + 32] = 1.0
    return r2


def _masks_np():
    # tri[j, ql] = 1 if key j may attend query ql within a diagonal block
    j = np.arange(128)[:, None]
    ql = np.arange(128)[None, :]
    return (j <= ql).astype(np.float32)                         # [128, 128]


def build_program():
    import concourse.bass as bass
    import concourse.tile as tile
    from concourse import bacc, mybir
    from contextlib import ExitStack

    BF = _bf16()
    f32 = mybir.dt.float32
    bf16 = mybir.dt.bfloat16

    nc = bacc.Bacc(None, target_bir_lowering=False, debug=False)

    # xT pre-tiled on host: xTr[kc, tch, p, t] = x[b].T[kc*128+p, tch*512+t]
    xT = nc.declare_dram_parameter("xT", [D // 128, T // 512, 128, 512], bf16, isOutput=False)
    # weights pre-tiled on host to partition-major so each load is one
    # contiguous descriptor per partition (4x fewer descriptors -> 2x faster
    # serial DMA at the head): wq[p, kc, c] = W_qkv[kc*128+p, c]
    wq = nc.declare_dram_parameter("wq", [128, D // 128, 256], bf16, isOutput=False)
    wk = nc.declare_dram_parameter("wk", [128, D // 128, 256], bf16, isOutput=False)
    wv = nc.declare_dram_parameter("wv", [128, D // 128, 256], bf16, isOutput=False)
    wp = nc.declare_dram_parameter("wp", [128, 2, D], bf16, isOutput=False)
    # yT tiled: yTr[ech, tch, p, t] = yT_partial[ech*128+p, tch*512+t], bf16
    yT = nc.declare_dram_parameter("yT", [8, T // 512, 128, 512], bf16, isOutput=True)

    cos2_np, sin2_np, _sinadj_np = _rope_tables_np()
    cos_d = nc.inline_tensor(cos2_np.astype(BF), name="cos2")
    sin_d = nc.inline_tensor(sin2_np.astype(BF), name="sin2")
    r2_d = nc.inline_tensor(_r2_np().astype(BF), name="r2")
    masks_d = nc.inline_tensor(_masks_np().astype(BF), name="masks")

    NT = T // 512            # 4 t-chunks
    NJ = T // 128            # 16 key blocks
    KC = D // 128            # 8 contraction chunks
    Exp = mybir.ActivationFunctionType.Exp

    with tile.TileContext(nc) as tc, ExitStack() as ctx:
        # --- persistent SBUF ---
        wts = ctx.enter_context(tc.tile_pool(name="wts", bufs=1))
        packs = ctx.enter_context(tc.tile_pool(name="packs", bufs=1))
        consts = ctx.enter_context(tc.tile_pool(name="consts", bufs=1))

        # --- working pools (xts first: its loads gate the first matmuls) ---
        xts = ctx.enter_context(tc.tile_pool(name="xts", bufs=2))

        wq_sb = wts.tile([128, KC, 256], bf16, tag="wq")
        wk_sb = wts.tile([128, KC, 256], bf16, tag="wk")
        wv_sb = wts.tile([128, KC, 256], bf16, tag="wv")
        wp_sb = wts.tile([128, 2, 1024], bf16, tag="wp")
        xt_tiles = {}

        def prefetch_xt(tch):
            if tch >= NT or tch in xt_tiles:
                return
            xt = xts.tile([128, KC, 512], bf16, tag="xt", name=f"xt{tch}")
            # split so the first accumulation chunks start sooner
            nsplit = 4 if tch == 0 else 2
            step = KC // nsplit
            for s in range(nsplit):
                nc.sync.dma_start(
                    out=xt[:, s * step:(s + 1) * step, :],
                    in_=xT[s * step:(s + 1) * step, tch].rearrange("k p t -> p k t"))
            xt_tiles[tch] = xt

        cos_sb = consts.tile([128, T], bf16, tag="cos")
        sin_sb = consts.tile([128, T], bf16, tag="sin")
        r2_sb = consts.tile([128, 128], bf16, tag="r2")
        masks_sb = consts.tile([128, 128], bf16, tag="masks")
        # The head is serial-DMA-bound: issue in consumption order across
        # BOTH descriptor generators (sync -> HWDGE, gpsimd -> Pool SWDGE).
        xt0 = xts.tile([128, KC, 512], bf16, tag="xt", name="xt0")
        xt_tiles[0] = xt0
        nc.sync.dma_start(out=wq_sb[:, 0:2, :], in_=wq[:, 0:2, :])
        nc.gpsimd.dma_start(out=r2_sb[:], in_=r2_d[:, :])
        nc.sync.dma_start(out=wq_sb[:, 2:8, :], in_=wq[:, 2:8, :])
        nc.gpsimd.dma_start(out=xt0[:, 4:6, :], in_=xT[4:6, 0].rearrange("k p t -> p k t"))
        nc.sync.dma_start(out=xt0[:, 0:2, :], in_=xT[0:2, 0].rearrange("k p t -> p k t"))
        nc.gpsimd.dma_start(out=xt0[:, 6:8, :], in_=xT[6:8, 0].rearrange("k p t -> p k t"))
        nc.sync.dma_start(out=xt0[:, 2:4, :], in_=xT[2:4, 0].rearrange("k p t -> p k t"))
        nc.sync.dma_start(out=wk_sb[:], in_=wk[:, :, :])
        # wv before cos/sin: the PE's V blocks consume wv directly, while
        # cos/sin only gate the DVE-side rope chain (hidden behind V's mms)
        nc.gpsimd.dma_start(out=wv_sb[:], in_=wv[:, :, :])
        nc.gpsimd.dma_start(out=cos_sb[:], in_=cos_d[:, :])
        nc.gpsimd.dma_start(out=sin_sb[:], in_=sin_d[:, :])
        # xt1 ahead of masks/wp: A(1)'s fill matmuls consume it at ~15us,
        # while the masks gate only B(0)'s Pool muls and wp only C(0) (~40us)
        xt1 = xts.tile([128, KC, 512], bf16, tag="xt", name="xt1")
        xt_tiles[1] = xt1
        for s in range(2):
            nc.gpsimd.dma_start(
                out=xt1[:, 4 * s:4 * (s + 1), :],
                in_=xT[4 * s:4 * (s + 1), 1].rearrange("k p t -> p k t"))
        nc.gpsimd.dma_start(out=masks_sb[:], in_=masks_d[:, :])
        nc.gpsimd.dma_start(out=wp_sb[:], in_=wp[:, :, :])

        # packs: [128, pack p, T] so one rope add can write both packs' slices
        qt_sb = packs.tile([128, 2, T], bf16, tag="qt")
        kt_sb = packs.tile([128, 2, T], bf16, tag="kt")
        ot_sb = packs.tile([128, 2, T], bf16, tag="ot")
        v_sb = packs.tile([128, NJ, HPC * (HD + 1)], bf16, tag="vaug")

        # ones columns of v_aug (fused softmax denominator)
        v_view = v_sb[:].rearrange("p j (h c) -> p j h c", h=HPC)
        nc.vector.memset(v_view[:, :, :, HD:HD + 1], 1.0)
        ones64 = consts.tile([1, 64], bf16, tag="ones64")
        nc.vector.memset(ones64[:], 1.0)
        # dummy activation pulls the Exp table load (1.3us) into the
        # DMA-bound head instead of B(0)'s first softmax
        warm = consts.tile([1, 2], bf16, tag="warm")
        with nc.allow_low_precision(reason="act table warmup"):
            nc.scalar.activation(warm[:], ones64[0:1, 0:2], Exp)

        # --- working pools ---
        tmps = ctx.enter_context(tc.tile_pool(name="tmps", bufs=3))
        pts = ctx.enter_context(tc.tile_pool(name="pts", bufs=8))
        outs = ctx.enter_context(tc.tile_pool(name="outs", bufs=3))
        smalls = ctx.enter_context(tc.tile_pool(name="smalls", bufs=2))

        # 8 PSUM banks: stp pairs 2x2, oacc pair 2, aux (acc/rot/vacc/pc) 2x1
        psSTP = ctx.enter_context(tc.tile_pool(name="psSTP", bufs=2, space="PSUM"))
        psOAC = ctx.enter_context(tc.tile_pool(name="psOAC", bufs=1, space="PSUM"))
        psAUX = ctx.enter_context(tc.tile_pool(name="psAUX", bufs=2, space="PSUM"))

        def emit_rope(tch, raw, dst, p):
            # rope(q) = q*cos + rot(q)*sin; the rotate-half must cross
            # partitions, which only the PE (or DMA) may do on HW, so it is
            # a [128,128] block-diag matmul against the bf16 raw copy.
            ts = slice(tch * 512, (tch + 1) * 512)
            rot = psAUX.tile([128, 512], f32, tag="aux", name="rot")
            nc.tensor.matmul(rot[:], r2_sb[:], raw[:], start=True, stop=True)
            tc_t = tmps.tile([128, 512], bf16, tag="tc", name="tc_t")
            nc.vector.tensor_mul(tc_t[:], raw[:], cos_sb[:, ts])
            ts_t = tmps.tile([128, 512], bf16, tag="ts", name="ts_t")
            with nc.allow_low_precision(reason="bf16 rope"):
                nc.vector.tensor_mul(ts_t[:], rot[:], sin_sb[:, ts])
                nc.vector.tensor_add(dst[:, p, ts], tc_t[:], ts_t[:])

        def emit_o(oacc, p, kj_max, kj, pt, lo):
            for hh in range(2):
                hl = 2 * p + hh
                nc.tensor.matmul(
                    oacc[0:65, 512 * hh + lo:512 * hh + 512],
                    v_sb[:, kj, 65 * hl:65 * hl + 65],
                    pt[:, 512 * hh + lo:512 * hh + 512],
                    start=(kj == 0), stop=(kj == kj_max - 1),
                )

        def attn_step(qi, p, oacc, pending, kj, kj_max):
            """One kj iteration: st pair matmuls -> single exp -> (mask),
            software-pipelined so the PE never queues behind its own exp."""
            r = kj - 4 * qi          # >= 0 on the causal diagonal
            lo = max(r, 0) * 128     # first valid column
            stp = psSTP.tile([128, 1024], f32, tag="stp", name="stp")
            for hh in range(2):
                off = 64 * hh
                nc.tensor.matmul(
                    stp[:, 512 * hh + lo:512 * hh + 512],
                    kt_sb[off:off + 64, p, kj * 128:(kj + 1) * 128],
                    qt_sb[off:off + 64, p, qi * 512 + lo:(qi + 1) * 512],
                    start=True, stop=True,
                )
            pt = pts.tile([128, 1024], bf16, tag="pt", name="pt")
            w = 512 - lo
            st3 = stp[:].rearrange("p (h t) -> p h t", h=2)[:, :, lo:512]
            pt3 = pt[:].rearrange("p (h t) -> p h t", h=2)[:, :, lo:512]
            with nc.allow_low_precision(reason="bf16 softmax weights"):
                nc.scalar.activation(pt3, st3, Exp, scale=0.125)
            if r >= 0:
                meng = nc.gpsimd if (USE_POOL_MASKS and qi < 3) else nc.vector
                for hh in range(2):
                    s = slice(512 * hh + lo, 512 * hh + lo + 128)
                    with nc.allow_low_precision(reason="bf16 mask"):
                        meng.tensor_mul(pt[:, s], pt[:, s], masks_sb[:])
            pending.append((kj, pt, lo))
            # keep the window tight on the very last stream: its PVs gate the
            # final norm -> C(3) tail, so don't let them queue up
            window = 1 if (qi == NT - 1 and p == 1) else 4
            while len(pending) > window:
                kj0, pt0, lo0 = pending.pop(0)
                emit_o(oacc, p, kj_max, kj0, pt0, lo0)

        def attn_norm(qi, p, oacc):
            qs = slice(qi * 512, (qi + 1) * 512)
            if qi == NT - 1 and p == 1:
                # tail stream: C(3) waits on this chain and nothing waits on
                # the oacc banks, so skip oct; emit recips, then pbs, then
                # muls so the per-head chains pipeline across DVE and Pool
                # instead of serializing on the in-order DVE queue
                rcps, bcls = [], []
                for hh in range(2):
                    s = slice(512 * hh, 512 * hh + 512)
                    rcp = smalls.tile([1, 512], bf16, tag="rcp", name="rcp")
                    with nc.allow_low_precision(reason="bf16 softmax recip"):
                        nc.vector.reciprocal(rcp[:], oacc[64:65, s])
                    rcps.append(rcp)
                for hh in range(2):
                    bcl = smalls.tile([64, 512], bf16, tag="bcl", name="bcl")
                    nc.gpsimd.partition_broadcast(bcl[0:64, :], rcps[hh][0:1, :])
                    bcls.append(bcl)
                for hh in range(2):
                    s = slice(512 * hh, 512 * hh + 512)
                    off = 64 * hh
                    with nc.allow_low_precision(reason="bf16 attn out"):
                        nc.vector.tensor_mul(ot_sb[off:off + 64, p, qs],
                                             oacc[0:64, s], bcls[hh][0:64, :])
                return
            # one bf16 copy releases the oacc PSUM pair early (the next
            # stream's PV accumulation is waiting on these 2 banks); the
            # ot mul then has exactly one PSUM operand (bc_ps), which is the
            # HW limit for TensorTensor.
            recip2 = smalls.tile([1, 1024], bf16, tag="recip2", name="recip2")
            with nc.allow_low_precision(reason="bf16 softmax recip"):
                nc.vector.reciprocal(recip2[:], oacc[64:65, :])
            oct = smalls.tile([65, 1024], bf16, tag="oct", name="oct")
            with nc.allow_low_precision(reason="bf16 attn out"):
                nc.vector.tensor_copy(oct[:], oacc[0:65, :])
            # partition-broadcast of the recip row: gpsimd custom op when
            # allowed, else a ones-column matmul on the PE (the only other
            # engine that may cross partitions on HW)
            if USE_POOL_PB:
                bc = smalls.tile([64, 1024], bf16, tag="bc", name="bc")
                for hh in range(2):
                    s = slice(512 * hh, 512 * hh + 512)
                    nc.gpsimd.partition_broadcast(bc[0:64, s], recip2[0:1, s])
                for hh in range(2):
                    off = 64 * hh
                    s = slice(512 * hh, 512 * hh + 512)
                    with nc.allow_low_precision(reason="bf16 attn out"):
                        nc.vector.tensor_mul(ot_sb[off:off + 64, p, qs],
                                             oct[0:64, s], bc[0:64, s])
            else:
                for hh in range(2):
                    s = slice(512 * hh, 512 * hh + 512)
                    bc_ps = psAUX.tile([64, 512], f32, tag="aux", name="bc_ps")
                    nc.tensor.matmul(bc_ps[:], ones64[:], recip2[0:1, s],
                                     start=True, stop=True)
                    off = 64 * hh
                    with nc.allow_low_precision(reason="bf16 attn out"):
                        nc.vector.tensor_mul(ot_sb[off:off + 64, p, qs],
                                             oct[0:64, s], bc_ps[:])

        def emit_rope_half(raw2, dst, p):
            """Head-only rope for ONE pack of a pair tile (tch 0).  Halves
            are emitted q0, k0, q1, k1 so B(0)'s first matmuls (which need
            only the p0 halves) aren't gated by the full serial DVE chain."""
            ts0 = slice(0, 512)
            s = slice(512 * p, 512 * p + 512)
            rot = psSTP.tile([128, 1024], f32, tag="stp", name="roth")
            nc.tensor.matmul(rot[:, 0:512], r2_sb[:], raw2[:, s],
                             start=True, stop=True)
            tc_t = tmps.tile([128, 512], bf16, tag="tc", name="tc_t")
            nc.vector.tensor_mul(tc_t[:], raw2[:, s], cos_sb[:, ts0])
            ts_t = tmps.tile([128, 512], bf16, tag="ts", name="ts_t")
            with nc.allow_low_precision(reason="bf16 rope"):
                nc.vector.tensor_mul(ts_t[:], rot[:, 0:512], sin_sb[:, ts0])
                nc.vector.tensor_add(dst[:, p, ts0], tc_t[:], ts_t[:])

        def a_unit_list(tch):
            """A(tch) as a list of emission closures (proj groups, V blocks).
            The rope skew chains across units via `state`.

            tch 0 is special: B isn't running yet, so the stp PSUM slots are
            free and the two packs of each projection accumulate side by side
            in one 2-bank tile with kc-interleaved matmuls -- every arriving
            xt0 chunk immediately feeds 4 matmuls instead of 2, halving the
            delivery-bound head stall."""
            if tch >= NT:
                return []
            state = {"pend": None, "pendq": []}

            def start():
                prefetch_xt(tch)

            def qk_pair_head(w_sb, dst, nm):
                def emit():
                    xt = xt_tiles[0]
                    accp = psSTP.tile([128, 1024], f32, tag="stp",
                                      name=f"accp_{nm}")
                    for kc in range(KC):
                        for p in range(2):
                            nc.tensor.matmul(
                                accp[:, 512 * p:512 * p + 512],
                                w_sb[:, kc, 128 * p:128 * (p + 1)],
                                xt[:, kc, :],
                                start=(kc == 0), stop=(kc == KC - 1),
                            )
                    raw2 = tmps.tile([128, 1024], bf16, tag="raw2", name="raw2")
                    with nc.allow_low_precision(reason="bf16 q/k"):
                        nc.vector.tensor_copy(raw2[:], accp[:])
                    halves = [
                        (lambda r=raw2, d=dst, pp=pp2: emit_rope_half(r, d, pp))
                        for pp2 in range(2)]
                    if state["pendq"]:
                        # k-unit: emit q-p0 now, weave k halves around q-p1
                        state["pendq"].pop(0)()
                        state["pendq"] = ([halves[0]] + state["pendq"]
                                          + [halves[1]])
                    else:
                        state["pendq"] = halves
                return emit

            def qk_group(w_sb, dst, p):
                def emit():
                    xt = xt_tiles[tch]
                    acc = psAUX.tile([128, 512], f32, tag="aux", name=f"acc{tch}_{p}")
                    for kc in range(KC):
                        nc.tensor.matmul(
                            acc[:],
                            w_sb[:, kc, 128 * p:128 * (p + 1)],
                            xt[:, kc, :],
                            start=(kc == 0), stop=(kc == KC - 1),
                        )
                    raw = tmps.tile([128, 512], bf16, tag="raw", name="raw")
                    ceng = nc.gpsimd if USE_POOL_COPIES else nc.vector
                    with nc.allow_low_precision(reason="bf16 q/k"):
                        ceng.tensor_copy(raw[:], acc[:])
                    if state["pend"] is not None:
                        state["pend"]()
                    state["pend"] = (lambda t=tch, r=raw, d=dst, pp=p:
                                     emit_rope(t, r, d, pp))
                return emit

            def v_block(jb):
                def emit():
                    xt = xt_tiles[tch]
                    jbg = tch * 4 + jb
                    vacc = psAUX.tile([128, 256], f32, tag="aux", name=f"vacc{jbg}")
                    for kc in range(KC):
                        nc.tensor.matmul(
                            vacc[:],
                            xt[:, kc, 128 * jb:128 * (jb + 1)],
                            wv_sb[:, kc, :],
                            start=(kc == 0), stop=(kc == KC - 1),
                        )
                    # flush the pending rope AFTER these matmuls: its rot
                    # matmul waits on a DVE raw copy, and the in-order PE
                    # would head-of-line stall on it otherwise
                    if state["pend"] is not None:
                        state["pend"]()
                        state["pend"] = None
                    elif state["pendq"]:
                        state["pendq"].pop(0)()
                    veng = nc.gpsimd if USE_POOL_COPIES else nc.vector
                    with nc.allow_low_precision(reason="bf16 v"):
                        veng.tensor_copy(
                            v_view[:, jbg, :, 0:HD],
                            vacc[:].rearrange("p (h c) -> p h c", h=HPC),
                        )
                    if jb == 3:
                        xt_tiles.pop(tch)
                        # prefetch the next chunk only now: issuing it any
                        # earlier puts its transfers ahead of the (critical)
                        # cos/sin/wv const loads in the serial DMA queue
                        prefetch_xt(tch + 1)
                return emit

            units = [start]
            if tch == 0:
                units.append(qk_pair_head(wq_sb, qt_sb, "q"))
                units.append(qk_pair_head(wk_sb, kt_sb, "k"))
            else:
                units.append(qk_group(wq_sb, qt_sb, 0))
                units.append(qk_group(wq_sb, qt_sb, 1))
                units.append(qk_group(wk_sb, kt_sb, 0))
                units.append(qk_group(wk_sb, kt_sb, 1))
            for jb in range(4):
                units.append(v_block(jb))
            return units

        def c_unit_list(tch):
            if tch < 0:
                return []
            ts = slice(tch * 512, (tch + 1) * 512)

            def c_block(ech):
                def emit():
                    pc = psAUX.tile([128, 512], f32, tag="aux", name=f"pc{tch}_{ech}")
                    for kd in range(2):
                        nc.tensor.matmul(
                            pc[:],
                            wp_sb[:, kd, ech * 128:(ech + 1) * 128],
                            ot_sb[:, kd, ts],
                            start=(kd == 0), stop=(kd == 1),
                        )
                    oc = outs.tile([128, 512], bf16, tag="oc", name="oc")
                    oeng = nc.gpsimd if (USE_POOL_COPIES and tch == 0) else nc.vector
                    with nc.allow_low_precision(reason="bf16 partial out"):
                        oeng.tensor_copy(oc[:], pc[:])
                    nc.sync.dma_start(out=yT[ech, tch], in_=oc[:])
                return emit
            return [c_block(e) for e in range(8)]

        def c_final(tch):
            """Tail-specific C: nothing else is left for the PE, so pipeline
            pairs of out-proj blocks through the (now free) 2-bank stp slots,
            alternating the PSUM->SBUF copy between DVE and Pool."""
            ts = slice(tch * 512, (tch + 1) * 512)
            # two waves of two pairs: BOTH in-flight pairs' kd=0 matmuls
            # (which only need ot pack 0, ready mid-B(3)) are emitted before
            # any norm-gated kd=1, so the in-order PE chews through them
            # during the final norm chain
            for wave in range(2):
                tiles = []
                for pair in (2 * wave, 2 * wave + 1):
                    pc2 = psSTP.tile([128, 1024], f32, tag="stp", name=f"pcf{pair}")
                    for half in range(2):
                        ech = 2 * pair + half
                        nc.tensor.matmul(
                            pc2[:, 512 * half:512 * half + 512],
                            wp_sb[:, 0, ech * 128:(ech + 1) * 128],
                            ot_sb[:, 0, ts],
                            start=True, stop=False,
                        )
                    tiles.append((pair, pc2))
                for pair, pc2 in tiles:
                    for half in range(2):
                        ech = 2 * pair + half
                        nc.tensor.matmul(
                            pc2[:, 512 * half:512 * half + 512],
                            wp_sb[:, 1, ech * 128:(ech + 1) * 128],
                            ot_sb[:, 1, ts],
                            start=False, stop=True,
                        )
                    oc2 = outs.tile([128, 1024], bf16, tag="oc2", name="oc2")
                    # split the PSUM->SBUF copy between DVE and the (idle at
                    # the tail) scalar engine so the pipeline is not
                    # copy-throughput-limited
                    with nc.allow_low_precision(reason="bf16 partial out"):
                        nc.scalar.activation(oc2[:, 0:512], pc2[:, 0:512],
                                             mybir.ActivationFunctionType.Copy)
                        nc.vector.tensor_copy(oc2[:, 512:1024], pc2[:, 512:1024])
                    nc.sync.dma_start(out=yT[2 * pair, tch], in_=oc2[:, 0:512])
                    nc.sync.dma_start(out=yT[2 * pair + 1, tch], in_=oc2[:, 512:1024])

        def interleave_fill(a_units, c_units):
            """start + qk units first (rope latency chain), C blocks woven
            between the chunky A units so aux-psum slots alternate."""
            fill = []
            a = list(a_units)
            c = list(c_units)
            if a:
                fill.append(a.pop(0))       # start (DMA prefetch) first
            while a or c:
                if a:
                    fill.append(a.pop(0))
                if c:
                    fill.append(c.pop(0))
            return fill

        def b_emit(qi, fill_units):
            """Emit B(qi)'s attention streams, sprinkling `fill_units`
            (A(qi+1) / C(qi-1) closures) between kj iterations so the PE
            always has independent matmul work queued behind exp waits."""
            kj_max = 4 * (qi + 1)
            n_slots = 2 * (kj_max + 1)
            fill = list(fill_units)
            n_fill = len(fill)
            slot = [0]

            def maybe_fill():
                # spread the n_fill units evenly across the n_slots slots
                k = (n_fill * (slot[0] + 1)) // n_slots - (n_fill * slot[0]) // n_slots
                for _ in range(k):
                    if fill:
                        fill.pop(0)()
                slot[0] += 1

            for p in range(2):
                oacc = psOAC.tile([128, 1024], f32, tag="oaccp", name=f"oacc{qi}_{p}")
                pending = []
                for kj in range(kj_max):
                    attn_step(qi, p, oacc, pending, kj, kj_max)
                    maybe_fill()
                while pending:
                    kj0, pt0, lo0 = pending.pop(0)
                    emit_o(oacc, p, kj_max, kj0, pt0, lo0)
                attn_norm(qi, p, oacc)
                maybe_fill()
            while fill:
                fill.pop(0)()

        # Fill rebalance: the later B(i) streams have the most ACT (exp) work
        # relative to their own PE work, so the out-proj C units are pushed as
        # late as their ot dependencies allow: B(3) gets C(1)+C(2) while the
        # A units go one chunk ahead as before.
        for u in a_unit_list(0):
            u()
        b_emit(0, interleave_fill(a_unit_list(1), []))
        b_emit(1, interleave_fill(a_unit_list(2), []))
        b_emit(2, interleave_fill(a_unit_list(3), c_unit_list(0)))
        b_emit(3, interleave_fill([], c_unit_list(1) + c_unit_list(2)))
        c_final(NT - 1)

    nc.compile()
    return nc


def get_program():
    global _PROGRAM
    if _PROGRAM is None:
        _PROGRAM = build_program()
    return _PROGRAM


def make_in_maps(x, W_qkv, W_proj):
    BF = _bf16()
    x = np.asarray(x, dtype=np.float32)
    W_qkv = np.asarray(W_qkv, dtype=np.float32)
    W_proj = np.asarray(W_proj, dtype=np.float32)
    in_maps = []
    xtr = {}
    for b in range(B):
        xt = x[b].T.reshape(D // 128, 128, T // 512, 512)
        xtr[b] = np.ascontiguousarray(xt.transpose(0, 2, 1, 3)).astype(BF)
    def tile_w(w):  # [D, 256] -> [128, D//128, 256] partition-major
        return np.ascontiguousarray(
            w.reshape(D // 128, 128, w.shape[1]).transpose(1, 0, 2)).astype(BF)

    for core in range(NCORES):
        b, g = divmod(core, 4)
        cs = slice(g * 256, (g + 1) * 256)
        in_maps.append({
            "xT": xtr[b],
            "wq": tile_w(W_qkv[:, 0 * D:1 * D][:, cs]),
            "wk": tile_w(W_qkv[:, 1 * D:2 * D][:, cs]),
            "wv": tile_w(W_qkv[:, 2 * D:3 * D][:, cs]),
            "wp": np.ascontiguousarray(
                W_proj[cs, :].reshape(2, 128, D).transpose(1, 0, 2)).astype(BF),
        })
    return in_maps


def gather_output(results):
    out = np.empty((B, T, D), dtype=np.float32)
    for b in range(B):
        acc = np.asarray(results[4 * b]["yT"]).astype(np.float32)
        for g in range(1, 4):
            acc += np.asarray(results[4 * b + g]["yT"]).astype(np.float32)
        # (ech, tch, p, t) -> yT (D, T) -> transpose to (T, D)
        yt = acc.transpose(0, 2, 1, 3).reshape(D, T)
        out[b] = yt.T
    return out


def kernel(x, W_qkv, W_proj, key_padding_mask=None, **_ignored):
    # key_padding_mask is all-True per the problem spec (fill: ones) -> no-op.
    from concourse.bass_utils import run_bass_kernel_spmd

    nc = get_program()
    in_maps = make_in_maps(x, W_qkv, W_proj)
    res = run_bass_kernel_spmd(nc, in_maps, list(range(NCORES)))
    return gather_output(res.results)
